# revision 22
# baseline (speedup 1.0000x reference)
"""Trainium2 Bass kernel for MultiHeadLatentAttention (MLA), 8-core SPMD.

Sharding: data-parallel over batch (4) x tensor-parallel over heads (2).
Core c handles batch c//2 and heads (c%2)*8 .. +8. Each core computes its
partial output projection; the host sums the two TP partials per batch and
adds the (v-bias-folded) output bias.

Device layout is feature-on-partition / token-on-free throughout, so every
projection is a plain matmul chain with no transposes. Attention uses
transposed scores (keys on partitions) so probs feed the AV matmul directly.

v2 notes (vs the 445us baseline):
- phase order qd -> allgather -> kv -> kNope -> q-rms -> v -> qu so the
  collective and both RMS latency chains hide under independent PE work
- rsqrt via exp(-0.5*ln(x)) on the scalar engine and softmax 1/den via the
  custom-DVE reciprocal_approx_fast: one activation table set, no 6.5us
  DVE reciprocals on the critical path
- per-(kc,qt) score PSUM holds BOTH heads of a pair -> single batched exp
- head-pair software pipelining: scores(hp) overlap den/outU/norm(hp-1)
- v-up bias matmuls removed (exact host fold: bo += Wo @ bkvu_v)
- sum-of-squares matmuls in bf16; 128x128 universal triangle mask
"""

import sys
from contextlib import ExitStack

import numpy as np
import ml_dtypes

for _p in ("/opt/trn_rl_repo", "/root/.axon_site/_ro/trn_rl_repo"):
    if _p not in sys.path:
        sys.path.append(_p)

import concourse.bass as bass  # noqa: E402
import concourse.mybir as mybir  # noqa: E402
from concourse import bacc  # noqa: E402
from concourse.bass_utils import run_bass_kernel_spmd  # noqa: E402
from concourse.tile import TileContext  # noqa: E402

# Problem shapes (hardcoded per contract)
B, S, D = 4, 1024, 2048
H = 16
QL, KVL = 1536, 512
NOPE, ROPE, VH = 128, 64, 128
QKH = NOPE + ROPE  # 192
EPS = 1e-6

P = 128
T = S          # tokens per core (one batch)
DC = D // P    # 16 X chunks
QC = QL // P   # 12 q-latent chunks
KC = KVL // P  # 4 kv-latent chunks
HH = H // 2    # 8 heads per core
NKV = KVL + ROPE  # 576
NEG = -1.0e4   # mask bias (exp underflows to exactly 0)

f32 = mybir.dt.float32
bf16 = mybir.dt.bfloat16
AF = mybir.ActivationFunctionType


def build_nc(start: int):
    nc = bacc.Bacc(None, target_bir_lowering=False, debug=False)

    dp = nc.declare_dram_parameter
    xt = dp("xt", [D, T], bf16, isOutput=False)           # X[b].T
    wqdl = dp("wqdl", [D, 6 * P], bf16, isOutput=False)   # local qd W.T
    wkvd = dp("wkvd", [D, 5 * P], bf16, isOutput=False)   # kv down W.T (padded)
    wqu = dp("wqu", [QL, QL], bf16, isOutput=False)        # (perm q) Wqu_eff.T
    wkvu = dp("wkvu", [KVL, 2048], bf16, isOutput=False)   # (perm kv) Wkvu_eff.T
    wo = dp("wo", [HH * VH, D], bf16, isOutput=False)     # Wo[:, slice].T
    bql_i = dp("bql", [P, 6], f32, isOutput=False)        # local qd bias
    bkvd_i = dp("bkvd", [P, 5], f32, isOutput=False)      # kv down bias
    bqu_i = dp("bqu", [P, QC], f32, isOutput=False)       # perm + scale
    bkvuk = dp("bkvuk", [P, HH], f32, isOutput=False)     # kNope part
    cos2 = dp("cos2", [P, T], f32, isOutput=False)        # duplicated rows
    sina = dp("sina", [P, T], f32, isOutput=False)        # sign-folded sin
    tri_i = dp("tri", [P, P], bf16, isOutput=False)       # diagonal-band mask
    outt = dp("outt", [D, T], f32, isOutput=True)

    cc_in = nc.dram_tensor("cc_in", [6 * P, T], bf16)
    cc_out = nc.dram_tensor("cc_out", [12 * P, T], bf16)
    RG = [[0, 1], [2, 3], [4, 5], [6, 7]]

    xt_r = xt.rearrange("(c p) t -> p c t", p=P)
    wqdl_r = wqdl.rearrange("(c p) m -> p c m", p=P)
    wkvd_r = wkvd.rearrange("(c p) m -> p c m", p=P)
    wqu_r = wqu.rearrange("(c p) m -> p c m", p=P)
    wkvu_r = wkvu.rearrange("(c p) m -> p c m", p=P)
    wo_r = wo.rearrange("(c p) m -> p c m", p=P)
    outt_r = outt.rearrange("(c p) t -> p c t", p=P)
    cc_in_r = cc_in.rearrange("(c p) t -> p c t", p=P)
    cc_out_r = cc_out.rearrange("(c p) t -> p c t", p=P)

    with TileContext(nc) as tc, ExitStack() as stk:
        const = stk.enter_context(tc.tile_pool(name="const", bufs=1))
        persist = stk.enter_context(tc.tile_pool(name="persist", bufs=1))

        # ---- constants in SBUF (X goes first; see phase 1) ----
        c_bql = const.tile([P, 6], f32)
        c_bkvd = const.tile([P, 5], f32)
        c_bqu = const.tile([P, QC], f32)
        c_bkvuk = const.tile([P, HH], f32)
        c_tri = const.tile([P, P], bf16)
        c_cos = const.tile([P, T], f32)
        c_sin = const.tile([P, T], f32)
        ones_bf = const.tile([P, P], bf16)
        nc.vector.memset(ones_bf[:], 1.0)
        eps_c = const.tile([P, 1], f32)
        nc.vector.memset(eps_c[:], EPS)

        # ---- persistent activations ----
        t_q = persist.tile([P, QC, T], bf16)      # q heads (nope 0-7, rope 8-11)
        t_kn = persist.tile([P, HH, T], bf16)     # kNope[feat, head, tok]
        t_v = persist.tile([P, T // P, HH * P], bf16)  # v[tok, tokchunk, hv]
        t_kr = persist.tile([P, T], bf16)         # kRot, rows duplicated
        t_ao = persist.tile([P, HH, T], bf16)     # attn out [vh, head, tok]
        rq = persist.tile([P, T], f32)            # q rms scale (per token)
        rkv = persist.tile([P, T], f32)           # kv rms scale

        # ====== phases 1+2: projections ======
        with tc.tile_pool(name="ph1", bufs=1) as ph1, \
             tc.tile_pool(name="wstream", bufs=2) as wst, \
             tc.tile_pool(name="wqu_p", bufs=2) as wqp, \
             tc.tile_pool(name="wkvu_p", bufs=2) as wkp, \
             tc.tile_pool(name="tmp", bufs=2) as tmp, \
             tc.tile_pool(name="psA", bufs=6, space="PSUM") as psA, \
             tc.tile_pool(name="psR", bufs=1, space="PSUM") as psR:

            # first two qd weights lead, then X split over sync/scalar with
            # only 1MB on gpsimd: the gpsimd queue must stay light so the
            # collective (stores -> AllGather -> readback) runs early
            t_x = ph1.tile([P, DC, T], bf16, name="t_x")
            w_qd = []
            for m in range(6):
                w_qd.append(wst.tile([P, DC, P], bf16, tag="wqd", bufs=4,
                                     name="w_qd"))
            nc.scalar.dma_start(w_qd[0][:], wqdl_r[:, :, bass.ts(0, P)])
            nc.sync.dma_start(w_qd[1][:], wqdl_r[:, :, bass.ts(1, P)])
            nc.sync.dma_start(t_x[:, 0:6, :], xt_r[:, 0:6, :])
            nc.scalar.dma_start(t_x[:, 6:12, :], xt_r[:, 6:12, :])
            nc.gpsimd.dma_start(t_x[:, 12:16, :], xt_r[:, 12:16, :])
            nc.scalar.dma_start(w_qd[2][:], wqdl_r[:, :, bass.ts(2, P)])
            nc.sync.dma_start(w_qd[3][:], wqdl_r[:, :, bass.ts(3, P)])
            nc.gpsimd.dma_start(c_bql[:], bql_i[:])
            nc.gpsimd.dma_start(c_bkvd[:], bkvd_i[:])
            nc.gpsimd.dma_start(c_bqu[:], bqu_i[:])
            nc.gpsimd.dma_start(c_bkvuk[:], bkvuk[:])
            nc.gpsimd.dma_start(c_tri[:], tri_i[:])
            t_kv = ph1.tile([P, 5, T], bf16, name="t_kv")
            t_qd = ph1.tile([P, QC, T], bf16, name="t_qd")

            def down_chain(wt, m_rows, bias_t, bcol, out_ap):
                # out[m_rows, T] = wt.T @ X + bias, as 2 half-token chains
                for tt in range(2):
                    ps = psA.tile([P, 512], f32, tag="ev", name="ps_ev")
                    psm = ps[:m_rows, :]
                    for c in range(DC):
                        nc.tensor.matmul(
                            psm, wt[:, c, :m_rows],
                            t_x[:, c, bass.ts(tt, 512)],
                            start=(c == 0), stop=(c == DC - 1),
                        )
                    nc.vector.tensor_scalar_add(
                        out=out_ap[:m_rows, bass.ts(tt, 512)], in0=psm,
                        scalar1=bias_t[:m_rows, bcol:bcol + 1])

            # ---- q down: local 6 chunks -> exchange -> full 12 in t_qd ----
            # The AllGather output is in global QL order [g0 | g1], so
            # overwriting ALL of t_qd with cc_out leaves every core with the
            # naturally-ordered full latent regardless of its group. The
            # gpsimd queue serializes stores -> collective -> readback.
            for m in range(6):
                if m >= 4:
                    eng = nc.scalar if m % 2 == 0 else nc.sync
                    eng.dma_start(w_qd[m][:], wqdl_r[:, :, bass.ts(m, P)])
                down_chain(w_qd[m], P, c_bql, m, t_qd[:, m, :])
                nc.gpsimd.dma_start(cc_in_r[:, m, :], t_qd[:, m, :])
            nc.gpsimd.collective_compute(
                "AllGather", mybir.AluOpType.bypass,
                replica_groups=RG,
                ins=[cc_in[:]], outs=[cc_out[:]],
            )
            nc.gpsimd.dma_start(t_qd[:, 0:6, :], cc_out_r[:, 0:6, :])

            # ---- kv down (c 0..3 latent, then rope chunk last) ----
            # the rope-chunk weight is fetched early on sync (own slot) so
            # the rope matmuls never wait behind the rms activations
            wt5 = wst.tile([P, DC, ROPE], bf16, tag="wkv5", bufs=1,
                           name="wt5")
            nc.sync.dma_start(wt5[:, :, :ROPE],
                              wkvd_r[:, :, bass.ds(512, ROPE)])
            for m in range(4):
                wt = wst.tile([P, DC, P], bf16, tag="wqd", bufs=4)
                eng = nc.scalar if m % 2 == 0 else nc.sync
                eng.dma_start(wt[:], wkvd_r[:, :, bass.ts(m, P)])
                down_chain(wt, P, c_bkvd, m, t_kv[:, m, :])
            nc.scalar.dma_start(c_cos[:], cos2[:])
            nc.scalar.dma_start(c_sin[:], sina[:])
            # kv rms (chunks 0..3) -- the sqrt + recip + scale latency chain
            # hides under the rope-chunk matmuls that follow
            ps_ms = psR.tile([P, 2, 512], f32, tag="ms", name="ps_ms")
            for tt in range(2):
                for c in range(KC):
                    sq = tmp.tile([P, 512], bf16, tag="sq")
                    nc.vector.tensor_mul(
                        sq[:], t_kv[:, c, bass.ts(tt, 512)],
                        t_kv[:, c, bass.ts(tt, 512)])
                    nc.tensor.matmul(
                        ps_ms[:, tt, :], ones_bf[:], sq[:],
                        start=(c == 0), stop=(c == KC - 1),
                    )
            # rsqrt(mean+eps): scalar Sqrt then custom-DVE fast reciprocal
            # (no Ln: it lives in a different act table set than Exp and
            # would thrash the table loads)
            lnt = tmp.tile([P, T], f32, tag="lnt", bufs=1, name="lnt")
            with tc.high_priority():
                for tt in range(2):
                    h = bass.ts(tt, 512)
                    nc.scalar.activation(lnt[:, h], ps_ms[:, tt, :],
                                         AF.Sqrt, bias=eps_c[:],
                                         scale=1.0 / KVL)
                    nc.vector.reciprocal_approx_fast(out=rkv[:, h],
                                                     in_=lnt[:, h])
                    for c in range(KC):
                        nc.vector.tensor_mul(
                            t_kv[:, c, h], t_kv[:, c, h], rkv[:, h])
            # rope chunk of kv-down (weight prefetched above)
            down_chain(wt5, ROPE, c_bkvd, 4, t_kv[:, 4, :])
            # RoPE on k (rows duplicated to 128 for the two packed heads);
            # swaps on the sync queue so the DVE is not left waiting
            swp = tmp.tile([P, T], bf16, tag="swp", name="swp",
                           bufs=1)[:ROPE, :]
            nc.sync.dma_start(swp[0:32, :], t_kv[32:64, 4, :])
            nc.sync.dma_start(swp[32:64, :], t_kv[0:32, 4, :])
            nc.vector.tensor_mul(t_kr[0:ROPE, :], t_kv[0:ROPE, 4, :],
                                 c_cos[0:ROPE, :])
            nc.vector.tensor_mul(swp[:], swp[:], c_sin[0:ROPE, :])
            nc.vector.tensor_add(t_kr[0:ROPE, :], t_kr[0:ROPE, :], swp[:])
            nc.sync.dma_start(t_kr[ROPE:P, :], t_kr[0:ROPE, :])

            # ---- kNope up-projection (bias add on scalar engine) ----
            # weights all on sync: the scalar queue head-blocks on the rms
            # activations and would delay them
            kn_w = []
            for m in range(HH):
                wt = wkp.tile([P, KC, P], bf16, tag="wkn", bufs=4,
                              name="kn_w")
                nc.sync.dma_start(wt[:], wkvu_r[:, :, bass.ts(m, P)])
                kn_w.append(wt)
            nc.sync.dma_start(t_qd[:, 6:12, :], cc_out_r[:, 6:12, :])
            for m in range(HH):
                wt = kn_w[m]
                for tt in range(2):
                    ps = psA.tile([P, 512], f32, tag="ev", name="ps_kn")
                    for c in range(KC):
                        nc.tensor.matmul(
                            ps, wt[:, c, :],
                            t_kv[:, c, bass.ts(tt, 512)],
                            start=(c == 0), stop=(c == KC - 1),
                        )
                    nc.scalar.activation(
                        t_kn[:, m, bass.ts(tt, 512)], ps, AF.Identity,
                        bias=c_bkvuk[:, m:m + 1])

            # ---- v up-projection (token-on-partition; copies on scalar) ----
            # runs before q-rms so the collective readback of t_qd finishes
            # in its shadow
            for gg in range(2):
                wt = wkp.tile([P, KC, 512], bf16, tag="wv")
                (nc.sync if gg == 0 else nc.scalar).dma_start(
                    wt[:], wkvu_r[:, :, bass.ds(1024 + gg * 512, 512)])
                for tcb in range(8):
                    ps = psA.tile([P, 512], f32, tag="ev", name="ps_v")
                    for c in range(KC):
                        nc.tensor.matmul(
                            ps,
                            t_kv[:, c, bass.ts(tcb, P)],
                            wt[:, c, :],
                            start=(c == 0), stop=(c == KC - 1),
                        )
                    nc.scalar.activation(
                        t_v[:, tcb, bass.ts(gg, 512)], ps, AF.Copy)

            # ---- q rms (t_qd now holds the full gathered latent) ----
            ps_mq = psR.tile([P, 2, 512], f32, tag="ms", name="ps_mq")
            lnq = tmp.tile([P, T], f32, tag="lnt", bufs=1, name="lnq")
            for tt in range(2):
                hs = bass.ts(tt, 512)
                for c in range(QC):
                    sq = tmp.tile([P, 512], bf16, tag="sq")
                    nc.gpsimd.tensor_mul(
                        sq[:], t_qd[:, c, hs], t_qd[:, c, hs])
                    nc.tensor.matmul(
                        ps_mq[:, tt, :], ones_bf[:], sq[:],
                        start=(c == 0), stop=(c == QC - 1),
                    )
                nc.scalar.activation(lnq[:, hs], ps_mq[:, tt, :], AF.Sqrt,
                                     bias=eps_c[:], scale=1.0 / QL)
                nc.vector.reciprocal_approx_fast(out=rq[:, hs],
                                                 in_=lnq[:, hs])

            # ---- q up-projection ----
            for m in (0, 1, 8, 2, 3, 9, 4, 5, 10, 6, 7, 11):
                wt = wqp.tile([P, QC, P], bf16, tag="wqu")
                eng = nc.scalar if m % 2 == 0 else nc.sync
                eng.dma_start(wt[:], wqu_r[:, :, bass.ts(m, P)])
                for tt in range(2):
                    tsl = bass.ts(tt, 512)
                    ps = psA.tile([P, 512], f32, tag="ev", name="ps_qu")
                    for c in range(QC):
                        nc.tensor.matmul(
                            ps, wt[:, c, :], t_qd[:, c, tsl],
                            start=(c == 0), stop=(c == QC - 1),
                        )
                    if m < 8:
                        qsb = tmp.tile([P, 512], bf16, tag="qsb", bufs=2)
                        nc.vector.tensor_mul(qsb[:], ps, rq[:, tsl])
                        nc.vector.tensor_scalar_add(
                            out=t_q[:, m, tsl], in0=qsb,
                            scalar1=c_bqu[:, m:m + 1],
                        )
                    else:
                        sq = tmp.tile([P, 512], f32, tag="ropestage",
                                      bufs=1)
                        nc.vector.tensor_mul(sq[:], ps, rq[:, tsl])
                        nc.vector.tensor_scalar_add(
                            out=sq[:], in0=sq, scalar1=c_bqu[:, m:m + 1],
                        )
                        swq = tmp.tile([P, 512], f32, tag="ropeswap",
                                       bufs=1)
                        for r0 in (0, 64):
                            nc.gpsimd.dma_start(swq[r0:r0 + 32, :],
                                                sq[r0 + 32:r0 + 64, :])
                            nc.gpsimd.dma_start(swq[r0 + 32:r0 + 64, :],
                                                sq[r0:r0 + 32, :])
                        nc.vector.tensor_mul(sq[:], sq[:], c_cos[:, tsl])
                        nc.vector.tensor_mul(swq[:], swq[:], c_sin[:, tsl])
                        nc.vector.tensor_add(sq[:], sq[:], swq[:])
                        nc.vector.tensor_copy(t_q[:, m, tsl], sq[:])

        # ====== phase 3: attention (transposed scores, max-free) ======
        def vis_kcs(qt):
            return [kc for kc in range(8)
                    if qt * 512 + 511 >= kc * P - start]

        with tc.tile_pool(name="att", bufs=2) as att, \
             tc.tile_pool(name="psS", bufs=2, space="PSUM") as psS, \
             tc.tile_pool(name="psD", bufs=1, space="PSUM") as psD, \
             tc.tile_pool(name="psU", bufs=2, space="PSUM") as psU:

            def scores_qt(hp, expts2, qt):
                # expts2 [P, head2, kc, q] for heads (2hp, 2hp+1)
                rc = 8 + hp
                for kc in vis_kcs(qt):
                    lo = max(qt * 512, kc * P - start)
                    w = qt * 512 + 512 - lo
                    rel = lo - qt * 512
                    sc2 = psS.tile([P, 2, 512], f32, tag="sc", name="sc2")
                    for h2 in range(2):
                        h = 2 * hp + h2
                        nc.tensor.matmul(
                            sc2[:, h2, rel:],
                            t_kn[:, h, bass.ts(kc, P)],
                            t_q[:, h, bass.ds(lo, w)],
                            start=True, stop=False,
                        )
                    for h2 in range(2):
                        r0 = h2 * ROPE
                        nc.tensor.matmul(
                            sc2[:, h2, rel:],
                            t_kr[r0:r0 + ROPE, bass.ts(kc, P)],
                            t_q[r0:r0 + ROPE, rc, bass.ds(lo, w)],
                            start=False, stop=True,
                        )
                    # partially-masked diagonal band
                    b_lo = max(lo, kc * P - start)
                    b_hi = min(qt * 512 + 512, kc * P - start + P)
                    bw = b_hi - b_lo
                    if bw > 0:
                        j0 = b_lo - (kc * P - start)
                        br = b_lo - qt * 512
                        for h2 in range(2):
                            nc.vector.tensor_add(
                                sc2[:, h2, br:br + bw],
                                sc2[:, h2, br:br + bw],
                                c_tri[:, j0:j0 + bw])
                    nc.scalar.activation(
                        expts2[:, :, kc, bass.ds(lo, w)],
                        sc2[:, :, rel:], AF.Exp)

            def den_outU_head(hp, expts2, h2):
                h = 2 * hp + h2
                den2 = psD.tile([P, 2, 512], f32, name="den2")
                for qt in range(2):
                    kcs = vis_kcs(qt)
                    for i, kc in enumerate(kcs):
                        lo = max(qt * 512, kc * P - start)
                        rel = lo - qt * 512
                        nc.tensor.matmul(
                            den2[:, qt, rel:], ones_bf[:],
                            expts2[:, h2, kc, bass.ds(lo, 512 - rel)],
                            start=(i == 0), stop=(i == len(kcs) - 1),
                        )
                rcp = att.tile([P, 2, 512], f32, tag="rcp", name="rcp")
                nc.vector.reciprocal_approx_fast(
                    out=rcp[:, :, :], in_=den2[:, :, :])
                for qt in range(2):
                    kcs = vis_kcs(qt)
                    outU = psU.tile([P, 512], f32, tag="outU", name="outU")
                    for i, kc in enumerate(kcs):
                        lo = max(qt * 512, kc * P - start)
                        rel = lo - qt * 512
                        nc.tensor.matmul(
                            outU[:, rel:], t_v[:, kc, bass.ts(h, P)],
                            expts2[:, h2, kc, bass.ds(lo, 512 - rel)],
                            start=(i == 0), stop=(i == len(kcs) - 1),
                        )
                    nc.vector.tensor_mul(
                        t_ao[:, h, bass.ts(qt, 512)], outU[:],
                        rcp[:, qt, :])

            # scores(hp) interleave with den/outU of hp-1 at qt granularity
            prev = None
            for hp in range(4):
                cur = att.tile([P, 2, 8, T], bf16, tag="expt", name="expt2")
                scores_qt(hp, cur, 0)
                if prev is not None:
                    den_outU_head(hp - 1, prev, 0)
                scores_qt(hp, cur, 1)
                if prev is not None:
                    den_outU_head(hp - 1, prev, 1)
                prev = cur
            den_outU_head(3, prev, 0)
            den_outU_head(3, prev, 1)

            # ====== phase 4: output projection ======
            for m in range(DC):
                wt = att.tile([P, HH, P], bf16, tag="wo", name="wo_t",
                              bufs=4)
                eng = nc.gpsimd if m % 2 == 0 else nc.sync
                eng.dma_start(wt[:], wo_r[:, :, bass.ts(m, P)])
                for tt in range(2):
                    ps = psU.tile([P, 512], f32, tag="outU", name="ps_o")
                    for c in range(HH):
                        nc.tensor.matmul(
                            ps, wt[:, c, :], t_ao[:, c, bass.ts(tt, 512)],
                            start=(c == 0), stop=(c == HH - 1),
                        )
                    ot = att.tile([P, 512], f32, tag="ot", name="ot",
                                  bufs=3)
                    nc.vector.tensor_copy(ot[:], ps)
                    nc.sync.dma_start(outt_r[:, m, bass.ts(tt, 512)], ot[:])

    nc.compile()
    return nc


_CACHE = {}


def _get_nc(start: int):
    if start not in _CACHE:
        _CACHE[start] = build_nc(start)
    return _CACHE[start]


def _prep_inputs(X, base_freq, Wqd, bqd, gq, Wqu, bqu, Wkv, bkv, gkv,
                 Wkvu, bkvu, Wo, bo, start):
    f = np.float32
    X = np.asarray(X, f)
    base_freq = np.asarray(base_freq, f)
    Wqd = np.asarray(Wqd, f); bqd = np.asarray(bqd, f)
    gq = np.asarray(gq, f); Wqu = np.asarray(Wqu, f); bqu = np.asarray(bqu, f)
    Wkv = np.asarray(Wkv, f); bkv = np.asarray(bkv, f)
    gkv = np.asarray(gkv, f); Wkvu = np.asarray(Wkvu, f)
    bkvu = np.asarray(bkvu, f)
    Wo = np.asarray(Wo, f); bo = np.asarray(bo, f)
    start = int(np.asarray(start).item())
    assert start >= 0

    scale = QKH ** (-0.5)
    bf = ml_dtypes.bfloat16

    # v-bias exact fold: probs sum to 1, so the v bias contributes
    # Wo @ bv to every token's output.
    bv = bkvu.reshape(H, NOPE + VH)[:, NOPE:].reshape(H * VH)
    bo_eff = bo + Wo @ bv

    # qd down W split 6/6 across the TP pair; kv down duplicated
    wqd_t = Wqd.T.astype(f)                                   # (D, QL)
    wkv_t = Wkv.T.astype(f)                                   # (D, NKV)
    wqdl, bql = [], []
    for g in range(2):
        wqdl.append(np.ascontiguousarray(
            wqd_t[:, g * 768:(g + 1) * 768]).astype(bf))
        bql.append(np.ascontiguousarray(
            bqd[g * 768:(g + 1) * 768].reshape(6, P).T))
    wkvd = np.concatenate([wkv_t[:, :576], np.zeros((D, 64), f)], 1)
    wkvd = np.ascontiguousarray(wkvd).astype(bf)
    bkvd_p = np.zeros((5 * P,), f); bkvd_p[:NKV] = bkv
    bkvd = np.ascontiguousarray(bkvd_p.reshape(5, P).T)

    ang = base_freq[:S]                                       # (S, ROPE)
    cos = np.ascontiguousarray(np.cos(ang).T.astype(f))       # (ROPE, S)
    sin = np.ascontiguousarray(np.sin(ang).T.astype(f))
    cos2 = np.ascontiguousarray(np.concatenate([cos, cos], 0))  # (128, S)
    sgn = np.ones((ROPE, 1), f); sgn[:ROPE // 2] = -1.0
    sins = sin * sgn                                          # sign-folded
    sina = np.ascontiguousarray(np.concatenate([sins, sins], 0))

    # universal diagonal-band mask: for the block at k = kc*P + p,
    # q = (kc*P - start) + j, visibility is p <= j.
    pp = np.arange(P)
    tri = np.where(pp[:, None] <= pp[None, :], 0.0, NEG).astype(bf)
    tri = np.ascontiguousarray(tri)

    # per head-group tensors
    perm_q = np.concatenate(
        [np.arange(h * QKH, h * QKH + NOPE) for h in range(HH)]
        + [np.arange(h * QKH + NOPE, (h + 1) * QKH) for h in range(HH)]
    )
    perm_kv = np.concatenate(
        [np.arange(h * (NOPE + VH), h * (NOPE + VH) + NOPE) for h in range(HH)]
        + [np.arange(h * (NOPE + VH) + NOPE, (h + 1) * (NOPE + VH))
           for h in range(HH)]
    )
    gmaps = []
    for g in range(2):
        rq = slice(g * HH * QKH, (g + 1) * HH * QKH)
        rkv = slice(g * HH * (NOPE + VH), (g + 1) * HH * (NOPE + VH))
        wqu_g = (Wqu[rq, :] * gq[None, :] * scale)[perm_q]    # (1536, QL)
        bqu_g = (bqu[rq] * scale)[perm_q]
        wkvu_g = (Wkvu[rkv, :] * gkv[None, :])[perm_kv]       # (2048, KVL)
        bkvu_g = bkvu[rkv][perm_kv]
        wo_g = Wo[:, g * HH * VH:(g + 1) * HH * VH]           # (D, 1024)
        gmaps.append({
            "wqu": np.ascontiguousarray(wqu_g.T).astype(bf),
            "bqu": np.ascontiguousarray(bqu_g.reshape(QC, P).T),
            "wkvu": np.ascontiguousarray(wkvu_g.T).astype(bf),
            "bkvuk": np.ascontiguousarray(
                bkvu_g[:HH * NOPE].reshape(HH, P).T),
            "wo": np.ascontiguousarray(wo_g.T).astype(bf),    # (1024, D)
        })

    xts = [np.ascontiguousarray(X[b].T).astype(bf) for b in range(B)]

    in_maps = []
    for c in range(8):
        b, g = c // 2, c % 2
        m = {
            "xt": xts[b], "wqdl": wqdl[g], "bql": bql[g],
            "wkvd": wkvd, "bkvd": bkvd,
            "cos2": cos2, "sina": sina, "tri": tri,
        }
        m.update(gmaps[g])
        in_maps.append(m)
    return in_maps, bo_eff, start


def kernel(**inputs) -> np.ndarray:
    in_maps, bo_eff, start = _prep_inputs(**inputs)
    nc = _get_nc(start)
    try:
        res = run_bass_kernel_spmd(nc, in_maps, core_ids=list(range(8)))
    except Exception:
        res = run_bass_kernel_spmd(nc, in_maps, core_ids=list(range(8)))
    out = np.empty((B, S, D), np.float32)
    for b in range(B):
        acc = res.results[2 * b]["outt"] + res.results[2 * b + 1]["outt"]
        out[b] = acc.T + bo_eff[None, :]
    return out


# revision 23
# speedup vs baseline: 1.0595x; 1.0595x over previous
"""Trainium2 Bass kernel for MultiHeadLatentAttention (MLA), 8-core SPMD.

Sharding: data-parallel over batch (4) x tensor-parallel over heads (2).
Core c handles batch c//2 and heads (c%2)*8 .. +8. Each core computes its
partial output projection; the host sums the two TP partials per batch and
adds the (v-bias-folded) output bias.

Device layout is feature-on-partition / token-on-free throughout, so every
projection is a plain matmul chain with no transposes. Attention uses
transposed scores (keys on partitions) so probs feed the AV matmul directly.

v2 notes (vs the 445us baseline):
- phase order qd -> allgather -> kv -> kNope -> q-rms -> v -> qu so the
  collective and both RMS latency chains hide under independent PE work
- rsqrt via exp(-0.5*ln(x)) on the scalar engine and softmax 1/den via the
  custom-DVE reciprocal_approx_fast: one activation table set, no 6.5us
  DVE reciprocals on the critical path
- per-(kc,qt) score PSUM holds BOTH heads of a pair -> single batched exp
- head-pair software pipelining: scores(hp) overlap den/outU/norm(hp-1)
- v-up bias matmuls removed (exact host fold: bo += Wo @ bkvu_v)
- sum-of-squares matmuls in bf16; 128x128 universal triangle mask
"""

import sys
from contextlib import ExitStack

import numpy as np
import ml_dtypes

for _p in ("/opt/trn_rl_repo", "/root/.axon_site/_ro/trn_rl_repo"):
    if _p not in sys.path:
        sys.path.append(_p)

import concourse.bass as bass  # noqa: E402
import concourse.mybir as mybir  # noqa: E402
from concourse import bacc  # noqa: E402
from concourse.bass_utils import run_bass_kernel_spmd  # noqa: E402
from concourse.tile import TileContext  # noqa: E402

# Problem shapes (hardcoded per contract)
B, S, D = 4, 1024, 2048
H = 16
QL, KVL = 1536, 512
NOPE, ROPE, VH = 128, 64, 128
QKH = NOPE + ROPE  # 192
EPS = 1e-6

P = 128
T = S          # tokens per core (one batch)
DC = D // P    # 16 X chunks
QC = QL // P   # 12 q-latent chunks
KC = KVL // P  # 4 kv-latent chunks
HH = H // 2    # 8 heads per core
NKV = KVL + ROPE  # 576
NEG = -1.0e4   # mask bias (exp underflows to exactly 0)

f32 = mybir.dt.float32
bf16 = mybir.dt.bfloat16
AF = mybir.ActivationFunctionType


def build_nc(start: int):
    nc = bacc.Bacc(None, target_bir_lowering=False, debug=False)

    dp = nc.declare_dram_parameter
    xt = dp("xt", [D, T], bf16, isOutput=False)           # X[b].T
    wqdl = dp("wqdl", [D, 6 * P], bf16, isOutput=False)   # local qd W.T
    wkvd = dp("wkvd", [D, 5 * P], bf16, isOutput=False)   # kv down W.T (padded)
    wqu = dp("wqu", [QL, QL], bf16, isOutput=False)        # (perm q) Wqu_eff.T
    wkvu = dp("wkvu", [KVL, 2048], bf16, isOutput=False)   # (perm kv) Wkvu_eff.T
    wo = dp("wo", [HH * VH, D], bf16, isOutput=False)     # Wo[:, slice].T
    bql_i = dp("bql", [P, 6], f32, isOutput=False)        # local qd bias
    bkvd_i = dp("bkvd", [P, 5], f32, isOutput=False)      # kv down bias
    bqu_i = dp("bqu", [P, QC], f32, isOutput=False)       # perm + scale
    bkvuk = dp("bkvuk", [P, HH], f32, isOutput=False)     # kNope part
    cos2 = dp("cos2", [P, T], f32, isOutput=False)        # duplicated rows
    sina = dp("sina", [P, T], f32, isOutput=False)        # sign-folded sin
    tri_i = dp("tri", [P, P], bf16, isOutput=False)       # diagonal-band mask
    outt = dp("outt", [D, T], f32, isOutput=True)

    cc_in = nc.dram_tensor("cc_in", [6 * P, T], bf16)
    cc_out = nc.dram_tensor("cc_out", [12 * P, T], bf16)
    RG = [[0, 1], [2, 3], [4, 5], [6, 7]]

    xt_r = xt.rearrange("(c p) t -> p c t", p=P)
    wqdl_r = wqdl.rearrange("(c p) m -> p c m", p=P)
    wkvd_r = wkvd.rearrange("(c p) m -> p c m", p=P)
    wqu_r = wqu.rearrange("(c p) m -> p c m", p=P)
    wkvu_r = wkvu.rearrange("(c p) m -> p c m", p=P)
    wo_r = wo.rearrange("(c p) m -> p c m", p=P)
    outt_r = outt.rearrange("(c p) t -> p c t", p=P)
    cc_in_r = cc_in.rearrange("(c p) t -> p c t", p=P)
    cc_out_r = cc_out.rearrange("(c p) t -> p c t", p=P)

    with TileContext(nc) as tc, ExitStack() as stk:
        const = stk.enter_context(tc.tile_pool(name="const", bufs=1))
        persist = stk.enter_context(tc.tile_pool(name="persist", bufs=1))

        # ---- constants in SBUF (X goes first; see phase 1) ----
        c_bql = const.tile([P, 6], f32)
        c_bkvd = const.tile([P, 5], f32)
        c_bqu = const.tile([P, QC], f32)
        c_bkvuk = const.tile([P, HH], f32)
        c_tri = const.tile([P, P], bf16)
        c_cos = const.tile([P, T], f32)
        c_sin = const.tile([P, T], f32)
        ones_bf = const.tile([P, P], bf16)
        nc.vector.memset(ones_bf[:], 1.0)
        eps_c = const.tile([P, 1], f32)
        nc.vector.memset(eps_c[:], EPS)

        # ---- persistent activations ----
        t_q = persist.tile([P, QC, T], bf16)      # q heads (nope 0-7, rope 8-11)
        t_kn = persist.tile([P, HH, T], bf16)     # kNope[feat, head, tok]
        t_v = persist.tile([P, T // P, HH * P], bf16)  # v[tok, tokchunk, hv]
        t_kr = persist.tile([P, T], bf16)         # kRot, rows duplicated
        t_ao = persist.tile([P, HH, T], bf16)     # attn out [vh, head, tok]
        rq = persist.tile([P, T], f32)            # q rms scale (per token)
        rkv = persist.tile([P, T], f32)           # kv rms scale

        # ====== phases 1+2: projections ======
        with tc.tile_pool(name="ph1", bufs=1) as ph1, \
             tc.tile_pool(name="wstream", bufs=2) as wst, \
             tc.tile_pool(name="wqu_p", bufs=2) as wqp, \
             tc.tile_pool(name="wkvu_p", bufs=2) as wkp, \
             tc.tile_pool(name="tmp", bufs=2) as tmp, \
             tc.tile_pool(name="psA", bufs=6, space="PSUM") as psA, \
             tc.tile_pool(name="psR", bufs=1, space="PSUM") as psR:

            # first two qd weights lead, then X split over sync/scalar with
            # only 1MB on gpsimd: the gpsimd queue must stay light so the
            # collective (stores -> AllGather -> readback) runs early
            t_x = ph1.tile([P, DC, T], bf16, name="t_x")
            w_qd = []
            for m in range(6):
                w_qd.append(wst.tile([P, DC, P], bf16, tag="wqd", bufs=4,
                                     name="w_qd"))
            nc.scalar.dma_start(w_qd[0][:], wqdl_r[:, :, bass.ts(0, P)])
            nc.sync.dma_start(w_qd[1][:], wqdl_r[:, :, bass.ts(1, P)])
            nc.sync.dma_start(t_x[:, 0:6, :], xt_r[:, 0:6, :])
            nc.scalar.dma_start(t_x[:, 6:12, :], xt_r[:, 6:12, :])
            nc.gpsimd.dma_start(t_x[:, 12:16, :], xt_r[:, 12:16, :])
            nc.scalar.dma_start(w_qd[2][:], wqdl_r[:, :, bass.ts(2, P)])
            nc.sync.dma_start(w_qd[3][:], wqdl_r[:, :, bass.ts(3, P)])
            nc.gpsimd.dma_start(c_bql[:], bql_i[:])
            nc.gpsimd.dma_start(c_bkvd[:], bkvd_i[:])
            nc.gpsimd.dma_start(c_bqu[:], bqu_i[:])
            nc.gpsimd.dma_start(c_bkvuk[:], bkvuk[:])
            nc.gpsimd.dma_start(c_tri[:], tri_i[:])
            t_kv = ph1.tile([P, 5, T], bf16, name="t_kv")
            t_qd = ph1.tile([P, QC, T], bf16, name="t_qd")

            def down_chain(wt, m_rows, bias_t, bcol, out_ap):
                # out[m_rows, T] = wt.T @ X + bias, as 2 half-token chains
                for tt in range(2):
                    ps = psA.tile([P, 512], f32, tag="ev", name="ps_ev")
                    psm = ps[:m_rows, :]
                    for c in range(DC):
                        nc.tensor.matmul(
                            psm, wt[:, c, :m_rows],
                            t_x[:, c, bass.ts(tt, 512)],
                            start=(c == 0), stop=(c == DC - 1),
                        )
                    nc.vector.tensor_scalar_add(
                        out=out_ap[:m_rows, bass.ts(tt, 512)], in0=psm,
                        scalar1=bias_t[:m_rows, bcol:bcol + 1])

            # ---- q down: local 6 chunks -> exchange -> full 12 in t_qd ----
            # The AllGather output is in global QL order [g0 | g1], so
            # overwriting ALL of t_qd with cc_out leaves every core with the
            # naturally-ordered full latent regardless of its group. The
            # gpsimd queue serializes stores -> collective -> readback.
            for m in range(6):
                if m >= 4:
                    eng = nc.scalar if m % 2 == 0 else nc.sync
                    eng.dma_start(w_qd[m][:], wqdl_r[:, :, bass.ts(m, P)])
                down_chain(w_qd[m], P, c_bql, m, t_qd[:, m, :])
                nc.gpsimd.dma_start(cc_in_r[:, m, :], t_qd[:, m, :])
            nc.gpsimd.collective_compute(
                "AllGather", mybir.AluOpType.bypass,
                replica_groups=RG,
                ins=[cc_in[:]], outs=[cc_out[:]],
            )
            nc.gpsimd.dma_start(t_qd[:, 0:6, :], cc_out_r[:, 0:6, :])

            # ---- kv down (c 0..3 latent, then rope chunk last) ----
            # the rope-chunk weight is fetched early on sync (own slot) so
            # the rope matmuls never wait behind the rms activations
            wt5 = wst.tile([P, DC, ROPE], bf16, tag="wkv5", bufs=1,
                           name="wt5")
            nc.sync.dma_start(wt5[:, :, :ROPE],
                              wkvd_r[:, :, bass.ds(512, ROPE)])
            for m in range(4):
                wt = wst.tile([P, DC, P], bf16, tag="wqd", bufs=4)
                eng = nc.scalar if m % 2 == 0 else nc.sync
                eng.dma_start(wt[:], wkvd_r[:, :, bass.ts(m, P)])
                down_chain(wt, P, c_bkvd, m, t_kv[:, m, :])
            nc.scalar.dma_start(c_cos[:], cos2[:])
            nc.scalar.dma_start(c_sin[:], sina[:])
            # kv rms (chunks 0..3) -- the sqrt + recip + scale latency chain
            # hides under the rope-chunk matmuls that follow
            ps_ms = psR.tile([P, 2, 512], f32, tag="ms", name="ps_ms")
            for tt in range(2):
                for c in range(KC):
                    sq = tmp.tile([P, 512], bf16, tag="sq")
                    nc.vector.tensor_mul(
                        sq[:], t_kv[:, c, bass.ts(tt, 512)],
                        t_kv[:, c, bass.ts(tt, 512)])
                    nc.tensor.matmul(
                        ps_ms[:, tt, :], ones_bf[:], sq[:],
                        start=(c == 0), stop=(c == KC - 1),
                    )
            # rsqrt(mean+eps): scalar Sqrt then custom-DVE fast reciprocal
            # (no Ln: it lives in a different act table set than Exp and
            # would thrash the table loads)
            lnt = tmp.tile([P, T], f32, tag="lnt", bufs=1, name="lnt")
            with tc.high_priority():
                for tt in range(2):
                    h = bass.ts(tt, 512)
                    nc.scalar.activation(lnt[:, h], ps_ms[:, tt, :],
                                         AF.Sqrt, bias=eps_c[:],
                                         scale=1.0 / KVL)
                    nc.vector.reciprocal_approx_fast(out=rkv[:, h],
                                                     in_=lnt[:, h])
                    for c in range(KC):
                        nc.vector.tensor_mul(
                            t_kv[:, c, h], t_kv[:, c, h], rkv[:, h])
            # rope chunk of kv-down (weight prefetched above)
            down_chain(wt5, ROPE, c_bkvd, 4, t_kv[:, 4, :])
            # RoPE on k (rows duplicated to 128 for the two packed heads);
            # swaps on the sync queue so the DVE is not left waiting
            swp = tmp.tile([P, T], bf16, tag="swp", name="swp",
                           bufs=1)[:ROPE, :]
            nc.sync.dma_start(swp[0:32, :], t_kv[32:64, 4, :])
            nc.sync.dma_start(swp[32:64, :], t_kv[0:32, 4, :])
            nc.vector.tensor_mul(t_kr[0:ROPE, :], t_kv[0:ROPE, 4, :],
                                 c_cos[0:ROPE, :])
            nc.vector.tensor_mul(swp[:], swp[:], c_sin[0:ROPE, :])
            nc.vector.tensor_add(t_kr[0:ROPE, :], t_kr[0:ROPE, :], swp[:])
            nc.sync.dma_start(t_kr[ROPE:P, :], t_kr[0:ROPE, :])

            # ---- kNope up-projection (bias add on scalar engine) ----
            # weights all on sync: the scalar queue head-blocks on the rms
            # activations and would delay them
            kn_w = []
            for m in range(HH):
                wt = wkp.tile([P, KC, P], bf16, tag="wkn", bufs=4,
                              name="kn_w")
                nc.sync.dma_start(wt[:], wkvu_r[:, :, bass.ts(m, P)])
                kn_w.append(wt)
            nc.sync.dma_start(t_qd[:, 6:12, :], cc_out_r[:, 6:12, :])
            for m in range(HH):
                wt = kn_w[m]
                for tt in range(2):
                    ps = psA.tile([P, 512], f32, tag="ev", name="ps_kn")
                    for c in range(KC):
                        nc.tensor.matmul(
                            ps, wt[:, c, :],
                            t_kv[:, c, bass.ts(tt, 512)],
                            start=(c == 0), stop=(c == KC - 1),
                        )
                    nc.scalar.activation(
                        t_kn[:, m, bass.ts(tt, 512)], ps, AF.Identity,
                        bias=c_bkvuk[:, m:m + 1])

            # ---- v up-projection (token-on-partition; copies on scalar) ----
            # runs before q-rms so the collective readback of t_qd finishes
            # in its shadow
            for gg in range(2):
                wt = wkp.tile([P, KC, 512], bf16, tag="wv")
                (nc.sync if gg == 0 else nc.scalar).dma_start(
                    wt[:], wkvu_r[:, :, bass.ds(1024 + gg * 512, 512)])
                for tcb in range(8):
                    ps = psA.tile([P, 512], f32, tag="ev", name="ps_v")
                    for c in range(KC):
                        nc.tensor.matmul(
                            ps,
                            t_kv[:, c, bass.ts(tcb, P)],
                            wt[:, c, :],
                            start=(c == 0), stop=(c == KC - 1),
                        )
                    nc.scalar.activation(
                        t_v[:, tcb, bass.ts(gg, 512)], ps, AF.Copy)

            # ---- q rms (t_qd now holds the full gathered latent) ----
            # pinned late on the scheduler's model clock: its DMA/collective
            # model is optimistic, and without the pin it hoists these ops
            # ahead of kv/kNope/v work, head-blocking whichever engine they
            # sit on until the readback really lands
            ps_mq = psR.tile([P, 2, 512], f32, tag="ms", name="ps_mq")
            lnq = tmp.tile([P, T], f32, tag="lnt", bufs=1, name="lnq")
            with tc.tile_wait_until(0.120):
                for tt in range(2):
                    hs = bass.ts(tt, 512)
                    for c in range(QC):
                        sq = tmp.tile([P, 512], bf16, tag="sq")
                        nc.vector.tensor_mul(
                            sq[:], t_qd[:, c, hs], t_qd[:, c, hs])
                        nc.tensor.matmul(
                            ps_mq[:, tt, :], ones_bf[:], sq[:],
                            start=(c == 0), stop=(c == QC - 1),
                        )
                    nc.scalar.activation(lnq[:, hs], ps_mq[:, tt, :],
                                         AF.Sqrt, bias=eps_c[:],
                                         scale=1.0 / QL)
                    nc.vector.reciprocal_approx_fast(out=rq[:, hs],
                                                     in_=lnq[:, hs])

            # ---- q up-projection ----
            for m in (0, 1, 8, 2, 3, 9, 4, 5, 10, 6, 7, 11):
                wt = wqp.tile([P, QC, P], bf16, tag="wqu")
                eng = nc.scalar if m % 2 == 0 else nc.sync
                eng.dma_start(wt[:], wqu_r[:, :, bass.ts(m, P)])
                for tt in range(2):
                    tsl = bass.ts(tt, 512)
                    ps = psA.tile([P, 512], f32, tag="ev", name="ps_qu")
                    for c in range(QC):
                        nc.tensor.matmul(
                            ps, wt[:, c, :], t_qd[:, c, tsl],
                            start=(c == 0), stop=(c == QC - 1),
                        )
                    if m < 8:
                        qsb = tmp.tile([P, 512], bf16, tag="qsb", bufs=2)
                        nc.vector.tensor_mul(qsb[:], ps, rq[:, tsl])
                        nc.vector.tensor_scalar_add(
                            out=t_q[:, m, tsl], in0=qsb,
                            scalar1=c_bqu[:, m:m + 1],
                        )
                    else:
                        sq = tmp.tile([P, 512], f32, tag="ropestage",
                                      bufs=1)
                        nc.vector.tensor_mul(sq[:], ps, rq[:, tsl])
                        nc.vector.tensor_scalar_add(
                            out=sq[:], in0=sq, scalar1=c_bqu[:, m:m + 1],
                        )
                        swq = tmp.tile([P, 512], f32, tag="ropeswap",
                                       bufs=1)
                        for r0 in (0, 64):
                            nc.gpsimd.dma_start(swq[r0:r0 + 32, :],
                                                sq[r0 + 32:r0 + 64, :])
                            nc.gpsimd.dma_start(swq[r0 + 32:r0 + 64, :],
                                                sq[r0:r0 + 32, :])
                        nc.vector.tensor_mul(sq[:], sq[:], c_cos[:, tsl])
                        nc.vector.tensor_mul(swq[:], swq[:], c_sin[:, tsl])
                        nc.vector.tensor_add(sq[:], sq[:], swq[:])
                        nc.vector.tensor_copy(t_q[:, m, tsl], sq[:])

        # ====== phase 3: attention (transposed scores, max-free) ======
        def vis_kcs(qt):
            return [kc for kc in range(8)
                    if qt * 512 + 511 >= kc * P - start]

        with tc.tile_pool(name="att", bufs=2) as att, \
             tc.tile_pool(name="psS", bufs=2, space="PSUM") as psS, \
             tc.tile_pool(name="psD", bufs=1, space="PSUM") as psD, \
             tc.tile_pool(name="psU", bufs=2, space="PSUM") as psU:

            def scores_qt(hp, expts2, qt):
                # expts2 [P, head2, kc, q] for heads (2hp, 2hp+1)
                rc = 8 + hp
                for kc in vis_kcs(qt):
                    lo = max(qt * 512, kc * P - start)
                    w = qt * 512 + 512 - lo
                    rel = lo - qt * 512
                    sc2 = psS.tile([P, 2, 512], f32, tag="sc", name="sc2")
                    for h2 in range(2):
                        h = 2 * hp + h2
                        nc.tensor.matmul(
                            sc2[:, h2, rel:],
                            t_kn[:, h, bass.ts(kc, P)],
                            t_q[:, h, bass.ds(lo, w)],
                            start=True, stop=False,
                        )
                    for h2 in range(2):
                        r0 = h2 * ROPE
                        nc.tensor.matmul(
                            sc2[:, h2, rel:],
                            t_kr[r0:r0 + ROPE, bass.ts(kc, P)],
                            t_q[r0:r0 + ROPE, rc, bass.ds(lo, w)],
                            start=False, stop=True,
                        )
                    # partially-masked diagonal band
                    b_lo = max(lo, kc * P - start)
                    b_hi = min(qt * 512 + 512, kc * P - start + P)
                    bw = b_hi - b_lo
                    if bw > 0:
                        j0 = b_lo - (kc * P - start)
                        br = b_lo - qt * 512
                        for h2 in range(2):
                            nc.vector.tensor_add(
                                sc2[:, h2, br:br + bw],
                                sc2[:, h2, br:br + bw],
                                c_tri[:, j0:j0 + bw])
                    nc.scalar.activation(
                        expts2[:, :, kc, bass.ds(lo, w)],
                        sc2[:, :, rel:], AF.Exp)

            def den_outU_head(hp, expts2, h2):
                h = 2 * hp + h2
                den2 = psD.tile([P, 2, 512], f32, name="den2")
                for qt in range(2):
                    kcs = vis_kcs(qt)
                    for i, kc in enumerate(kcs):
                        lo = max(qt * 512, kc * P - start)
                        rel = lo - qt * 512
                        nc.tensor.matmul(
                            den2[:, qt, rel:], ones_bf[:],
                            expts2[:, h2, kc, bass.ds(lo, 512 - rel)],
                            start=(i == 0), stop=(i == len(kcs) - 1),
                        )
                rcp = att.tile([P, 2, 512], f32, tag="rcp", name="rcp")
                nc.vector.reciprocal_approx_fast(
                    out=rcp[:, :, :], in_=den2[:, :, :])
                for qt in range(2):
                    kcs = vis_kcs(qt)
                    outU = psU.tile([P, 512], f32, tag="outU", name="outU")
                    for i, kc in enumerate(kcs):
                        lo = max(qt * 512, kc * P - start)
                        rel = lo - qt * 512
                        nc.tensor.matmul(
                            outU[:, rel:], t_v[:, kc, bass.ts(h, P)],
                            expts2[:, h2, kc, bass.ds(lo, 512 - rel)],
                            start=(i == 0), stop=(i == len(kcs) - 1),
                        )
                    nc.vector.tensor_mul(
                        t_ao[:, h, bass.ts(qt, 512)], outU[:],
                        rcp[:, qt, :])

            # scores(hp) interleave with den/outU of hp-1 at qt granularity
            prev = None
            for hp in range(4):
                cur = att.tile([P, 2, 8, T], bf16, tag="expt", name="expt2")
                scores_qt(hp, cur, 0)
                if prev is not None:
                    den_outU_head(hp - 1, prev, 0)
                scores_qt(hp, cur, 1)
                if prev is not None:
                    den_outU_head(hp - 1, prev, 1)
                prev = cur
            den_outU_head(3, prev, 0)
            den_outU_head(3, prev, 1)

            # ====== phase 4: output projection ======
            for m in range(DC):
                wt = att.tile([P, HH, P], bf16, tag="wo", name="wo_t",
                              bufs=4)
                eng = nc.gpsimd if m % 2 == 0 else nc.sync
                eng.dma_start(wt[:], wo_r[:, :, bass.ts(m, P)])
                for tt in range(2):
                    ps = psU.tile([P, 512], f32, tag="outU", name="ps_o")
                    for c in range(HH):
                        nc.tensor.matmul(
                            ps, wt[:, c, :], t_ao[:, c, bass.ts(tt, 512)],
                            start=(c == 0), stop=(c == HH - 1),
                        )
                    ot = att.tile([P, 512], f32, tag="ot", name="ot",
                                  bufs=3)
                    nc.vector.tensor_copy(ot[:], ps)
                    nc.sync.dma_start(outt_r[:, m, bass.ts(tt, 512)], ot[:])

    nc.compile()
    return nc


_CACHE = {}


def _get_nc(start: int):
    if start not in _CACHE:
        _CACHE[start] = build_nc(start)
    return _CACHE[start]


def _prep_inputs(X, base_freq, Wqd, bqd, gq, Wqu, bqu, Wkv, bkv, gkv,
                 Wkvu, bkvu, Wo, bo, start):
    f = np.float32
    X = np.asarray(X, f)
    base_freq = np.asarray(base_freq, f)
    Wqd = np.asarray(Wqd, f); bqd = np.asarray(bqd, f)
    gq = np.asarray(gq, f); Wqu = np.asarray(Wqu, f); bqu = np.asarray(bqu, f)
    Wkv = np.asarray(Wkv, f); bkv = np.asarray(bkv, f)
    gkv = np.asarray(gkv, f); Wkvu = np.asarray(Wkvu, f)
    bkvu = np.asarray(bkvu, f)
    Wo = np.asarray(Wo, f); bo = np.asarray(bo, f)
    start = int(np.asarray(start).item())
    assert start >= 0

    scale = QKH ** (-0.5)
    bf = ml_dtypes.bfloat16

    # v-bias exact fold: probs sum to 1, so the v bias contributes
    # Wo @ bv to every token's output.
    bv = bkvu.reshape(H, NOPE + VH)[:, NOPE:].reshape(H * VH)
    bo_eff = bo + Wo @ bv

    # qd down W split 6/6 across the TP pair; kv down duplicated
    wqd_t = Wqd.T.astype(f)                                   # (D, QL)
    wkv_t = Wkv.T.astype(f)                                   # (D, NKV)
    wqdl, bql = [], []
    for g in range(2):
        wqdl.append(np.ascontiguousarray(
            wqd_t[:, g * 768:(g + 1) * 768]).astype(bf))
        bql.append(np.ascontiguousarray(
            bqd[g * 768:(g + 1) * 768].reshape(6, P).T))
    wkvd = np.concatenate([wkv_t[:, :576], np.zeros((D, 64), f)], 1)
    wkvd = np.ascontiguousarray(wkvd).astype(bf)
    bkvd_p = np.zeros((5 * P,), f); bkvd_p[:NKV] = bkv
    bkvd = np.ascontiguousarray(bkvd_p.reshape(5, P).T)

    ang = base_freq[:S]                                       # (S, ROPE)
    cos = np.ascontiguousarray(np.cos(ang).T.astype(f))       # (ROPE, S)
    sin = np.ascontiguousarray(np.sin(ang).T.astype(f))
    cos2 = np.ascontiguousarray(np.concatenate([cos, cos], 0))  # (128, S)
    sgn = np.ones((ROPE, 1), f); sgn[:ROPE // 2] = -1.0
    sins = sin * sgn                                          # sign-folded
    sina = np.ascontiguousarray(np.concatenate([sins, sins], 0))

    # universal diagonal-band mask: for the block at k = kc*P + p,
    # q = (kc*P - start) + j, visibility is p <= j.
    pp = np.arange(P)
    tri = np.where(pp[:, None] <= pp[None, :], 0.0, NEG).astype(bf)
    tri = np.ascontiguousarray(tri)

    # per head-group tensors
    perm_q = np.concatenate(
        [np.arange(h * QKH, h * QKH + NOPE) for h in range(HH)]
        + [np.arange(h * QKH + NOPE, (h + 1) * QKH) for h in range(HH)]
    )
    perm_kv = np.concatenate(
        [np.arange(h * (NOPE + VH), h * (NOPE + VH) + NOPE) for h in range(HH)]
        + [np.arange(h * (NOPE + VH) + NOPE, (h + 1) * (NOPE + VH))
           for h in range(HH)]
    )
    gmaps = []
    for g in range(2):
        rq = slice(g * HH * QKH, (g + 1) * HH * QKH)
        rkv = slice(g * HH * (NOPE + VH), (g + 1) * HH * (NOPE + VH))
        wqu_g = (Wqu[rq, :] * gq[None, :] * scale)[perm_q]    # (1536, QL)
        bqu_g = (bqu[rq] * scale)[perm_q]
        wkvu_g = (Wkvu[rkv, :] * gkv[None, :])[perm_kv]       # (2048, KVL)
        bkvu_g = bkvu[rkv][perm_kv]
        wo_g = Wo[:, g * HH * VH:(g + 1) * HH * VH]           # (D, 1024)
        gmaps.append({
            "wqu": np.ascontiguousarray(wqu_g.T).astype(bf),
            "bqu": np.ascontiguousarray(bqu_g.reshape(QC, P).T),
            "wkvu": np.ascontiguousarray(wkvu_g.T).astype(bf),
            "bkvuk": np.ascontiguousarray(
                bkvu_g[:HH * NOPE].reshape(HH, P).T),
            "wo": np.ascontiguousarray(wo_g.T).astype(bf),    # (1024, D)
        })

    xts = [np.ascontiguousarray(X[b].T).astype(bf) for b in range(B)]

    in_maps = []
    for c in range(8):
        b, g = c // 2, c % 2
        m = {
            "xt": xts[b], "wqdl": wqdl[g], "bql": bql[g],
            "wkvd": wkvd, "bkvd": bkvd,
            "cos2": cos2, "sina": sina, "tri": tri,
        }
        m.update(gmaps[g])
        in_maps.append(m)
    return in_maps, bo_eff, start


def kernel(**inputs) -> np.ndarray:
    in_maps, bo_eff, start = _prep_inputs(**inputs)
    nc = _get_nc(start)
    try:
        res = run_bass_kernel_spmd(nc, in_maps, core_ids=list(range(8)))
    except Exception:
        res = run_bass_kernel_spmd(nc, in_maps, core_ids=list(range(8)))
    out = np.empty((B, S, D), np.float32)
    for b in range(B):
        acc = res.results[2 * b]["outt"] + res.results[2 * b + 1]["outt"]
        out[b] = acc.T + bo_eff[None, :]
    return out


# revision 24
# speedup vs baseline: 1.0674x; 1.0075x over previous
"""Trainium2 Bass kernel for MultiHeadLatentAttention (MLA), 8-core SPMD.

Sharding: data-parallel over batch (4) x tensor-parallel over heads (2).
Core c handles batch c//2 and heads (c%2)*8 .. +8. Each core computes its
partial output projection; the host sums the two TP partials per batch and
adds the (v-bias-folded) output bias.

Device layout is feature-on-partition / token-on-free throughout, so every
projection is a plain matmul chain with no transposes. Attention uses
transposed scores (keys on partitions) so probs feed the AV matmul directly.

v2 notes (vs the 445us baseline):
- phase order qd -> allgather -> kv -> kNope -> q-rms -> v -> qu so the
  collective and both RMS latency chains hide under independent PE work
- rsqrt via exp(-0.5*ln(x)) on the scalar engine and softmax 1/den via the
  custom-DVE reciprocal_approx_fast: one activation table set, no 6.5us
  DVE reciprocals on the critical path
- per-(kc,qt) score PSUM holds BOTH heads of a pair -> single batched exp
- head-pair software pipelining: scores(hp) overlap den/outU/norm(hp-1)
- v-up bias matmuls removed (exact host fold: bo += Wo @ bkvu_v)
- sum-of-squares matmuls in bf16; 128x128 universal triangle mask
"""

import sys
from contextlib import ExitStack

import numpy as np
import ml_dtypes

for _p in ("/opt/trn_rl_repo", "/root/.axon_site/_ro/trn_rl_repo"):
    if _p not in sys.path:
        sys.path.append(_p)

import concourse.bass as bass  # noqa: E402
import concourse.mybir as mybir  # noqa: E402
from concourse import bacc  # noqa: E402
from concourse.bass_utils import run_bass_kernel_spmd  # noqa: E402
from concourse.tile import TileContext  # noqa: E402

# Problem shapes (hardcoded per contract)
B, S, D = 4, 1024, 2048
H = 16
QL, KVL = 1536, 512
NOPE, ROPE, VH = 128, 64, 128
QKH = NOPE + ROPE  # 192
EPS = 1e-6

P = 128
T = S          # tokens per core (one batch)
DC = D // P    # 16 X chunks
QC = QL // P   # 12 q-latent chunks
KC = KVL // P  # 4 kv-latent chunks
HH = H // 2    # 8 heads per core
NKV = KVL + ROPE  # 576
NEG = -1.0e4   # mask bias (exp underflows to exactly 0)

f32 = mybir.dt.float32
bf16 = mybir.dt.bfloat16
AF = mybir.ActivationFunctionType


def build_nc(start: int):
    nc = bacc.Bacc(None, target_bir_lowering=False, debug=False)

    dp = nc.declare_dram_parameter
    xt = dp("xt", [D, T], bf16, isOutput=False)           # X[b].T
    wqdl = dp("wqdl", [D, 6 * P], bf16, isOutput=False)   # local qd W.T
    wkvd = dp("wkvd", [D, 5 * P], bf16, isOutput=False)   # kv down W.T (padded)
    wqu = dp("wqu", [QL, QL], bf16, isOutput=False)        # (perm q) Wqu_eff.T
    wkvu = dp("wkvu", [KVL, 2048], bf16, isOutput=False)   # (perm kv) Wkvu_eff.T
    wo = dp("wo", [HH * VH, D], bf16, isOutput=False)     # Wo[:, slice].T
    bql_i = dp("bql", [P, 6], f32, isOutput=False)        # local qd bias
    bkvd_i = dp("bkvd", [P, 5], f32, isOutput=False)      # kv down bias
    bqu_i = dp("bqu", [P, QC], f32, isOutput=False)       # perm + scale
    bkvuk = dp("bkvuk", [P, HH], f32, isOutput=False)     # kNope part
    cos2 = dp("cos2", [P, T], f32, isOutput=False)        # duplicated rows
    sina = dp("sina", [P, T], f32, isOutput=False)        # sign-folded sin
    tri_i = dp("tri", [P, P], bf16, isOutput=False)       # diagonal-band mask
    outt = dp("outt", [D, T], f32, isOutput=True)

    cc_in = nc.dram_tensor("cc_in", [6 * P, T], bf16)
    cc_out = nc.dram_tensor("cc_out", [12 * P, T], bf16)
    RG = [[0, 1], [2, 3], [4, 5], [6, 7]]

    xt_r = xt.rearrange("(c p) t -> p c t", p=P)
    wqdl_r = wqdl.rearrange("(c p) m -> p c m", p=P)
    wkvd_r = wkvd.rearrange("(c p) m -> p c m", p=P)
    wqu_r = wqu.rearrange("(c p) m -> p c m", p=P)
    wkvu_r = wkvu.rearrange("(c p) m -> p c m", p=P)
    wo_r = wo.rearrange("(c p) m -> p c m", p=P)
    outt_r = outt.rearrange("(c p) t -> p c t", p=P)
    cc_in_r = cc_in.rearrange("(c p) t -> p c t", p=P)
    cc_out_r = cc_out.rearrange("(c p) t -> p c t", p=P)

    with TileContext(nc) as tc, ExitStack() as stk:
        const = stk.enter_context(tc.tile_pool(name="const", bufs=1))
        persist = stk.enter_context(tc.tile_pool(name="persist", bufs=1))

        # ---- constants in SBUF (X goes first; see phase 1) ----
        c_bql = const.tile([P, 6], f32)
        c_bkvd = const.tile([P, 5], f32)
        c_bqu = const.tile([P, QC], f32)
        c_bkvuk = const.tile([P, HH], f32)
        c_tri = const.tile([P, P], bf16)
        c_cos = const.tile([P, T], f32)
        c_sin = const.tile([P, T], f32)
        ones_bf = const.tile([P, P], bf16)
        nc.vector.memset(ones_bf[:], 1.0)
        eps_c = const.tile([P, 1], f32)
        nc.vector.memset(eps_c[:], EPS)

        # ---- persistent activations ----
        t_q = persist.tile([P, QC, T], bf16)      # q heads (nope 0-7, rope 8-11)
        t_kn = persist.tile([P, HH, T], bf16)     # kNope[feat, head, tok]
        t_v = persist.tile([P, T // P, HH * P], bf16)  # v[tok, tokchunk, hv]
        t_kr = persist.tile([P, T], bf16)         # kRot, rows duplicated
        t_ao = persist.tile([P, HH, T], bf16)     # attn out [vh, head, tok]
        rq = persist.tile([P, T], f32)            # q rms scale (per token)
        rkv = persist.tile([P, T], f32)           # kv rms scale

        # ====== phases 1+2: projections ======
        with tc.tile_pool(name="ph1", bufs=1) as ph1, \
             tc.tile_pool(name="wstream", bufs=2) as wst, \
             tc.tile_pool(name="wqu_p", bufs=2) as wqp, \
             tc.tile_pool(name="wkvu_p", bufs=2) as wkp, \
             tc.tile_pool(name="tmp", bufs=2) as tmp, \
             tc.tile_pool(name="psA", bufs=6, space="PSUM") as psA, \
             tc.tile_pool(name="psR", bufs=1, space="PSUM") as psR:

            # first two qd weights lead, then X split over sync/scalar with
            # only 1MB on gpsimd: the gpsimd queue must stay light so the
            # collective (stores -> AllGather -> readback) runs early
            t_x = ph1.tile([P, DC, T], bf16, name="t_x")
            w_qd = []
            for m in range(6):
                w_qd.append(wst.tile([P, DC, P], bf16, tag="wqd", bufs=4,
                                     name="w_qd"))
            nc.scalar.dma_start(w_qd[0][:], wqdl_r[:, :, bass.ts(0, P)])
            nc.sync.dma_start(w_qd[1][:], wqdl_r[:, :, bass.ts(1, P)])
            nc.sync.dma_start(t_x[:, 0:6, :], xt_r[:, 0:6, :])
            nc.scalar.dma_start(t_x[:, 6:12, :], xt_r[:, 6:12, :])
            nc.gpsimd.dma_start(t_x[:, 12:16, :], xt_r[:, 12:16, :])
            nc.scalar.dma_start(w_qd[2][:], wqdl_r[:, :, bass.ts(2, P)])
            nc.sync.dma_start(w_qd[3][:], wqdl_r[:, :, bass.ts(3, P)])
            nc.gpsimd.dma_start(c_bql[:], bql_i[:])
            nc.gpsimd.dma_start(c_bkvd[:], bkvd_i[:])
            nc.gpsimd.dma_start(c_bqu[:], bqu_i[:])
            nc.gpsimd.dma_start(c_bkvuk[:], bkvuk[:])
            nc.gpsimd.dma_start(c_tri[:], tri_i[:])
            t_kv = ph1.tile([P, 5, T], bf16, name="t_kv")
            t_qd = ph1.tile([P, QC, T], bf16, name="t_qd")

            def down_chain(wt, m_rows, bias_t, bcol, out_ap):
                # out[m_rows, T] = wt.T @ X + bias, as 2 half-token chains
                for tt in range(2):
                    ps = psA.tile([P, 512], f32, tag="ev", name="ps_ev")
                    psm = ps[:m_rows, :]
                    for c in range(DC):
                        nc.tensor.matmul(
                            psm, wt[:, c, :m_rows],
                            t_x[:, c, bass.ts(tt, 512)],
                            start=(c == 0), stop=(c == DC - 1),
                        )
                    nc.vector.tensor_scalar_add(
                        out=out_ap[:m_rows, bass.ts(tt, 512)], in0=psm,
                        scalar1=bias_t[:m_rows, bcol:bcol + 1])

            # ---- q down: local 6 chunks -> exchange -> full 12 in t_qd ----
            # The AllGather output is in global QL order [g0 | g1], so
            # overwriting ALL of t_qd with cc_out leaves every core with the
            # naturally-ordered full latent regardless of its group. The
            # gpsimd queue serializes stores -> collective -> readback.
            for m in range(6):
                if m >= 4:
                    eng = nc.scalar if m % 2 == 0 else nc.sync
                    eng.dma_start(w_qd[m][:], wqdl_r[:, :, bass.ts(m, P)])
                down_chain(w_qd[m], P, c_bql, m, t_qd[:, m, :])
                nc.gpsimd.dma_start(cc_in_r[:, m, :], t_qd[:, m, :])
            nc.gpsimd.collective_compute(
                "AllGather", mybir.AluOpType.bypass,
                replica_groups=RG,
                ins=[cc_in[:]], outs=[cc_out[:]],
            )
            nc.gpsimd.dma_start(t_qd[:, 0:6, :], cc_out_r[:, 0:6, :])

            # ---- kv down (c 0..3 latent, then rope chunk last) ----
            # the rope-chunk weight is fetched early on sync (own slot) so
            # the rope matmuls never wait behind the rms activations
            wt5 = wst.tile([P, DC, ROPE], bf16, tag="wkv5", bufs=1,
                           name="wt5")
            nc.sync.dma_start(wt5[:, :, :ROPE],
                              wkvd_r[:, :, bass.ds(512, ROPE)])
            for m in range(4):
                wt = wst.tile([P, DC, P], bf16, tag="wqd", bufs=4)
                eng = nc.scalar if m % 2 == 0 else nc.sync
                eng.dma_start(wt[:], wkvd_r[:, :, bass.ts(m, P)])
                down_chain(wt, P, c_bkvd, m, t_kv[:, m, :])
            nc.scalar.dma_start(c_cos[:], cos2[:])
            nc.scalar.dma_start(c_sin[:], sina[:])
            # kv rms (chunks 0..3) -- the sqrt + recip + scale latency chain
            # hides under the rope-chunk matmuls that follow
            ps_ms = psR.tile([P, 2, 512], f32, tag="ms", name="ps_ms")
            for tt in range(2):
                for c in range(KC):
                    sq = tmp.tile([P, 512], bf16, tag="sq")
                    nc.vector.tensor_mul(
                        sq[:], t_kv[:, c, bass.ts(tt, 512)],
                        t_kv[:, c, bass.ts(tt, 512)])
                    nc.tensor.matmul(
                        ps_ms[:, tt, :], ones_bf[:], sq[:],
                        start=(c == 0), stop=(c == KC - 1),
                    )
            # rsqrt(mean+eps): scalar Sqrt then custom-DVE fast reciprocal
            # (no Ln: it lives in a different act table set than Exp and
            # would thrash the table loads)
            with tc.high_priority():
                for tt in range(2):
                    h = bass.ts(tt, 512)
                    nc.scalar.activation(rkv[:, h], ps_ms[:, tt, :],
                                         AF.Sqrt, bias=eps_c[:],
                                         scale=1.0 / KVL)
                    nc.vector.reciprocal_approx_fast(out=rkv[:, h],
                                                     in_=rkv[:, h])
                    for c in range(KC):
                        nc.vector.tensor_mul(
                            t_kv[:, c, h], t_kv[:, c, h], rkv[:, h])
            # rope chunk of kv-down (weight prefetched above)
            down_chain(wt5, ROPE, c_bkvd, 4, t_kv[:, 4, :])
            # RoPE on k (rows duplicated to 128 for the two packed heads);
            # swaps on the sync queue so the DVE is not left waiting
            swp = tmp.tile([P, T], bf16, tag="swp", name="swp",
                           bufs=1)[:ROPE, :]
            nc.sync.dma_start(swp[0:32, :], t_kv[32:64, 4, :])
            nc.sync.dma_start(swp[32:64, :], t_kv[0:32, 4, :])
            nc.vector.tensor_mul(t_kr[0:ROPE, :], t_kv[0:ROPE, 4, :],
                                 c_cos[0:ROPE, :])
            nc.vector.tensor_mul(swp[:], swp[:], c_sin[0:ROPE, :])
            nc.vector.tensor_add(t_kr[0:ROPE, :], t_kr[0:ROPE, :], swp[:])
            nc.sync.dma_start(t_kr[ROPE:P, :], t_kr[0:ROPE, :])

            # ---- kNope up-projection (bias add on scalar engine) ----
            # weights all on sync: the scalar queue head-blocks on the rms
            # activations and would delay them
            kn_w = []
            for m in range(HH):
                wt = wkp.tile([P, KC, P], bf16, tag="wkn", bufs=6,
                              name="kn_w")
                nc.sync.dma_start(wt[:], wkvu_r[:, :, bass.ts(m, P)])
                kn_w.append(wt)
            nc.sync.dma_start(t_qd[:, 6:12, :], cc_out_r[:, 6:12, :])
            for m in range(HH):
                wt = kn_w[m]
                for tt in range(2):
                    ps = psA.tile([P, 512], f32, tag="ev", name="ps_kn")
                    for c in range(KC):
                        nc.tensor.matmul(
                            ps, wt[:, c, :],
                            t_kv[:, c, bass.ts(tt, 512)],
                            start=(c == 0), stop=(c == KC - 1),
                        )
                    nc.scalar.activation(
                        t_kn[:, m, bass.ts(tt, 512)], ps, AF.Identity,
                        bias=c_bkvuk[:, m:m + 1])

            # ---- v up-projection (token-on-partition; copies on scalar) ----
            # runs before q-rms so the collective readback of t_qd finishes
            # in its shadow
            for gg in range(2):
                wt = wkp.tile([P, KC, 512], bf16, tag="wv")
                (nc.sync if gg == 0 else nc.scalar).dma_start(
                    wt[:], wkvu_r[:, :, bass.ds(1024 + gg * 512, 512)])
                for tcb in range(8):
                    ps = psA.tile([P, 512], f32, tag="ev", name="ps_v")
                    for c in range(KC):
                        nc.tensor.matmul(
                            ps,
                            t_kv[:, c, bass.ts(tcb, P)],
                            wt[:, c, :],
                            start=(c == 0), stop=(c == KC - 1),
                        )
                    nc.scalar.activation(
                        t_v[:, tcb, bass.ts(gg, 512)], ps, AF.Copy)

            # ---- q rms (t_qd now holds the full gathered latent) ----
            # pinned late on the scheduler's model clock: its DMA/collective
            # model is optimistic, and without the pin it hoists these ops
            # ahead of kv/kNope/v work, head-blocking whichever engine they
            # sit on until the readback really lands
            ps_mq = psR.tile([P, 2, 512], f32, tag="ms", name="ps_mq")
            with tc.tile_wait_until(0.120):
                for tt in range(2):
                    hs = bass.ts(tt, 512)
                    for c in range(QC):
                        sq = tmp.tile([P, 512], bf16, tag="sq")
                        nc.vector.tensor_mul(
                            sq[:], t_qd[:, c, hs], t_qd[:, c, hs])
                        nc.tensor.matmul(
                            ps_mq[:, tt, :], ones_bf[:], sq[:],
                            start=(c == 0), stop=(c == QC - 1),
                        )
                    nc.scalar.activation(rq[:, hs], ps_mq[:, tt, :],
                                         AF.Sqrt, bias=eps_c[:],
                                         scale=1.0 / QL)
                    nc.vector.reciprocal_approx_fast(out=rq[:, hs],
                                                     in_=rq[:, hs])

            # ---- q up-projection ----
            for m in (8, 0, 1, 9, 2, 3, 10, 4, 5, 11, 6, 7):
                wt = wqp.tile([P, QC, P], bf16, tag="wqu")
                eng = nc.scalar if m % 2 == 0 else nc.sync
                eng.dma_start(wt[:], wqu_r[:, :, bass.ts(m, P)])
                for tt in range(2):
                    tsl = bass.ts(tt, 512)
                    ps = psA.tile([P, 512], f32, tag="ev", name="ps_qu")
                    for c in range(QC):
                        nc.tensor.matmul(
                            ps, wt[:, c, :], t_qd[:, c, tsl],
                            start=(c == 0), stop=(c == QC - 1),
                        )
                    if m < 8:
                        qsb = tmp.tile([P, 512], bf16, tag="qsb", bufs=2)
                        nc.vector.tensor_mul(qsb[:], ps, rq[:, tsl])
                        nc.scalar.activation(
                            t_q[:, m, tsl], qsb, AF.Identity,
                            bias=c_bqu[:, m:m + 1],
                        )
                    else:
                        sq = tmp.tile([P, 512], f32, tag="ropestage",
                                      bufs=1)
                        nc.vector.tensor_mul(sq[:], ps, rq[:, tsl])
                        nc.vector.tensor_scalar_add(
                            out=sq[:], in0=sq, scalar1=c_bqu[:, m:m + 1],
                        )
                        swq = tmp.tile([P, 512], f32, tag="ropeswap",
                                       bufs=1)
                        for r0 in (0, 64):
                            nc.gpsimd.dma_start(swq[r0:r0 + 32, :],
                                                sq[r0 + 32:r0 + 64, :])
                            nc.gpsimd.dma_start(swq[r0 + 32:r0 + 64, :],
                                                sq[r0:r0 + 32, :])
                        nc.vector.tensor_mul(sq[:], sq[:], c_cos[:, tsl])
                        nc.vector.tensor_mul(swq[:], swq[:], c_sin[:, tsl])
                        nc.vector.tensor_add(sq[:], sq[:], swq[:])
                        nc.scalar.activation(t_q[:, m, tsl], sq[:], AF.Copy)

        # ====== phase 3: attention (transposed scores, max-free) ======
        def vis_kcs(qt):
            return [kc for kc in range(8)
                    if qt * 512 + 511 >= kc * P - start]

        with tc.tile_pool(name="att", bufs=2) as att, \
             tc.tile_pool(name="psS", bufs=2, space="PSUM") as psS, \
             tc.tile_pool(name="psD", bufs=1, space="PSUM") as psD, \
             tc.tile_pool(name="psU", bufs=2, space="PSUM") as psU:

            def scores_qt(hp, expts2, qt):
                # expts2 [P, head2, kc, q] for heads (2hp, 2hp+1)
                rc = 8 + hp
                for kc in vis_kcs(qt):
                    lo = max(qt * 512, kc * P - start)
                    w = qt * 512 + 512 - lo
                    rel = lo - qt * 512
                    sc2 = psS.tile([P, 2, 512], f32, tag="sc", name="sc2")
                    for h2 in range(2):
                        h = 2 * hp + h2
                        nc.tensor.matmul(
                            sc2[:, h2, rel:],
                            t_kn[:, h, bass.ts(kc, P)],
                            t_q[:, h, bass.ds(lo, w)],
                            start=True, stop=False,
                        )
                    for h2 in range(2):
                        r0 = h2 * ROPE
                        nc.tensor.matmul(
                            sc2[:, h2, rel:],
                            t_kr[r0:r0 + ROPE, bass.ts(kc, P)],
                            t_q[r0:r0 + ROPE, rc, bass.ds(lo, w)],
                            start=False, stop=True,
                        )
                    # partially-masked diagonal band
                    b_lo = max(lo, kc * P - start)
                    b_hi = min(qt * 512 + 512, kc * P - start + P)
                    bw = b_hi - b_lo
                    if bw > 0:
                        j0 = b_lo - (kc * P - start)
                        br = b_lo - qt * 512
                        for h2 in range(2):
                            nc.vector.tensor_add(
                                sc2[:, h2, br:br + bw],
                                sc2[:, h2, br:br + bw],
                                c_tri[:, j0:j0 + bw])
                    nc.scalar.activation(
                        expts2[:, :, kc, bass.ds(lo, w)],
                        sc2[:, :, rel:], AF.Exp)

            def den_outU_head(hp, expts2, h2):
                h = 2 * hp + h2
                den2 = psD.tile([P, 2, 512], f32, name="den2")
                for qt in range(2):
                    kcs = vis_kcs(qt)
                    for i, kc in enumerate(kcs):
                        lo = max(qt * 512, kc * P - start)
                        rel = lo - qt * 512
                        nc.tensor.matmul(
                            den2[:, qt, rel:], ones_bf[:],
                            expts2[:, h2, kc, bass.ds(lo, 512 - rel)],
                            start=(i == 0), stop=(i == len(kcs) - 1),
                        )
                rcp = att.tile([P, 2, 512], f32, tag="rcp", name="rcp")
                nc.vector.reciprocal_approx_fast(
                    out=rcp[:, :, :], in_=den2[:, :, :])
                for qt in range(2):
                    kcs = vis_kcs(qt)
                    outU = psU.tile([P, 512], f32, tag="outU", name="outU")
                    for i, kc in enumerate(kcs):
                        lo = max(qt * 512, kc * P - start)
                        rel = lo - qt * 512
                        nc.tensor.matmul(
                            outU[:, rel:], t_v[:, kc, bass.ts(h, P)],
                            expts2[:, h2, kc, bass.ds(lo, 512 - rel)],
                            start=(i == 0), stop=(i == len(kcs) - 1),
                        )
                    nc.vector.tensor_mul(
                        t_ao[:, h, bass.ts(qt, 512)], outU[:],
                        rcp[:, qt, :])

            # scores(hp) interleave with den/outU of hp-1 at qt granularity
            prev = None
            for hp in range(4):
                cur = att.tile([P, 2, 8, T], bf16, tag="expt", name="expt2")
                scores_qt(hp, cur, 0)
                if prev is not None:
                    den_outU_head(hp - 1, prev, 0)
                scores_qt(hp, cur, 1)
                if prev is not None:
                    den_outU_head(hp - 1, prev, 1)
                prev = cur
            den_outU_head(3, prev, 0)
            den_outU_head(3, prev, 1)

            # ====== phase 4: output projection ======
            for m in range(DC):
                wt = att.tile([P, HH, P], bf16, tag="wo", name="wo_t",
                              bufs=4)
                eng = nc.gpsimd if m % 2 == 0 else nc.sync
                eng.dma_start(wt[:], wo_r[:, :, bass.ts(m, P)])
                for tt in range(2):
                    ps = psU.tile([P, 512], f32, tag="outU", name="ps_o")
                    for c in range(HH):
                        nc.tensor.matmul(
                            ps, wt[:, c, :], t_ao[:, c, bass.ts(tt, 512)],
                            start=(c == 0), stop=(c == HH - 1),
                        )
                    ot = att.tile([P, 512], f32, tag="ot", name="ot",
                                  bufs=3)
                    nc.vector.tensor_copy(ot[:], ps)
                    nc.sync.dma_start(outt_r[:, m, bass.ts(tt, 512)], ot[:])

    nc.compile()
    return nc


_CACHE = {}


def _get_nc(start: int):
    if start not in _CACHE:
        _CACHE[start] = build_nc(start)
    return _CACHE[start]


def _prep_inputs(X, base_freq, Wqd, bqd, gq, Wqu, bqu, Wkv, bkv, gkv,
                 Wkvu, bkvu, Wo, bo, start):
    f = np.float32
    X = np.asarray(X, f)
    base_freq = np.asarray(base_freq, f)
    Wqd = np.asarray(Wqd, f); bqd = np.asarray(bqd, f)
    gq = np.asarray(gq, f); Wqu = np.asarray(Wqu, f); bqu = np.asarray(bqu, f)
    Wkv = np.asarray(Wkv, f); bkv = np.asarray(bkv, f)
    gkv = np.asarray(gkv, f); Wkvu = np.asarray(Wkvu, f)
    bkvu = np.asarray(bkvu, f)
    Wo = np.asarray(Wo, f); bo = np.asarray(bo, f)
    start = int(np.asarray(start).item())
    assert start >= 0

    scale = QKH ** (-0.5)
    bf = ml_dtypes.bfloat16

    # v-bias exact fold: probs sum to 1, so the v bias contributes
    # Wo @ bv to every token's output.
    bv = bkvu.reshape(H, NOPE + VH)[:, NOPE:].reshape(H * VH)
    bo_eff = bo + Wo @ bv

    # qd down W split 6/6 across the TP pair; kv down duplicated
    wqd_t = Wqd.T.astype(f)                                   # (D, QL)
    wkv_t = Wkv.T.astype(f)                                   # (D, NKV)
    wqdl, bql = [], []
    for g in range(2):
        wqdl.append(np.ascontiguousarray(
            wqd_t[:, g * 768:(g + 1) * 768]).astype(bf))
        bql.append(np.ascontiguousarray(
            bqd[g * 768:(g + 1) * 768].reshape(6, P).T))
    wkvd = np.concatenate([wkv_t[:, :576], np.zeros((D, 64), f)], 1)
    wkvd = np.ascontiguousarray(wkvd).astype(bf)
    bkvd_p = np.zeros((5 * P,), f); bkvd_p[:NKV] = bkv
    bkvd = np.ascontiguousarray(bkvd_p.reshape(5, P).T)

    ang = base_freq[:S]                                       # (S, ROPE)
    cos = np.ascontiguousarray(np.cos(ang).T.astype(f))       # (ROPE, S)
    sin = np.ascontiguousarray(np.sin(ang).T.astype(f))
    cos2 = np.ascontiguousarray(np.concatenate([cos, cos], 0))  # (128, S)
    sgn = np.ones((ROPE, 1), f); sgn[:ROPE // 2] = -1.0
    sins = sin * sgn                                          # sign-folded
    sina = np.ascontiguousarray(np.concatenate([sins, sins], 0))

    # universal diagonal-band mask: for the block at k = kc*P + p,
    # q = (kc*P - start) + j, visibility is p <= j.
    pp = np.arange(P)
    tri = np.where(pp[:, None] <= pp[None, :], 0.0, NEG).astype(bf)
    tri = np.ascontiguousarray(tri)

    # per head-group tensors
    perm_q = np.concatenate(
        [np.arange(h * QKH, h * QKH + NOPE) for h in range(HH)]
        + [np.arange(h * QKH + NOPE, (h + 1) * QKH) for h in range(HH)]
    )
    perm_kv = np.concatenate(
        [np.arange(h * (NOPE + VH), h * (NOPE + VH) + NOPE) for h in range(HH)]
        + [np.arange(h * (NOPE + VH) + NOPE, (h + 1) * (NOPE + VH))
           for h in range(HH)]
    )
    gmaps = []
    for g in range(2):
        rq = slice(g * HH * QKH, (g + 1) * HH * QKH)
        rkv = slice(g * HH * (NOPE + VH), (g + 1) * HH * (NOPE + VH))
        wqu_g = (Wqu[rq, :] * gq[None, :] * scale)[perm_q]    # (1536, QL)
        bqu_g = (bqu[rq] * scale)[perm_q]
        wkvu_g = (Wkvu[rkv, :] * gkv[None, :])[perm_kv]       # (2048, KVL)
        bkvu_g = bkvu[rkv][perm_kv]
        wo_g = Wo[:, g * HH * VH:(g + 1) * HH * VH]           # (D, 1024)
        gmaps.append({
            "wqu": np.ascontiguousarray(wqu_g.T).astype(bf),
            "bqu": np.ascontiguousarray(bqu_g.reshape(QC, P).T),
            "wkvu": np.ascontiguousarray(wkvu_g.T).astype(bf),
            "bkvuk": np.ascontiguousarray(
                bkvu_g[:HH * NOPE].reshape(HH, P).T),
            "wo": np.ascontiguousarray(wo_g.T).astype(bf),    # (1024, D)
        })

    xts = [np.ascontiguousarray(X[b].T).astype(bf) for b in range(B)]

    in_maps = []
    for c in range(8):
        b, g = c // 2, c % 2
        m = {
            "xt": xts[b], "wqdl": wqdl[g], "bql": bql[g],
            "wkvd": wkvd, "bkvd": bkvd,
            "cos2": cos2, "sina": sina, "tri": tri,
        }
        m.update(gmaps[g])
        in_maps.append(m)
    return in_maps, bo_eff, start


def kernel(**inputs) -> np.ndarray:
    in_maps, bo_eff, start = _prep_inputs(**inputs)
    nc = _get_nc(start)
    try:
        res = run_bass_kernel_spmd(nc, in_maps, core_ids=list(range(8)))
    except Exception:
        res = run_bass_kernel_spmd(nc, in_maps, core_ids=list(range(8)))
    out = np.empty((B, S, D), np.float32)
    for b in range(B):
        acc = res.results[2 * b]["outt"] + res.results[2 * b + 1]["outt"]
        out[b] = acc.T + bo_eff[None, :]
    return out


# revision 25
# speedup vs baseline: 1.0950x; 1.0258x over previous
"""Trainium2 Bass kernel for MultiHeadLatentAttention (MLA), 8-core SPMD.

Sharding: data-parallel over batch (4) x tensor-parallel over heads (2).
Core c handles batch c//2 and heads (c%2)*8 .. +8. Each core computes its
partial output projection; the host sums the two TP partials per batch and
adds the (v-bias-folded) output bias.

Device layout is feature-on-partition / token-on-free throughout, so every
projection is a plain matmul chain with no transposes. Attention uses
transposed scores (keys on partitions) so probs feed the AV matmul directly.

v2 notes (vs the 445us baseline):
- phase order qd -> allgather -> kv -> kNope -> q-rms -> v -> qu so the
  collective and both RMS latency chains hide under independent PE work
- rsqrt via exp(-0.5*ln(x)) on the scalar engine and softmax 1/den via the
  custom-DVE reciprocal_approx_fast: one activation table set, no 6.5us
  DVE reciprocals on the critical path
- per-(kc,qt) score PSUM holds BOTH heads of a pair -> single batched exp
- head-pair software pipelining: scores(hp) overlap den/outU/norm(hp-1)
- v-up bias matmuls removed (exact host fold: bo += Wo @ bkvu_v)
- sum-of-squares matmuls in bf16; 128x128 universal triangle mask
"""

import sys
from contextlib import ExitStack

import numpy as np
import ml_dtypes

for _p in ("/opt/trn_rl_repo", "/root/.axon_site/_ro/trn_rl_repo"):
    if _p not in sys.path:
        sys.path.append(_p)

import concourse.bass as bass  # noqa: E402
import concourse.mybir as mybir  # noqa: E402
from concourse import bacc  # noqa: E402
from concourse.bass_utils import run_bass_kernel_spmd  # noqa: E402
from concourse.tile import TileContext  # noqa: E402

# Problem shapes (hardcoded per contract)
B, S, D = 4, 1024, 2048
H = 16
QL, KVL = 1536, 512
NOPE, ROPE, VH = 128, 64, 128
QKH = NOPE + ROPE  # 192
EPS = 1e-6

P = 128
T = S          # tokens per core (one batch)
DC = D // P    # 16 X chunks
QC = QL // P   # 12 q-latent chunks
KC = KVL // P  # 4 kv-latent chunks
HH = H // 2    # 8 heads per core
NKV = KVL + ROPE  # 576
NEG = -1.0e4   # mask bias (exp underflows to exactly 0)

f32 = mybir.dt.float32
bf16 = mybir.dt.bfloat16
AF = mybir.ActivationFunctionType


def build_nc(start: int):
    nc = bacc.Bacc(None, target_bir_lowering=False, debug=False)

    dp = nc.declare_dram_parameter
    xt = dp("xt", [D, T], bf16, isOutput=False)           # X[b].T
    wqdl = dp("wqdl", [D, 6 * P], bf16, isOutput=False)   # local qd W.T
    wkvd = dp("wkvd", [D, 5 * P], bf16, isOutput=False)   # kv down W.T (padded)
    wqu = dp("wqu", [QL, QL], bf16, isOutput=False)        # (perm q) Wqu_eff.T
    wkvu = dp("wkvu", [KVL, 2048], bf16, isOutput=False)   # (perm kv) Wkvu_eff.T
    wo = dp("wo", [HH * VH, D], bf16, isOutput=False)     # Wo[:, slice].T
    bql_i = dp("bql", [P, 6], f32, isOutput=False)        # local qd bias
    bkvd_i = dp("bkvd", [P, 5], f32, isOutput=False)      # kv down bias
    bqu_i = dp("bqu", [P, QC], f32, isOutput=False)       # perm + scale
    bkvuk = dp("bkvuk", [P, HH], f32, isOutput=False)     # kNope part
    cos2 = dp("cos2", [P, T], f32, isOutput=False)        # duplicated rows
    sina = dp("sina", [P, T], f32, isOutput=False)        # sign-folded sin
    tri_i = dp("tri", [P, P], bf16, isOutput=False)       # diagonal-band mask
    outt = dp("outt", [D, T], f32, isOutput=True)

    cc_in = nc.dram_tensor("cc_in", [6 * P, T], bf16)
    cc_out = nc.dram_tensor("cc_out", [12 * P, T], bf16)
    RG = [[0, 1], [2, 3], [4, 5], [6, 7]]

    xt_r = xt.rearrange("(c p) t -> p c t", p=P)
    wqdl_r = wqdl.rearrange("(c p) m -> p c m", p=P)
    wkvd_r = wkvd.rearrange("(c p) m -> p c m", p=P)
    wqu_r = wqu.rearrange("(c p) m -> p c m", p=P)
    wkvu_r = wkvu.rearrange("(c p) m -> p c m", p=P)
    wo_r = wo.rearrange("(c p) m -> p c m", p=P)
    outt_r = outt.rearrange("(c p) t -> p c t", p=P)
    cc_in_r = cc_in.rearrange("(c p) t -> p c t", p=P)
    cc_out_r = cc_out.rearrange("(c p) t -> p c t", p=P)

    with TileContext(nc) as tc, ExitStack() as stk:
        const = stk.enter_context(tc.tile_pool(name="const", bufs=1))
        persist = stk.enter_context(tc.tile_pool(name="persist", bufs=1))

        # ---- constants in SBUF (X goes first; see phase 1) ----
        c_bql = const.tile([P, 6], f32)
        c_bkvd = const.tile([P, 5], f32)
        c_bqu = const.tile([P, QC], f32)
        c_bkvuk = const.tile([P, HH], f32)
        c_tri = const.tile([P, P], bf16)
        c_cos = const.tile([P, T], f32)
        c_sin = const.tile([P, T], f32)
        ones_bf = const.tile([P, P], bf16)
        nc.vector.memset(ones_bf[:], 1.0)
        eps_c = const.tile([P, 1], f32)
        nc.vector.memset(eps_c[:], EPS)

        # ---- persistent activations ----
        t_q = persist.tile([P, QC, T], bf16)      # q heads (nope 0-7, rope 8-11)
        t_kn = persist.tile([P, HH, T], bf16)     # kNope[feat, head, tok]
        t_v = persist.tile([P, T // P, HH * P], bf16)  # v[tok, tokchunk, hv]
        t_kr = persist.tile([P, T], bf16)         # kRot, rows duplicated
        t_ao = persist.tile([P, HH, T], bf16)     # attn out [vh, head, tok]
        rq = persist.tile([P, T], f32)            # q rms scale (per token)
        rkv = persist.tile([P, T], f32)           # kv rms scale

        # ====== phases 1+2: projections ======
        with tc.tile_pool(name="ph1", bufs=1) as ph1, \
             tc.tile_pool(name="wstream", bufs=2) as wst, \
             tc.tile_pool(name="wqu_p", bufs=2) as wqp, \
             tc.tile_pool(name="wkvu_p", bufs=2) as wkp, \
             tc.tile_pool(name="tmp", bufs=2) as tmp, \
             tc.tile_pool(name="psA", bufs=6, space="PSUM") as psA, \
             tc.tile_pool(name="psR", bufs=1, space="PSUM") as psR:

            # first two qd weights lead, then X split over sync/scalar with
            # only 1MB on gpsimd: the gpsimd queue must stay light so the
            # collective (stores -> AllGather -> readback) runs early
            t_x = ph1.tile([P, DC, T], bf16, name="t_x")
            w_qd = []
            for m in range(6):
                w_qd.append(wst.tile([P, DC, P], bf16, tag="wqd", bufs=4,
                                     name="w_qd"))
            nc.scalar.dma_start(w_qd[0][:], wqdl_r[:, :, bass.ts(0, P)])
            nc.sync.dma_start(w_qd[1][:], wqdl_r[:, :, bass.ts(1, P)])
            nc.sync.dma_start(t_x[:, 0:6, :], xt_r[:, 0:6, :])
            nc.scalar.dma_start(t_x[:, 6:12, :], xt_r[:, 6:12, :])
            nc.gpsimd.dma_start(t_x[:, 12:16, :], xt_r[:, 12:16, :])
            nc.scalar.dma_start(w_qd[2][:], wqdl_r[:, :, bass.ts(2, P)])
            nc.sync.dma_start(w_qd[3][:], wqdl_r[:, :, bass.ts(3, P)])
            nc.gpsimd.dma_start(c_bql[:], bql_i[:])
            nc.gpsimd.dma_start(c_bkvd[:], bkvd_i[:])
            nc.gpsimd.dma_start(c_bqu[:], bqu_i[:])
            nc.gpsimd.dma_start(c_bkvuk[:], bkvuk[:])
            nc.gpsimd.dma_start(c_tri[:], tri_i[:])
            t_kv = ph1.tile([P, 5, T], bf16, name="t_kv")
            t_qd = ph1.tile([P, QC, T], bf16, name="t_qd")

            def down_chain(wt, m_rows, bias_t, bcol, out_ap):
                # out[m_rows, T] = wt.T @ X + bias, as 2 half-token chains
                for tt in range(2):
                    ps = psA.tile([P, 512], f32, tag="ev", name="ps_ev")
                    psm = ps[:m_rows, :]
                    for c in range(DC):
                        nc.tensor.matmul(
                            psm, wt[:, c, :m_rows],
                            t_x[:, c, bass.ts(tt, 512)],
                            start=(c == 0), stop=(c == DC - 1),
                        )
                    nc.vector.tensor_scalar_add(
                        out=out_ap[:m_rows, bass.ts(tt, 512)], in0=psm,
                        scalar1=bias_t[:m_rows, bcol:bcol + 1])

            # ---- q down: local 6 chunks -> exchange -> full 12 in t_qd ----
            # The AllGather output is in global QL order [g0 | g1], so
            # overwriting ALL of t_qd with cc_out leaves every core with the
            # naturally-ordered full latent regardless of its group. The
            # gpsimd queue serializes stores -> collective -> readback.
            for m in range(6):
                if m >= 4:
                    eng = nc.scalar if m % 2 == 0 else nc.sync
                    eng.dma_start(w_qd[m][:], wqdl_r[:, :, bass.ts(m, P)])
                down_chain(w_qd[m], P, c_bql, m, t_qd[:, m, :])
                nc.gpsimd.dma_start(cc_in_r[:, m, :], t_qd[:, m, :])
            nc.gpsimd.collective_compute(
                "AllGather", mybir.AluOpType.bypass,
                replica_groups=RG,
                ins=[cc_in[:]], outs=[cc_out[:]],
            )
            nc.gpsimd.dma_start(t_qd[:, 0:6, :], cc_out_r[:, 0:6, :])
            nc.gpsimd.dma_start(t_qd[:, 6:12, :], cc_out_r[:, 6:12, :])

            # ---- kv down (c 0..3 latent, then rope chunk last) ----
            # the rope-chunk weight is fetched early on sync (own slot) so
            # the rope matmuls never wait behind the rms activations
            wt5 = wst.tile([P, DC, ROPE], bf16, tag="wkv5", bufs=1,
                           name="wt5")
            nc.sync.dma_start(wt5[:, :, :ROPE],
                              wkvd_r[:, :, bass.ds(512, ROPE)])
            for m in range(4):
                wt = wst.tile([P, DC, P], bf16, tag="wqd", bufs=4)
                eng = nc.scalar if m % 2 == 0 else nc.sync
                eng.dma_start(wt[:], wkvd_r[:, :, bass.ts(m, P)])
                down_chain(wt, P, c_bkvd, m, t_kv[:, m, :])
            nc.scalar.dma_start(c_cos[:], cos2[:])
            nc.scalar.dma_start(c_sin[:], sina[:])
            # kv rms (chunks 0..3) -- the sqrt + recip + scale latency chain
            # hides under the rope-chunk matmuls that follow
            ps_ms = psR.tile([P, 2, 512], f32, tag="ms", name="ps_ms")
            for tt in range(2):
                for c in range(KC):
                    sq = tmp.tile([P, 512], bf16, tag="sq")
                    nc.vector.tensor_mul(
                        sq[:], t_kv[:, c, bass.ts(tt, 512)],
                        t_kv[:, c, bass.ts(tt, 512)])
                    nc.tensor.matmul(
                        ps_ms[:, tt, :], ones_bf[:], sq[:],
                        start=(c == 0), stop=(c == KC - 1),
                    )
            # rsqrt(mean+eps): scalar Sqrt then custom-DVE fast reciprocal
            # (no Ln: it lives in a different act table set than Exp and
            # would thrash the table loads)
            with tc.high_priority():
                for tt in range(2):
                    h = bass.ts(tt, 512)
                    nc.scalar.activation(rkv[:, h], ps_ms[:, tt, :],
                                         AF.Sqrt, bias=eps_c[:],
                                         scale=1.0 / KVL)
                    nc.vector.reciprocal_approx_fast(out=rkv[:, h],
                                                     in_=rkv[:, h])
                    for c in range(KC):
                        nc.vector.tensor_mul(
                            t_kv[:, c, h], t_kv[:, c, h], rkv[:, h])
            # rope chunk of kv-down (weight prefetched above)
            down_chain(wt5, ROPE, c_bkvd, 4, t_kv[:, 4, :])
            # RoPE on k (rows duplicated to 128 for the two packed heads);
            # swaps on the sync queue so the DVE is not left waiting
            swp = tmp.tile([P, T], bf16, tag="swp", name="swp",
                           bufs=1)[:ROPE, :]
            nc.sync.dma_start(swp[0:32, :], t_kv[32:64, 4, :])
            nc.sync.dma_start(swp[32:64, :], t_kv[0:32, 4, :])
            nc.vector.tensor_mul(t_kr[0:ROPE, :], t_kv[0:ROPE, 4, :],
                                 c_cos[0:ROPE, :])
            nc.vector.tensor_mul(swp[:], swp[:], c_sin[0:ROPE, :])
            nc.vector.tensor_add(t_kr[0:ROPE, :], t_kr[0:ROPE, :], swp[:])
            nc.sync.dma_start(t_kr[ROPE:P, :], t_kr[0:ROPE, :])

            # ---- kNope up-projection (bias add on scalar engine) ----
            # weights all on sync: the scalar queue head-blocks on the rms
            # activations and would delay them
            kn_w = []
            for m in range(HH):
                wt = wkp.tile([P, KC, P], bf16, tag="wkn", bufs=6,
                              name="kn_w")
                nc.sync.dma_start(wt[:], wkvu_r[:, :, bass.ts(m, P)])
                kn_w.append(wt)
            for m in range(HH):
                wt = kn_w[m]
                for tt in range(2):
                    ps = psA.tile([P, 512], f32, tag="ev", name="ps_kn")
                    for c in range(KC):
                        nc.tensor.matmul(
                            ps, wt[:, c, :],
                            t_kv[:, c, bass.ts(tt, 512)],
                            start=(c == 0), stop=(c == KC - 1),
                        )
                    nc.scalar.activation(
                        t_kn[:, m, bass.ts(tt, 512)], ps, AF.Identity,
                        bias=c_bkvuk[:, m:m + 1])

            # ---- v up-projection (token-on-partition; copies on scalar) ----
            # runs before q-rms so the collective readback of t_qd finishes
            # in its shadow
            for gg in range(2):
                wt = wkp.tile([P, KC, 512], bf16, tag="wv")
                (nc.sync if gg == 0 else nc.scalar).dma_start(
                    wt[:], wkvu_r[:, :, bass.ds(1024 + gg * 512, 512)])
                for tcb in range(8):
                    ps = psA.tile([P, 512], f32, tag="ev", name="ps_v")
                    for c in range(KC):
                        nc.tensor.matmul(
                            ps,
                            t_kv[:, c, bass.ts(tcb, P)],
                            wt[:, c, :],
                            start=(c == 0), stop=(c == KC - 1),
                        )
                    nc.scalar.activation(
                        t_v[:, tcb, bass.ts(gg, 512)], ps, AF.Copy)

            # ---- q rms (t_qd now holds the full gathered latent) ----
            # pinned late on the scheduler's model clock: its DMA/collective
            # model is optimistic, and without the pin it hoists these ops
            # ahead of kv/kNope/v work, head-blocking whichever engine they
            # sit on until the readback really lands
            ps_mq = psR.tile([P, 2, 512], f32, tag="ms", name="ps_mq")
            with tc.tile_wait_until(0.120):
                for tt in range(2):
                    hs = bass.ts(tt, 512)
                    for c in range(QC):
                        sq = tmp.tile([P, 512], bf16, tag="sq")
                        nc.vector.tensor_mul(
                            sq[:], t_qd[:, c, hs], t_qd[:, c, hs])
                        nc.tensor.matmul(
                            ps_mq[:, tt, :], ones_bf[:], sq[:],
                            start=(c == 0), stop=(c == QC - 1),
                        )
                    nc.scalar.activation(rq[:, hs], ps_mq[:, tt, :],
                                         AF.Sqrt, bias=eps_c[:],
                                         scale=1.0 / QL)
                    nc.vector.reciprocal_approx_fast(out=rq[:, hs],
                                                     in_=rq[:, hs])

            # ---- q up-projection ----
            for m in (8, 0, 1, 9, 2, 3, 10, 4, 5, 11, 6, 7):
                wt = wqp.tile([P, QC, P], bf16, tag="wqu")
                eng = nc.scalar if m % 2 == 0 else nc.sync
                eng.dma_start(wt[:], wqu_r[:, :, bass.ts(m, P)])
                for tt in range(2):
                    tsl = bass.ts(tt, 512)
                    ps = psA.tile([P, 512], f32, tag="ev", name="ps_qu")
                    for c in range(QC):
                        nc.tensor.matmul(
                            ps, wt[:, c, :], t_qd[:, c, tsl],
                            start=(c == 0), stop=(c == QC - 1),
                        )
                    if m < 8:
                        qsb = tmp.tile([P, 512], bf16, tag="qsb", bufs=2)
                        nc.vector.tensor_mul(qsb[:], ps, rq[:, tsl])
                        nc.scalar.activation(
                            t_q[:, m, tsl], qsb, AF.Identity,
                            bias=c_bqu[:, m:m + 1],
                        )
                    else:
                        sq = tmp.tile([P, 512], f32, tag="ropestage",
                                      bufs=1)
                        nc.vector.tensor_mul(sq[:], ps, rq[:, tsl])
                        nc.vector.tensor_scalar_add(
                            out=sq[:], in0=sq, scalar1=c_bqu[:, m:m + 1],
                        )
                        swq = tmp.tile([P, 512], f32, tag="ropeswap",
                                       bufs=1)
                        for r0 in (0, 64):
                            nc.gpsimd.dma_start(swq[r0:r0 + 32, :],
                                                sq[r0 + 32:r0 + 64, :])
                            nc.gpsimd.dma_start(swq[r0 + 32:r0 + 64, :],
                                                sq[r0:r0 + 32, :])
                        nc.vector.tensor_mul(sq[:], sq[:], c_cos[:, tsl])
                        nc.vector.tensor_mul(swq[:], swq[:], c_sin[:, tsl])
                        nc.vector.tensor_add(sq[:], sq[:], swq[:])
                        nc.scalar.activation(t_q[:, m, tsl], sq[:], AF.Copy)

        # ====== phase 3: attention (transposed scores, max-free) ======
        def vis_kcs(qt):
            return [kc for kc in range(8)
                    if qt * 512 + 511 >= kc * P - start]

        with tc.tile_pool(name="att", bufs=2) as att, \
             tc.tile_pool(name="psS", bufs=2, space="PSUM") as psS, \
             tc.tile_pool(name="psD", bufs=1, space="PSUM") as psD, \
             tc.tile_pool(name="psU", bufs=2, space="PSUM") as psU:

            def scores_qt(hp, expts2, qt):
                # expts2 [P, head2, kc, q] for heads (2hp, 2hp+1)
                rc = 8 + hp
                for kc in vis_kcs(qt):
                    lo = max(qt * 512, kc * P - start)
                    w = qt * 512 + 512 - lo
                    rel = lo - qt * 512
                    sc2 = psS.tile([P, 2, 512], f32, tag="sc", name="sc2")
                    for h2 in range(2):
                        h = 2 * hp + h2
                        nc.tensor.matmul(
                            sc2[:, h2, rel:],
                            t_kn[:, h, bass.ts(kc, P)],
                            t_q[:, h, bass.ds(lo, w)],
                            start=True, stop=False,
                        )
                    for h2 in range(2):
                        r0 = h2 * ROPE
                        nc.tensor.matmul(
                            sc2[:, h2, rel:],
                            t_kr[r0:r0 + ROPE, bass.ts(kc, P)],
                            t_q[r0:r0 + ROPE, rc, bass.ds(lo, w)],
                            start=False, stop=True,
                        )
                    # partially-masked diagonal band
                    b_lo = max(lo, kc * P - start)
                    b_hi = min(qt * 512 + 512, kc * P - start + P)
                    bw = b_hi - b_lo
                    if bw > 0:
                        j0 = b_lo - (kc * P - start)
                        br = b_lo - qt * 512
                        for h2 in range(2):
                            nc.vector.tensor_add(
                                sc2[:, h2, br:br + bw],
                                sc2[:, h2, br:br + bw],
                                c_tri[:, j0:j0 + bw])
                    nc.scalar.activation(
                        expts2[:, :, kc, bass.ds(lo, w)],
                        sc2[:, :, rel:], AF.Exp)

            def den_outU_head(hp, expts2, h2):
                h = 2 * hp + h2
                den2 = psD.tile([P, 2, 512], f32, name="den2")
                for qt in range(2):
                    kcs = vis_kcs(qt)
                    for i, kc in enumerate(kcs):
                        lo = max(qt * 512, kc * P - start)
                        rel = lo - qt * 512
                        nc.tensor.matmul(
                            den2[:, qt, rel:], ones_bf[:],
                            expts2[:, h2, kc, bass.ds(lo, 512 - rel)],
                            start=(i == 0), stop=(i == len(kcs) - 1),
                        )
                rcp = att.tile([P, 2, 512], f32, tag="rcp", name="rcp")
                nc.vector.reciprocal_approx_fast(
                    out=rcp[:, :, :], in_=den2[:, :, :])
                for qt in range(2):
                    kcs = vis_kcs(qt)
                    outU = psU.tile([P, 512], f32, tag="outU", name="outU")
                    for i, kc in enumerate(kcs):
                        lo = max(qt * 512, kc * P - start)
                        rel = lo - qt * 512
                        nc.tensor.matmul(
                            outU[:, rel:], t_v[:, kc, bass.ts(h, P)],
                            expts2[:, h2, kc, bass.ds(lo, 512 - rel)],
                            start=(i == 0), stop=(i == len(kcs) - 1),
                        )
                    nc.vector.tensor_mul(
                        t_ao[:, h, bass.ts(qt, 512)], outU[:],
                        rcp[:, qt, :])

            # scores(hp) interleave with den/outU of hp-1 at qt granularity
            prev = None
            for hp in range(4):
                cur = att.tile([P, 2, 8, T], bf16, tag="expt", name="expt2")
                scores_qt(hp, cur, 0)
                if prev is not None:
                    den_outU_head(hp - 1, prev, 0)
                scores_qt(hp, cur, 1)
                if prev is not None:
                    den_outU_head(hp - 1, prev, 1)
                prev = cur
            den_outU_head(3, prev, 0)
            den_outU_head(3, prev, 1)

            # ====== phase 4: output projection ======
            for m in range(DC):
                wt = att.tile([P, HH, P], bf16, tag="wo", name="wo_t",
                              bufs=4)
                eng = nc.gpsimd if m % 2 == 0 else nc.sync
                eng.dma_start(wt[:], wo_r[:, :, bass.ts(m, P)])
                for tt in range(2):
                    ps = psU.tile([P, 512], f32, tag="outU", name="ps_o")
                    for c in range(HH):
                        nc.tensor.matmul(
                            ps, wt[:, c, :], t_ao[:, c, bass.ts(tt, 512)],
                            start=(c == 0), stop=(c == HH - 1),
                        )
                    ot = att.tile([P, 512], f32, tag="ot", name="ot",
                                  bufs=3)
                    nc.vector.tensor_copy(ot[:], ps)
                    nc.sync.dma_start(outt_r[:, m, bass.ts(tt, 512)], ot[:])

    nc.compile()
    return nc


_CACHE = {}


def _get_nc(start: int):
    if start not in _CACHE:
        _CACHE[start] = build_nc(start)
    return _CACHE[start]


def _prep_inputs(X, base_freq, Wqd, bqd, gq, Wqu, bqu, Wkv, bkv, gkv,
                 Wkvu, bkvu, Wo, bo, start):
    f = np.float32
    X = np.asarray(X, f)
    base_freq = np.asarray(base_freq, f)
    Wqd = np.asarray(Wqd, f); bqd = np.asarray(bqd, f)
    gq = np.asarray(gq, f); Wqu = np.asarray(Wqu, f); bqu = np.asarray(bqu, f)
    Wkv = np.asarray(Wkv, f); bkv = np.asarray(bkv, f)
    gkv = np.asarray(gkv, f); Wkvu = np.asarray(Wkvu, f)
    bkvu = np.asarray(bkvu, f)
    Wo = np.asarray(Wo, f); bo = np.asarray(bo, f)
    start = int(np.asarray(start).item())
    assert start >= 0

    scale = QKH ** (-0.5)
    bf = ml_dtypes.bfloat16

    # v-bias exact fold: probs sum to 1, so the v bias contributes
    # Wo @ bv to every token's output.
    bv = bkvu.reshape(H, NOPE + VH)[:, NOPE:].reshape(H * VH)
    bo_eff = bo + Wo @ bv

    # qd down W split 6/6 across the TP pair; kv down duplicated
    wqd_t = Wqd.T.astype(f)                                   # (D, QL)
    wkv_t = Wkv.T.astype(f)                                   # (D, NKV)
    wqdl, bql = [], []
    for g in range(2):
        wqdl.append(np.ascontiguousarray(
            wqd_t[:, g * 768:(g + 1) * 768]).astype(bf))
        bql.append(np.ascontiguousarray(
            bqd[g * 768:(g + 1) * 768].reshape(6, P).T))
    wkvd = np.concatenate([wkv_t[:, :576], np.zeros((D, 64), f)], 1)
    wkvd = np.ascontiguousarray(wkvd).astype(bf)
    bkvd_p = np.zeros((5 * P,), f); bkvd_p[:NKV] = bkv
    bkvd = np.ascontiguousarray(bkvd_p.reshape(5, P).T)

    ang = base_freq[:S]                                       # (S, ROPE)
    cos = np.ascontiguousarray(np.cos(ang).T.astype(f))       # (ROPE, S)
    sin = np.ascontiguousarray(np.sin(ang).T.astype(f))
    cos2 = np.ascontiguousarray(np.concatenate([cos, cos], 0))  # (128, S)
    sgn = np.ones((ROPE, 1), f); sgn[:ROPE // 2] = -1.0
    sins = sin * sgn                                          # sign-folded
    sina = np.ascontiguousarray(np.concatenate([sins, sins], 0))

    # universal diagonal-band mask: for the block at k = kc*P + p,
    # q = (kc*P - start) + j, visibility is p <= j.
    pp = np.arange(P)
    tri = np.where(pp[:, None] <= pp[None, :], 0.0, NEG).astype(bf)
    tri = np.ascontiguousarray(tri)

    # per head-group tensors
    perm_q = np.concatenate(
        [np.arange(h * QKH, h * QKH + NOPE) for h in range(HH)]
        + [np.arange(h * QKH + NOPE, (h + 1) * QKH) for h in range(HH)]
    )
    perm_kv = np.concatenate(
        [np.arange(h * (NOPE + VH), h * (NOPE + VH) + NOPE) for h in range(HH)]
        + [np.arange(h * (NOPE + VH) + NOPE, (h + 1) * (NOPE + VH))
           for h in range(HH)]
    )
    gmaps = []
    for g in range(2):
        rq = slice(g * HH * QKH, (g + 1) * HH * QKH)
        rkv = slice(g * HH * (NOPE + VH), (g + 1) * HH * (NOPE + VH))
        wqu_g = (Wqu[rq, :] * gq[None, :] * scale)[perm_q]    # (1536, QL)
        bqu_g = (bqu[rq] * scale)[perm_q]
        wkvu_g = (Wkvu[rkv, :] * gkv[None, :])[perm_kv]       # (2048, KVL)
        bkvu_g = bkvu[rkv][perm_kv]
        wo_g = Wo[:, g * HH * VH:(g + 1) * HH * VH]           # (D, 1024)
        gmaps.append({
            "wqu": np.ascontiguousarray(wqu_g.T).astype(bf),
            "bqu": np.ascontiguousarray(bqu_g.reshape(QC, P).T),
            "wkvu": np.ascontiguousarray(wkvu_g.T).astype(bf),
            "bkvuk": np.ascontiguousarray(
                bkvu_g[:HH * NOPE].reshape(HH, P).T),
            "wo": np.ascontiguousarray(wo_g.T).astype(bf),    # (1024, D)
        })

    xts = [np.ascontiguousarray(X[b].T).astype(bf) for b in range(B)]

    in_maps = []
    for c in range(8):
        b, g = c // 2, c % 2
        m = {
            "xt": xts[b], "wqdl": wqdl[g], "bql": bql[g],
            "wkvd": wkvd, "bkvd": bkvd,
            "cos2": cos2, "sina": sina, "tri": tri,
        }
        m.update(gmaps[g])
        in_maps.append(m)
    return in_maps, bo_eff, start


def kernel(**inputs) -> np.ndarray:
    in_maps, bo_eff, start = _prep_inputs(**inputs)
    nc = _get_nc(start)
    try:
        res = run_bass_kernel_spmd(nc, in_maps, core_ids=list(range(8)))
    except Exception:
        res = run_bass_kernel_spmd(nc, in_maps, core_ids=list(range(8)))
    out = np.empty((B, S, D), np.float32)
    for b in range(B):
        acc = res.results[2 * b]["outt"] + res.results[2 * b + 1]["outt"]
        out[b] = acc.T + bo_eff[None, :]
    return out


# revision 27
# speedup vs baseline: 1.1012x; 1.0057x over previous
"""Trainium2 Bass kernel for MultiHeadLatentAttention (MLA), 8-core SPMD.

Sharding: data-parallel over batch (4) x tensor-parallel over heads (2).
Core c handles batch c//2 and heads (c%2)*8 .. +8. Each core computes its
partial output projection; the host sums the two TP partials per batch and
adds the (v-bias-folded) output bias.

Device layout is feature-on-partition / token-on-free throughout, so every
projection is a plain matmul chain with no transposes. Attention uses
transposed scores (keys on partitions) so probs feed the AV matmul directly.

v2 notes (vs the 445us baseline):
- phase order qd -> allgather -> kv -> kNope -> q-rms -> v -> qu so the
  collective and both RMS latency chains hide under independent PE work
- rsqrt via exp(-0.5*ln(x)) on the scalar engine and softmax 1/den via the
  custom-DVE reciprocal_approx_fast: one activation table set, no 6.5us
  DVE reciprocals on the critical path
- per-(kc,qt) score PSUM holds BOTH heads of a pair -> single batched exp
- head-pair software pipelining: scores(hp) overlap den/outU/norm(hp-1)
- v-up bias matmuls removed (exact host fold: bo += Wo @ bkvu_v)
- sum-of-squares matmuls in bf16; 128x128 universal triangle mask
"""

import sys
from contextlib import ExitStack

import numpy as np
import ml_dtypes

for _p in ("/opt/trn_rl_repo", "/root/.axon_site/_ro/trn_rl_repo"):
    if _p not in sys.path:
        sys.path.append(_p)

import concourse.bass as bass  # noqa: E402
import concourse.mybir as mybir  # noqa: E402
from concourse import bacc  # noqa: E402
from concourse.bass_utils import run_bass_kernel_spmd  # noqa: E402
from concourse.tile import TileContext  # noqa: E402

# Problem shapes (hardcoded per contract)
B, S, D = 4, 1024, 2048
H = 16
QL, KVL = 1536, 512
NOPE, ROPE, VH = 128, 64, 128
QKH = NOPE + ROPE  # 192
EPS = 1e-6

P = 128
T = S          # tokens per core (one batch)
DC = D // P    # 16 X chunks
QC = QL // P   # 12 q-latent chunks
KC = KVL // P  # 4 kv-latent chunks
HH = H // 2    # 8 heads per core
NKV = KVL + ROPE  # 576
NEG = -1.0e4   # mask bias (exp underflows to exactly 0)

f32 = mybir.dt.float32
bf16 = mybir.dt.bfloat16
AF = mybir.ActivationFunctionType


def build_nc(start: int):
    nc = bacc.Bacc(None, target_bir_lowering=False, debug=False)

    dp = nc.declare_dram_parameter
    xt = dp("xt", [D, T], bf16, isOutput=False)           # X[b].T
    wqdl = dp("wqdl", [D, 6 * P], bf16, isOutput=False)   # local qd W.T
    wkvd = dp("wkvd", [D, 5 * P], bf16, isOutput=False)   # kv down W.T (padded)
    wqu = dp("wqu", [QL, QL], bf16, isOutput=False)        # (perm q) Wqu_eff.T
    wkvu = dp("wkvu", [KVL, 2048], bf16, isOutput=False)   # (perm kv) Wkvu_eff.T
    wo = dp("wo", [HH * VH, D], bf16, isOutput=False)     # Wo[:, slice].T
    bql_i = dp("bql", [P, 6], f32, isOutput=False)        # local qd bias
    bkvd_i = dp("bkvd", [P, 5], f32, isOutput=False)      # kv down bias
    bqu_i = dp("bqu", [P, QC], f32, isOutput=False)       # perm + scale
    bkvuk = dp("bkvuk", [P, HH], f32, isOutput=False)     # kNope part
    cos2 = dp("cos2", [P, T], bf16, isOutput=False)       # duplicated rows
    sina = dp("sina", [P, T], bf16, isOutput=False)       # sign-folded sin
    tri_i = dp("tri", [P, P], bf16, isOutput=False)       # diagonal-band mask
    rotm_i = dp("rotm", [P, P], bf16, isOutput=False)     # rot-half permutation
    outt = dp("outt", [D, T], f32, isOutput=True)

    cc_in = nc.dram_tensor("cc_in", [6 * P, T], bf16)
    cc_out = nc.dram_tensor("cc_out", [12 * P, T], bf16)
    RG = [[0, 1], [2, 3], [4, 5], [6, 7]]

    xt_r = xt.rearrange("(c p) t -> p c t", p=P)
    wqdl_r = wqdl.rearrange("(c p) m -> p c m", p=P)
    wkvd_r = wkvd.rearrange("(c p) m -> p c m", p=P)
    wqu_r = wqu.rearrange("(c p) m -> p c m", p=P)
    wkvu_r = wkvu.rearrange("(c p) m -> p c m", p=P)
    wo_r = wo.rearrange("(c p) m -> p c m", p=P)
    outt_r = outt.rearrange("(c p) t -> p c t", p=P)
    cc_in_r = cc_in.rearrange("(c p) t -> p c t", p=P)
    cc_out_r = cc_out.rearrange("(c p) t -> p c t", p=P)

    with TileContext(nc) as tc, ExitStack() as stk:
        const = stk.enter_context(tc.tile_pool(name="const", bufs=1))
        persist = stk.enter_context(tc.tile_pool(name="persist", bufs=1))

        # ---- constants in SBUF (X goes first; see phase 1) ----
        c_bql = const.tile([P, 6], f32)
        c_bkvd = const.tile([P, 5], f32)
        c_bqu = const.tile([P, QC], f32)
        c_bkvuk = const.tile([P, HH], f32)
        c_tri = const.tile([P, P], bf16)
        c_rotm = const.tile([P, P], bf16)
        c_cos = const.tile([P, T], bf16)
        c_sin = const.tile([P, T], bf16)
        ones_bf = const.tile([P, P], bf16)
        nc.vector.memset(ones_bf[:], 1.0)
        eps_c = const.tile([P, 1], f32)
        nc.vector.memset(eps_c[:], EPS)

        # ---- persistent activations ----
        t_q = persist.tile([P, QC, T], bf16)      # q heads (nope 0-7, rope 8-11)
        t_kn = persist.tile([P, HH, T], bf16)     # kNope[feat, head, tok]
        t_v = persist.tile([P, T // P, HH * P], bf16)  # v[tok, tokchunk, hv]
        t_kr = persist.tile([P, T], bf16)         # kRot, rows duplicated
        t_ao = persist.tile([P, HH, T], bf16)     # attn out [vh, head, tok]
        rq = persist.tile([P, T], f32)            # q rms scale (per token)
        rkv = persist.tile([P, T], f32)           # kv rms scale

        # ====== phases 1+2: projections ======
        with tc.tile_pool(name="ph1", bufs=1) as ph1, \
             tc.tile_pool(name="wstream", bufs=2) as wst, \
             tc.tile_pool(name="wqu_p", bufs=2) as wqp, \
             tc.tile_pool(name="wkvu_p", bufs=2) as wkp, \
             tc.tile_pool(name="tmp", bufs=2) as tmp, \
             tc.tile_pool(name="psA", bufs=6, space="PSUM") as psA, \
             tc.tile_pool(name="psR", bufs=1, space="PSUM") as psR:

            # first two qd weights lead, then X split over sync/scalar with
            # only 1MB on gpsimd: the gpsimd queue must stay light so the
            # collective (stores -> AllGather -> readback) runs early
            t_x = ph1.tile([P, DC, T], bf16, name="t_x")
            w_qd = []
            for m in range(6):
                w_qd.append(wst.tile([P, DC, P], bf16, tag="wqd", bufs=4,
                                     name="w_qd"))
            nc.scalar.dma_start(w_qd[0][:], wqdl_r[:, :, bass.ts(0, P)])
            nc.sync.dma_start(w_qd[1][:], wqdl_r[:, :, bass.ts(1, P)])
            nc.sync.dma_start(t_x[:, 0:6, :], xt_r[:, 0:6, :])
            nc.scalar.dma_start(t_x[:, 6:12, :], xt_r[:, 6:12, :])
            nc.gpsimd.dma_start(t_x[:, 12:16, :], xt_r[:, 12:16, :])
            nc.scalar.dma_start(w_qd[2][:], wqdl_r[:, :, bass.ts(2, P)])
            nc.sync.dma_start(w_qd[3][:], wqdl_r[:, :, bass.ts(3, P)])
            nc.gpsimd.dma_start(c_bql[:], bql_i[:])
            nc.gpsimd.dma_start(c_bkvd[:], bkvd_i[:])
            nc.gpsimd.dma_start(c_bqu[:], bqu_i[:])
            nc.gpsimd.dma_start(c_bkvuk[:], bkvuk[:])
            nc.gpsimd.dma_start(c_tri[:], tri_i[:])
            nc.gpsimd.dma_start(c_rotm[:], rotm_i[:])
            t_kv = ph1.tile([P, 5, T], bf16, name="t_kv")
            t_qd = ph1.tile([P, QC, T], bf16, name="t_qd")

            def down_chain(wt, m_rows, bias_t, bcol, out_ap):
                # out[m_rows, T] = wt.T @ X + bias, as 2 half-token chains
                for tt in range(2):
                    ps = psA.tile([P, 512], f32, tag="ev", name="ps_ev")
                    psm = ps[:m_rows, :]
                    for c in range(DC):
                        nc.tensor.matmul(
                            psm, wt[:, c, :m_rows],
                            t_x[:, c, bass.ts(tt, 512)],
                            start=(c == 0), stop=(c == DC - 1),
                        )
                    nc.vector.tensor_scalar_add(
                        out=out_ap[:m_rows, bass.ts(tt, 512)], in0=psm,
                        scalar1=bias_t[:m_rows, bcol:bcol + 1])

            # ---- q down: local 6 chunks -> exchange -> full 12 in t_qd ----
            # The AllGather output is in global QL order [g0 | g1], so
            # overwriting ALL of t_qd with cc_out leaves every core with the
            # naturally-ordered full latent regardless of its group. The
            # gpsimd queue serializes stores -> collective -> readback.
            for m in range(6):
                if m >= 4:
                    eng = nc.scalar if m % 2 == 0 else nc.sync
                    eng.dma_start(w_qd[m][:], wqdl_r[:, :, bass.ts(m, P)])
                down_chain(w_qd[m], P, c_bql, m, t_qd[:, m, :])
                nc.gpsimd.dma_start(cc_in_r[:, m, :], t_qd[:, m, :])
            nc.gpsimd.collective_compute(
                "AllGather", mybir.AluOpType.bypass,
                replica_groups=RG,
                ins=[cc_in[:]], outs=[cc_out[:]],
            )
            nc.gpsimd.dma_start(t_qd[:, 0:6, :], cc_out_r[:, 0:6, :])
            nc.gpsimd.dma_start(t_qd[:, 6:12, :], cc_out_r[:, 6:12, :])

            # ---- kv down (c 0..3 latent, then rope chunk last) ----
            # the rope-chunk weight is fetched early on sync (own slot) so
            # the rope matmuls never wait behind the rms activations
            wt5 = wst.tile([P, DC, ROPE], bf16, tag="wkv5", bufs=1,
                           name="wt5")
            nc.sync.dma_start(wt5[:, :, :ROPE],
                              wkvd_r[:, :, bass.ds(512, ROPE)])
            for m in range(4):
                wt = wst.tile([P, DC, P], bf16, tag="wqd", bufs=4)
                eng = nc.scalar if m % 2 == 0 else nc.sync
                eng.dma_start(wt[:], wkvd_r[:, :, bass.ts(m, P)])
                down_chain(wt, P, c_bkvd, m, t_kv[:, m, :])
            nc.scalar.dma_start(c_cos[:], cos2[:])
            nc.scalar.dma_start(c_sin[:], sina[:])
            # kv rms (chunks 0..3) -- the sqrt + recip + scale latency chain
            # hides under the rope-chunk matmuls that follow
            ps_ms = psR.tile([P, 2, 512], f32, tag="ms", name="ps_ms")
            for tt in range(2):
                for c in range(KC):
                    sq = tmp.tile([P, 512], bf16, tag="sq")
                    nc.vector.tensor_mul(
                        sq[:], t_kv[:, c, bass.ts(tt, 512)],
                        t_kv[:, c, bass.ts(tt, 512)])
                    nc.tensor.matmul(
                        ps_ms[:, tt, :], ones_bf[:], sq[:],
                        start=(c == 0), stop=(c == KC - 1),
                    )
            # rsqrt(mean+eps): scalar Sqrt then custom-DVE fast reciprocal
            # (no Ln: it lives in a different act table set than Exp and
            # would thrash the table loads)
            with tc.high_priority():
                for tt in range(2):
                    h = bass.ts(tt, 512)
                    nc.scalar.activation(rkv[:, h], ps_ms[:, tt, :],
                                         AF.Sqrt, bias=eps_c[:],
                                         scale=1.0 / KVL)
                    nc.vector.reciprocal_approx_fast(out=rkv[:, h],
                                                     in_=rkv[:, h])
                    for c in range(KC):
                        nc.vector.tensor_mul(
                            t_kv[:, c, h], t_kv[:, c, h], rkv[:, h])
            # rope chunk of kv-down (weight prefetched above)
            down_chain(wt5, ROPE, c_bkvd, 4, t_kv[:, 4, :])
            # RoPE on k (rows duplicated to 128 for the two packed heads);
            # swaps on the sync queue so the DVE is not left waiting
            swp = tmp.tile([P, T], bf16, tag="swp", name="swp",
                           bufs=1)[:ROPE, :]
            nc.sync.dma_start(swp[0:32, :], t_kv[32:64, 4, :])
            nc.sync.dma_start(swp[32:64, :], t_kv[0:32, 4, :])
            nc.vector.tensor_mul(t_kr[0:ROPE, :], t_kv[0:ROPE, 4, :],
                                 c_cos[0:ROPE, :])
            nc.vector.tensor_mul(swp[:], swp[:], c_sin[0:ROPE, :])
            nc.vector.tensor_add(t_kr[0:ROPE, :], t_kr[0:ROPE, :], swp[:])
            nc.sync.dma_start(t_kr[ROPE:P, :], t_kr[0:ROPE, :])

            # ---- kNope up-projection (bias add on scalar engine) ----
            # weights all on sync: the scalar queue head-blocks on the rms
            # activations and would delay them
            kn_w = []
            for m in range(HH):
                wt = wkp.tile([P, KC, P], bf16, tag="wkn", bufs=6,
                              name="kn_w")
                nc.sync.dma_start(wt[:], wkvu_r[:, :, bass.ts(m, P)])
                kn_w.append(wt)
            for m in range(HH):
                wt = kn_w[m]
                for tt in range(2):
                    ps = psA.tile([P, 512], f32, tag="ev", name="ps_kn")
                    for c in range(KC):
                        nc.tensor.matmul(
                            ps, wt[:, c, :],
                            t_kv[:, c, bass.ts(tt, 512)],
                            start=(c == 0), stop=(c == KC - 1),
                        )
                    nc.scalar.activation(
                        t_kn[:, m, bass.ts(tt, 512)], ps, AF.Identity,
                        bias=c_bkvuk[:, m:m + 1])

            # ---- v up-projection (token-on-partition; copies on scalar) ----
            # runs before q-rms so the collective readback of t_qd finishes
            # in its shadow
            for gg in range(2):
                wt = wkp.tile([P, KC, 512], bf16, tag="wv")
                (nc.sync if gg == 0 else nc.scalar).dma_start(
                    wt[:], wkvu_r[:, :, bass.ds(1024 + gg * 512, 512)])
                for tcb in range(8):
                    ps = psA.tile([P, 512], f32, tag="ev", name="ps_v")
                    for c in range(KC):
                        nc.tensor.matmul(
                            ps,
                            t_kv[:, c, bass.ts(tcb, P)],
                            wt[:, c, :],
                            start=(c == 0), stop=(c == KC - 1),
                        )
                    nc.scalar.activation(
                        t_v[:, tcb, bass.ts(gg, 512)], ps, AF.Copy)

            # ---- q rms (t_qd now holds the full gathered latent) ----
            # pinned late on the scheduler's model clock: its DMA/collective
            # model is optimistic, and without the pin it hoists these ops
            # ahead of kv/kNope/v work, head-blocking whichever engine they
            # sit on until the readback really lands
            ps_mq = psR.tile([P, 2, 512], f32, tag="ms", name="ps_mq")
            with tc.tile_wait_until(0.120):
                for tt in range(2):
                    hs = bass.ts(tt, 512)
                    for c in range(QC):
                        sq = tmp.tile([P, 512], bf16, tag="sq")
                        nc.vector.tensor_mul(
                            sq[:], t_qd[:, c, hs], t_qd[:, c, hs])
                        nc.tensor.matmul(
                            ps_mq[:, tt, :], ones_bf[:], sq[:],
                            start=(c == 0), stop=(c == QC - 1),
                        )
                    nc.scalar.activation(rq[:, hs], ps_mq[:, tt, :],
                                         AF.Sqrt, bias=eps_c[:],
                                         scale=1.0 / QL)
                    nc.vector.reciprocal_approx_fast(out=rq[:, hs],
                                                     in_=rq[:, hs])

            # ---- q up-projection ----
            for m in (8, 0, 1, 9, 2, 3, 10, 4, 5, 11, 6, 7):
                wt = wqp.tile([P, QC, P], bf16, tag="wqu")
                eng = nc.scalar if m % 2 == 0 else nc.sync
                eng.dma_start(wt[:], wqu_r[:, :, bass.ts(m, P)])
                for tt in range(2):
                    tsl = bass.ts(tt, 512)
                    ps = psA.tile([P, 512], f32, tag="ev", name="ps_qu")
                    for c in range(QC):
                        nc.tensor.matmul(
                            ps, wt[:, c, :], t_qd[:, c, tsl],
                            start=(c == 0), stop=(c == QC - 1),
                        )
                    if m < 8:
                        qsb = tmp.tile([P, 512], bf16, tag="qsb", bufs=2)
                        nc.vector.tensor_mul(qsb[:], ps, rq[:, tsl])
                        nc.scalar.activation(
                            t_q[:, m, tsl], qsb, AF.Identity,
                            bias=c_bqu[:, m:m + 1],
                        )
                    else:
                        sq = tmp.tile([P, 512], bf16, tag="ropestage",
                                      bufs=2)
                        nc.vector.tensor_mul(sq[:], ps, rq[:, tsl])
                        nc.vector.tensor_scalar_add(
                            out=sq[:], in0=sq, scalar1=c_bqu[:, m:m + 1],
                        )
                        # rotate-half via a PE permutation matmul: frees the
                        # DVE/DMA chain that used to gate the attention start
                        swq = psA.tile([P, 512], f32, tag="ev",
                                       name="ps_rot")
                        nc.tensor.matmul(swq[:], c_rotm[:], sq[:],
                                         start=True, stop=True)
                        nc.vector.tensor_mul(sq[:], sq[:], c_cos[:, tsl])
                        swb = tmp.tile([P, 512], bf16, tag="ropeswap",
                                       bufs=2)
                        nc.vector.tensor_mul(swb[:], swq[:], c_sin[:, tsl])
                        nc.vector.tensor_add(t_q[:, m, tsl], sq[:], swb[:])

        # ====== phase 3: attention (transposed scores, max-free) ======
        def vis_kcs(qt):
            return [kc for kc in range(8)
                    if qt * 512 + 511 >= kc * P - start]

        with tc.tile_pool(name="att", bufs=2) as att, \
             tc.tile_pool(name="psS", bufs=2, space="PSUM") as psS, \
             tc.tile_pool(name="psD", bufs=1, space="PSUM") as psD, \
             tc.tile_pool(name="psU", bufs=2, space="PSUM") as psU:

            def scores_qt(hp, expts2, qt):
                # expts2 [P, head2, kc, q] for heads (2hp, 2hp+1)
                rc = 8 + hp
                for kc in vis_kcs(qt):
                    lo = max(qt * 512, kc * P - start)
                    w = qt * 512 + 512 - lo
                    rel = lo - qt * 512
                    sc2 = psS.tile([P, 2, 512], f32, tag="sc", name="sc2")
                    for h2 in range(2):
                        h = 2 * hp + h2
                        nc.tensor.matmul(
                            sc2[:, h2, rel:],
                            t_kn[:, h, bass.ts(kc, P)],
                            t_q[:, h, bass.ds(lo, w)],
                            start=True, stop=False,
                        )
                    for h2 in range(2):
                        r0 = h2 * ROPE
                        nc.tensor.matmul(
                            sc2[:, h2, rel:],
                            t_kr[r0:r0 + ROPE, bass.ts(kc, P)],
                            t_q[r0:r0 + ROPE, rc, bass.ds(lo, w)],
                            start=False, stop=True,
                        )
                    # partially-masked diagonal band
                    b_lo = max(lo, kc * P - start)
                    b_hi = min(qt * 512 + 512, kc * P - start + P)
                    bw = b_hi - b_lo
                    if bw > 0:
                        j0 = b_lo - (kc * P - start)
                        br = b_lo - qt * 512
                        for h2 in range(2):
                            nc.vector.tensor_add(
                                sc2[:, h2, br:br + bw],
                                sc2[:, h2, br:br + bw],
                                c_tri[:, j0:j0 + bw])
                    nc.scalar.activation(
                        expts2[:, :, kc, bass.ds(lo, w)],
                        sc2[:, :, rel:], AF.Exp)

            def den_outU_head(hp, expts2, h2):
                h = 2 * hp + h2
                den2 = psD.tile([P, 2, 512], f32, name="den2")
                for qt in range(2):
                    kcs = vis_kcs(qt)
                    for i, kc in enumerate(kcs):
                        lo = max(qt * 512, kc * P - start)
                        rel = lo - qt * 512
                        nc.tensor.matmul(
                            den2[:, qt, rel:], ones_bf[:],
                            expts2[:, h2, kc, bass.ds(lo, 512 - rel)],
                            start=(i == 0), stop=(i == len(kcs) - 1),
                        )
                rcp = att.tile([P, 2, 512], f32, tag="rcp", name="rcp")
                nc.vector.reciprocal_approx_fast(
                    out=rcp[:, :, :], in_=den2[:, :, :])
                for qt in range(2):
                    kcs = vis_kcs(qt)
                    outU = psU.tile([P, 512], f32, tag="outU", name="outU")
                    for i, kc in enumerate(kcs):
                        lo = max(qt * 512, kc * P - start)
                        rel = lo - qt * 512
                        nc.tensor.matmul(
                            outU[:, rel:], t_v[:, kc, bass.ts(h, P)],
                            expts2[:, h2, kc, bass.ds(lo, 512 - rel)],
                            start=(i == 0), stop=(i == len(kcs) - 1),
                        )
                    nc.vector.tensor_mul(
                        t_ao[:, h, bass.ts(qt, 512)], outU[:],
                        rcp[:, qt, :])

            # scores(hp) interleave with den/outU of hp-1 at qt granularity
            prev = None
            for hp in range(4):
                cur = att.tile([P, 2, 8, T], bf16, tag="expt", name="expt2")
                scores_qt(hp, cur, 0)
                if prev is not None:
                    den_outU_head(hp - 1, prev, 0)
                scores_qt(hp, cur, 1)
                if prev is not None:
                    den_outU_head(hp - 1, prev, 1)
                prev = cur
            den_outU_head(3, prev, 0)
            den_outU_head(3, prev, 1)

            # ====== phase 4: output projection ======
            for m in range(DC):
                wt = att.tile([P, HH, P], bf16, tag="wo", name="wo_t",
                              bufs=4)
                eng = nc.gpsimd if m % 2 == 0 else nc.sync
                eng.dma_start(wt[:], wo_r[:, :, bass.ts(m, P)])
                for tt in range(2):
                    ps = psU.tile([P, 512], f32, tag="outU", name="ps_o")
                    for c in range(HH):
                        nc.tensor.matmul(
                            ps, wt[:, c, :], t_ao[:, c, bass.ts(tt, 512)],
                            start=(c == 0), stop=(c == HH - 1),
                        )
                    ot = att.tile([P, 512], f32, tag="ot", name="ot",
                                  bufs=3)
                    nc.vector.tensor_copy(ot[:], ps)
                    nc.sync.dma_start(outt_r[:, m, bass.ts(tt, 512)], ot[:])

    nc.compile()
    return nc


_CACHE = {}


def _get_nc(start: int):
    if start not in _CACHE:
        _CACHE[start] = build_nc(start)
    return _CACHE[start]


def _prep_inputs(X, base_freq, Wqd, bqd, gq, Wqu, bqu, Wkv, bkv, gkv,
                 Wkvu, bkvu, Wo, bo, start):
    f = np.float32
    X = np.asarray(X, f)
    base_freq = np.asarray(base_freq, f)
    Wqd = np.asarray(Wqd, f); bqd = np.asarray(bqd, f)
    gq = np.asarray(gq, f); Wqu = np.asarray(Wqu, f); bqu = np.asarray(bqu, f)
    Wkv = np.asarray(Wkv, f); bkv = np.asarray(bkv, f)
    gkv = np.asarray(gkv, f); Wkvu = np.asarray(Wkvu, f)
    bkvu = np.asarray(bkvu, f)
    Wo = np.asarray(Wo, f); bo = np.asarray(bo, f)
    start = int(np.asarray(start).item())
    assert start >= 0

    scale = QKH ** (-0.5)
    bf = ml_dtypes.bfloat16

    # v-bias exact fold: probs sum to 1, so the v bias contributes
    # Wo @ bv to every token's output.
    bv = bkvu.reshape(H, NOPE + VH)[:, NOPE:].reshape(H * VH)
    bo_eff = bo + Wo @ bv

    # qd down W split 6/6 across the TP pair; kv down duplicated
    wqd_t = Wqd.T.astype(f)                                   # (D, QL)
    wkv_t = Wkv.T.astype(f)                                   # (D, NKV)
    wqdl, bql = [], []
    for g in range(2):
        wqdl.append(np.ascontiguousarray(
            wqd_t[:, g * 768:(g + 1) * 768]).astype(bf))
        bql.append(np.ascontiguousarray(
            bqd[g * 768:(g + 1) * 768].reshape(6, P).T))
    wkvd = np.concatenate([wkv_t[:, :576], np.zeros((D, 64), f)], 1)
    wkvd = np.ascontiguousarray(wkvd).astype(bf)
    bkvd_p = np.zeros((5 * P,), f); bkvd_p[:NKV] = bkv
    bkvd = np.ascontiguousarray(bkvd_p.reshape(5, P).T)

    ang = base_freq[:S]                                       # (S, ROPE)
    cos = np.ascontiguousarray(np.cos(ang).T.astype(f))       # (ROPE, S)
    sin = np.ascontiguousarray(np.sin(ang).T.astype(f))
    cos2 = np.ascontiguousarray(
        np.concatenate([cos, cos], 0)).astype(bf)             # (128, S)
    sgn = np.ones((ROPE, 1), f); sgn[:ROPE // 2] = -1.0
    sins = sin * sgn                                          # sign-folded
    sina = np.ascontiguousarray(np.concatenate([sins, sins], 0)).astype(bf)
    # rot-half permutation (unsigned 32<->32 swap inside each 64 block;
    # the sign lives in the sign-folded sin table)
    rotm = np.zeros((P, P), f)
    for i in range(P):
        b, il = i // 64, i % 64
        rotm[b * 64 + (il + 32) % 64, i] = 1.0
    rotm = np.ascontiguousarray(rotm).astype(bf)

    # universal diagonal-band mask: for the block at k = kc*P + p,
    # q = (kc*P - start) + j, visibility is p <= j.
    pp = np.arange(P)
    tri = np.where(pp[:, None] <= pp[None, :], 0.0, NEG).astype(bf)
    tri = np.ascontiguousarray(tri)

    # per head-group tensors
    perm_q = np.concatenate(
        [np.arange(h * QKH, h * QKH + NOPE) for h in range(HH)]
        + [np.arange(h * QKH + NOPE, (h + 1) * QKH) for h in range(HH)]
    )
    perm_kv = np.concatenate(
        [np.arange(h * (NOPE + VH), h * (NOPE + VH) + NOPE) for h in range(HH)]
        + [np.arange(h * (NOPE + VH) + NOPE, (h + 1) * (NOPE + VH))
           for h in range(HH)]
    )
    gmaps = []
    for g in range(2):
        rq = slice(g * HH * QKH, (g + 1) * HH * QKH)
        rkv = slice(g * HH * (NOPE + VH), (g + 1) * HH * (NOPE + VH))
        wqu_g = (Wqu[rq, :] * gq[None, :] * scale)[perm_q]    # (1536, QL)
        bqu_g = (bqu[rq] * scale)[perm_q]
        wkvu_g = (Wkvu[rkv, :] * gkv[None, :])[perm_kv]       # (2048, KVL)
        bkvu_g = bkvu[rkv][perm_kv]
        wo_g = Wo[:, g * HH * VH:(g + 1) * HH * VH]           # (D, 1024)
        gmaps.append({
            "wqu": np.ascontiguousarray(wqu_g.T).astype(bf),
            "bqu": np.ascontiguousarray(bqu_g.reshape(QC, P).T),
            "wkvu": np.ascontiguousarray(wkvu_g.T).astype(bf),
            "bkvuk": np.ascontiguousarray(
                bkvu_g[:HH * NOPE].reshape(HH, P).T),
            "wo": np.ascontiguousarray(wo_g.T).astype(bf),    # (1024, D)
        })

    xts = [np.ascontiguousarray(X[b].T).astype(bf) for b in range(B)]

    in_maps = []
    for c in range(8):
        b, g = c // 2, c % 2
        m = {
            "xt": xts[b], "wqdl": wqdl[g], "bql": bql[g],
            "wkvd": wkvd, "bkvd": bkvd,
            "cos2": cos2, "sina": sina, "tri": tri, "rotm": rotm,
        }
        m.update(gmaps[g])
        in_maps.append(m)
    return in_maps, bo_eff, start


def kernel(**inputs) -> np.ndarray:
    in_maps, bo_eff, start = _prep_inputs(**inputs)
    nc = _get_nc(start)
    try:
        res = run_bass_kernel_spmd(nc, in_maps, core_ids=list(range(8)))
    except Exception:
        res = run_bass_kernel_spmd(nc, in_maps, core_ids=list(range(8)))
    out = np.empty((B, S, D), np.float32)
    for b in range(B):
        acc = res.results[2 * b]["outt"] + res.results[2 * b + 1]["outt"]
        out[b] = acc.T + bo_eff[None, :]
    return out


# revision 28
# speedup vs baseline: 1.1168x; 1.0141x over previous
"""Trainium2 Bass kernel for MultiHeadLatentAttention (MLA), 8-core SPMD.

Sharding: data-parallel over batch (4) x tensor-parallel over heads (2).
Core c handles batch c//2 and heads (c%2)*8 .. +8. Each core computes its
partial output projection; the host sums the two TP partials per batch and
adds the (v-bias-folded) output bias.

Device layout is feature-on-partition / token-on-free throughout, so every
projection is a plain matmul chain with no transposes. Attention uses
transposed scores (keys on partitions) so probs feed the AV matmul directly.

v2 notes (vs the 445us baseline):
- phase order qd -> allgather -> kv -> kNope -> q-rms -> v -> qu so the
  collective and both RMS latency chains hide under independent PE work
- rsqrt via exp(-0.5*ln(x)) on the scalar engine and softmax 1/den via the
  custom-DVE reciprocal_approx_fast: one activation table set, no 6.5us
  DVE reciprocals on the critical path
- per-(kc,qt) score PSUM holds BOTH heads of a pair -> single batched exp
- head-pair software pipelining: scores(hp) overlap den/outU/norm(hp-1)
- v-up bias matmuls removed (exact host fold: bo += Wo @ bkvu_v)
- sum-of-squares matmuls in bf16; 128x128 universal triangle mask
"""

import sys
from contextlib import ExitStack

import numpy as np
import ml_dtypes

for _p in ("/opt/trn_rl_repo", "/root/.axon_site/_ro/trn_rl_repo"):
    if _p not in sys.path:
        sys.path.append(_p)

import concourse.bass as bass  # noqa: E402
import concourse.mybir as mybir  # noqa: E402
from concourse import bacc  # noqa: E402
from concourse.bass_utils import run_bass_kernel_spmd  # noqa: E402
from concourse.tile import TileContext  # noqa: E402

# Problem shapes (hardcoded per contract)
B, S, D = 4, 1024, 2048
H = 16
QL, KVL = 1536, 512
NOPE, ROPE, VH = 128, 64, 128
QKH = NOPE + ROPE  # 192
EPS = 1e-6

P = 128
T = S          # tokens per core (one batch)
DC = D // P    # 16 X chunks
QC = QL // P   # 12 q-latent chunks
KC = KVL // P  # 4 kv-latent chunks
HH = H // 2    # 8 heads per core
NKV = KVL + ROPE  # 576
NEG = -1.0e4   # mask bias (exp underflows to exactly 0)

f32 = mybir.dt.float32
bf16 = mybir.dt.bfloat16
AF = mybir.ActivationFunctionType


def build_nc(start: int):
    nc = bacc.Bacc(None, target_bir_lowering=False, debug=False)

    dp = nc.declare_dram_parameter
    xt = dp("xt", [D, T], bf16, isOutput=False)           # X[b].T
    wqdl = dp("wqdl", [D, 6 * P], bf16, isOutput=False)   # local qd W.T
    wkvd = dp("wkvd", [D, 5 * P], bf16, isOutput=False)   # kv down W.T (padded)
    wqu = dp("wqu", [QL, QL], bf16, isOutput=False)        # (perm q) Wqu_eff.T
    wkvu = dp("wkvu", [KVL, 2048], bf16, isOutput=False)   # (perm kv) Wkvu_eff.T
    wo = dp("wo", [HH * VH, D], bf16, isOutput=False)     # Wo[:, slice].T
    bql_i = dp("bql", [P, 6], f32, isOutput=False)        # local qd bias
    bkvd_i = dp("bkvd", [P, 5], f32, isOutput=False)      # kv down bias
    bqu_i = dp("bqu", [P, QC], f32, isOutput=False)       # perm + scale
    bkvuk = dp("bkvuk", [P, HH], f32, isOutput=False)     # kNope part
    cos2 = dp("cos2", [P, T], bf16, isOutput=False)       # duplicated rows
    sina = dp("sina", [P, T], bf16, isOutput=False)       # sign-folded sin
    tri_i = dp("tri", [P, P], bf16, isOutput=False)       # diagonal-band mask
    rotm_i = dp("rotm", [P, P], bf16, isOutput=False)     # rot-half permutation
    outt = dp("outt", [D, T], f32, isOutput=True)

    cc_in = nc.dram_tensor("cc_in", [6 * P, T], bf16)
    cc_out = nc.dram_tensor("cc_out", [12 * P, T], bf16)
    RG = [[0, 1], [2, 3], [4, 5], [6, 7]]

    xt_r = xt.rearrange("(c p) t -> p c t", p=P)
    wqdl_r = wqdl.rearrange("(c p) m -> p c m", p=P)
    wkvd_r = wkvd.rearrange("(c p) m -> p c m", p=P)
    wqu_r = wqu.rearrange("(c p) m -> p c m", p=P)
    wkvu_r = wkvu.rearrange("(c p) m -> p c m", p=P)
    wo_r = wo.rearrange("(c p) m -> p c m", p=P)
    outt_r = outt.rearrange("(c p) t -> p c t", p=P)
    cc_in_r = cc_in.rearrange("(c p) t -> p c t", p=P)
    cc_out_r = cc_out.rearrange("(c p) t -> p c t", p=P)

    with TileContext(nc) as tc, ExitStack() as stk:
        const = stk.enter_context(tc.tile_pool(name="const", bufs=1))
        persist = stk.enter_context(tc.tile_pool(name="persist", bufs=1))

        # ---- constants in SBUF (X goes first; see phase 1) ----
        c_bql = const.tile([P, 6], f32)
        c_bkvd = const.tile([P, 5], f32)
        c_bqu = const.tile([P, QC], f32)
        c_bkvuk = const.tile([P, HH], f32)
        c_tri = const.tile([P, P], bf16)
        c_rotm = const.tile([P, P], bf16)
        c_cos = const.tile([P, T], bf16)
        c_sin = const.tile([P, T], bf16)
        ones_bf = const.tile([P, P], bf16)
        nc.vector.memset(ones_bf[:], 1.0)
        eps_c = const.tile([P, 1], f32)
        nc.vector.memset(eps_c[:], EPS)

        # ---- persistent activations ----
        t_q = persist.tile([P, QC, T], bf16)      # q heads (nope 0-7, rope 8-11)
        t_kn = persist.tile([P, HH, T], bf16)     # kNope[feat, head, tok]
        t_v = persist.tile([P, T // P, HH * P], bf16)  # v[tok, tokchunk, hv]
        t_kr = persist.tile([P, T], bf16)         # kRot, rows duplicated
        t_ao = persist.tile([P, HH, T], bf16)     # attn out [vh, head, tok]
        rq = persist.tile([P, T], f32)            # q rms scale (per token)
        rkv = persist.tile([P, T], f32)           # kv rms scale

        # ====== phases 1+2: projections ======
        with tc.tile_pool(name="ph1", bufs=1) as ph1, \
             tc.tile_pool(name="wstream", bufs=2) as wst, \
             tc.tile_pool(name="wqu_p", bufs=2) as wqp, \
             tc.tile_pool(name="wkvu_p", bufs=2) as wkp, \
             tc.tile_pool(name="tmp", bufs=2) as tmp, \
             tc.tile_pool(name="psA", bufs=6, space="PSUM") as psA, \
             tc.tile_pool(name="psR", bufs=1, space="PSUM") as psR:

            # first two qd weights lead, then X split over sync/scalar with
            # only 1MB on gpsimd: the gpsimd queue must stay light so the
            # collective (stores -> AllGather -> readback) runs early
            t_x = ph1.tile([P, DC, T], bf16, name="t_x")
            w_qd = []
            for m in range(6):
                w_qd.append(wst.tile([P, DC, P], bf16, tag="wqd", bufs=4,
                                     name="w_qd"))
            nc.scalar.dma_start(w_qd[0][:], wqdl_r[:, :, bass.ts(0, P)])
            nc.sync.dma_start(w_qd[1][:], wqdl_r[:, :, bass.ts(1, P)])
            nc.sync.dma_start(t_x[:, 0:6, :], xt_r[:, 0:6, :])
            nc.scalar.dma_start(t_x[:, 6:12, :], xt_r[:, 6:12, :])
            nc.gpsimd.dma_start(t_x[:, 12:16, :], xt_r[:, 12:16, :])
            nc.scalar.dma_start(w_qd[2][:], wqdl_r[:, :, bass.ts(2, P)])
            nc.sync.dma_start(w_qd[3][:], wqdl_r[:, :, bass.ts(3, P)])
            nc.gpsimd.dma_start(c_bql[:], bql_i[:])
            nc.gpsimd.dma_start(c_bkvd[:], bkvd_i[:])
            nc.gpsimd.dma_start(c_bqu[:], bqu_i[:])
            nc.gpsimd.dma_start(c_bkvuk[:], bkvuk[:])
            nc.gpsimd.dma_start(c_tri[:], tri_i[:])
            nc.gpsimd.dma_start(c_rotm[:], rotm_i[:])
            t_kv = ph1.tile([P, 5, T], bf16, name="t_kv")
            t_qd = ph1.tile([P, QC, T], bf16, name="t_qd")

            def down_chain(wt, m_rows, bias_t, bcol, out_ap):
                # out[m_rows, T] = wt.T @ X + bias, as 2 half-token chains
                for tt in range(2):
                    ps = psA.tile([P, 512], f32, tag="ev", name="ps_ev")
                    psm = ps[:m_rows, :]
                    for c in range(DC):
                        nc.tensor.matmul(
                            psm, wt[:, c, :m_rows],
                            t_x[:, c, bass.ts(tt, 512)],
                            start=(c == 0), stop=(c == DC - 1),
                        )
                    nc.vector.tensor_scalar_add(
                        out=out_ap[:m_rows, bass.ts(tt, 512)], in0=psm,
                        scalar1=bias_t[:m_rows, bcol:bcol + 1])

            # ---- q down: local 6 chunks -> exchange -> full 12 in t_qd ----
            # The AllGather output is in global QL order [g0 | g1], so
            # overwriting ALL of t_qd with cc_out leaves every core with the
            # naturally-ordered full latent regardless of its group. The
            # gpsimd queue serializes stores -> collective -> readback.
            for m in range(6):
                if m >= 4:
                    eng = nc.scalar if m % 2 == 0 else nc.sync
                    eng.dma_start(w_qd[m][:], wqdl_r[:, :, bass.ts(m, P)])
                down_chain(w_qd[m], P, c_bql, m, t_qd[:, m, :])
                nc.gpsimd.dma_start(cc_in_r[:, m, :], t_qd[:, m, :])
            nc.gpsimd.collective_compute(
                "AllGather", mybir.AluOpType.bypass,
                replica_groups=RG,
                ins=[cc_in[:]], outs=[cc_out[:]],
            )
            nc.gpsimd.dma_start(t_qd[:, 0:6, :], cc_out_r[:, 0:6, :])
            nc.gpsimd.dma_start(t_qd[:, 6:12, :], cc_out_r[:, 6:12, :])

            # ---- kv down (c 0..3 latent, then rope chunk last) ----
            # the rope-chunk weight is fetched early on sync (own slot) so
            # the rope matmuls never wait behind the rms activations
            wt5 = wst.tile([P, DC, ROPE], bf16, tag="wkv5", bufs=1,
                           name="wt5")
            nc.sync.dma_start(wt5[:, :, :ROPE],
                              wkvd_r[:, :, bass.ds(512, ROPE)])
            for m in range(4):
                wt = wst.tile([P, DC, P], bf16, tag="wqd", bufs=4)
                eng = nc.scalar if m % 2 == 0 else nc.sync
                eng.dma_start(wt[:], wkvd_r[:, :, bass.ts(m, P)])
                down_chain(wt, P, c_bkvd, m, t_kv[:, m, :])
            nc.scalar.dma_start(c_cos[:], cos2[:])
            nc.scalar.dma_start(c_sin[:], sina[:])
            # kv rms (chunks 0..3) -- the sqrt + recip + scale latency chain
            # hides under the rope-chunk matmuls that follow
            ps_ms = psR.tile([P, 2, 512], f32, tag="ms", name="ps_ms")
            for tt in range(2):
                for c in range(KC):
                    sq = tmp.tile([P, 512], bf16, tag="sq")
                    nc.vector.tensor_mul(
                        sq[:], t_kv[:, c, bass.ts(tt, 512)],
                        t_kv[:, c, bass.ts(tt, 512)])
                    nc.tensor.matmul(
                        ps_ms[:, tt, :], ones_bf[:], sq[:],
                        start=(c == 0), stop=(c == KC - 1),
                    )
            # rsqrt(mean+eps): scalar Sqrt then custom-DVE fast reciprocal
            # (no Ln: it lives in a different act table set than Exp and
            # would thrash the table loads)
            with tc.high_priority():
                for tt in range(2):
                    h = bass.ts(tt, 512)
                    nc.scalar.activation(rkv[:, h], ps_ms[:, tt, :],
                                         AF.Sqrt, bias=eps_c[:],
                                         scale=1.0 / KVL)
                    nc.vector.reciprocal_approx_fast(out=rkv[:, h],
                                                     in_=rkv[:, h])
                    for c in range(KC):
                        nc.vector.tensor_mul(
                            t_kv[:, c, h], t_kv[:, c, h], rkv[:, h])
            # rope chunk of kv-down (weight prefetched above)
            down_chain(wt5, ROPE, c_bkvd, 4, t_kv[:, 4, :])
            # RoPE on k (rows duplicated to 128 for the two packed heads);
            # swaps on the sync queue so the DVE is not left waiting
            swp = tmp.tile([P, T], bf16, tag="swp", name="swp",
                           bufs=1)[:ROPE, :]
            nc.sync.dma_start(swp[0:32, :], t_kv[32:64, 4, :])
            nc.sync.dma_start(swp[32:64, :], t_kv[0:32, 4, :])
            nc.vector.tensor_mul(t_kr[0:ROPE, :], t_kv[0:ROPE, 4, :],
                                 c_cos[0:ROPE, :])
            nc.vector.tensor_mul(swp[:], swp[:], c_sin[0:ROPE, :])
            nc.vector.tensor_add(t_kr[0:ROPE, :], t_kr[0:ROPE, :], swp[:])
            nc.sync.dma_start(t_kr[ROPE:P, :], t_kr[0:ROPE, :])

            # ---- kNope up-projection (bias add on scalar engine) ----
            # weights all on sync: the scalar queue head-blocks on the rms
            # activations and would delay them
            kn_w = []
            for m in range(HH):
                wt = wkp.tile([P, KC, P], bf16, tag="wkn", bufs=8,
                              name="kn_w")
                nc.sync.dma_start(wt[:], wkvu_r[:, :, bass.ts(m, P)])
                kn_w.append(wt)
            for m in range(HH):
                wt = kn_w[m]
                for tt in range(2):
                    ps = psA.tile([P, 512], f32, tag="ev", name="ps_kn")
                    for c in range(KC):
                        nc.tensor.matmul(
                            ps, wt[:, c, :],
                            t_kv[:, c, bass.ts(tt, 512)],
                            start=(c == 0), stop=(c == KC - 1),
                        )
                    nc.scalar.activation(
                        t_kn[:, m, bass.ts(tt, 512)], ps, AF.Identity,
                        bias=c_bkvuk[:, m:m + 1])

            # ---- v up-projection (token-on-partition; copies on scalar) ----
            # runs before q-rms so the collective readback of t_qd finishes
            # in its shadow
            for gg in range(2):
                wt = wkp.tile([P, KC, 512], bf16, tag="wv")
                (nc.sync if gg == 0 else nc.scalar).dma_start(
                    wt[:], wkvu_r[:, :, bass.ds(1024 + gg * 512, 512)])
                for tcb in range(8):
                    ps = psA.tile([P, 512], f32, tag="ev", name="ps_v")
                    for c in range(KC):
                        nc.tensor.matmul(
                            ps,
                            t_kv[:, c, bass.ts(tcb, P)],
                            wt[:, c, :],
                            start=(c == 0), stop=(c == KC - 1),
                        )
                    nc.scalar.activation(
                        t_v[:, tcb, bass.ts(gg, 512)], ps, AF.Copy)

            # ---- q rms (t_qd now holds the full gathered latent) ----
            # pinned late on the scheduler's model clock: its DMA/collective
            # model is optimistic, and without the pin it hoists these ops
            # ahead of kv/kNope/v work, head-blocking whichever engine they
            # sit on until the readback really lands
            ps_mq = psR.tile([P, 2, 512], f32, tag="ms", name="ps_mq")
            with tc.tile_wait_until(0.120):
                for tt in range(2):
                    hs = bass.ts(tt, 512)
                    for c in range(QC):
                        sq = tmp.tile([P, 512], bf16, tag="sq")
                        nc.vector.tensor_mul(
                            sq[:], t_qd[:, c, hs], t_qd[:, c, hs])
                        nc.tensor.matmul(
                            ps_mq[:, tt, :], ones_bf[:], sq[:],
                            start=(c == 0), stop=(c == QC - 1),
                        )
                    nc.scalar.activation(rq[:, hs], ps_mq[:, tt, :],
                                         AF.Sqrt, bias=eps_c[:],
                                         scale=1.0 / QL)
                    nc.vector.reciprocal_approx_fast(out=rq[:, hs],
                                                     in_=rq[:, hs])

            # ---- q up-projection ----
            # post-processing of chunk m's psums is issued after chunk m+1's
            # matmul chains, so the PE never waits on the DVE stage tiles
            def qu_post(m, ps, tt):
                tsl = bass.ts(tt, 512)
                if m < 8:
                    qsb = tmp.tile([P, 512], bf16, tag="qsb", bufs=2)
                    nc.vector.tensor_mul(qsb[:], ps, rq[:, tsl])
                    nc.scalar.activation(
                        t_q[:, m, tsl], qsb, AF.Identity,
                        bias=c_bqu[:, m:m + 1],
                    )
                else:
                    sq = tmp.tile([P, 512], bf16, tag="ropestage",
                                  bufs=2)
                    nc.vector.tensor_mul(sq[:], ps, rq[:, tsl])
                    nc.vector.tensor_scalar_add(
                        out=sq[:], in0=sq, scalar1=c_bqu[:, m:m + 1],
                    )
                    # rotate-half via a PE permutation matmul
                    swq = psA.tile([P, 512], f32, tag="ev", name="ps_rot")
                    nc.tensor.matmul(swq[:], c_rotm[:], sq[:],
                                     start=True, stop=True)
                    nc.vector.tensor_mul(sq[:], sq[:], c_cos[:, tsl])
                    swb = tmp.tile([P, 512], bf16, tag="ropeswap",
                                   bufs=2)
                    nc.vector.tensor_mul(swb[:], swq[:], c_sin[:, tsl])
                    nc.vector.tensor_add(t_q[:, m, tsl], sq[:], swb[:])

            pend = None
            for m in (8, 0, 1, 9, 2, 3, 10, 4, 5, 11, 6, 7):
                wt = wqp.tile([P, QC, P], bf16, tag="wqu")
                eng = nc.scalar if m % 2 == 0 else nc.sync
                eng.dma_start(wt[:], wqu_r[:, :, bass.ts(m, P)])
                cur = []
                for tt in range(2):
                    tsl = bass.ts(tt, 512)
                    ps = psA.tile([P, 512], f32, tag="ev", name="ps_qu")
                    for c in range(QC):
                        nc.tensor.matmul(
                            ps, wt[:, c, :], t_qd[:, c, tsl],
                            start=(c == 0), stop=(c == QC - 1),
                        )
                    cur.append(ps)
                if pend is not None:
                    pm, pps = pend
                    for tt in range(2):
                        qu_post(pm, pps[tt], tt)
                pend = (m, cur)
            pm, pps = pend
            for tt in range(2):
                qu_post(pm, pps[tt], tt)

        # ====== phase 3: attention (transposed scores, max-free) ======
        def vis_kcs(qt):
            return [kc for kc in range(8)
                    if qt * 512 + 511 >= kc * P - start]

        with tc.tile_pool(name="att", bufs=2) as att, \
             tc.tile_pool(name="psS", bufs=2, space="PSUM") as psS, \
             tc.tile_pool(name="psD", bufs=1, space="PSUM") as psD, \
             tc.tile_pool(name="psU", bufs=2, space="PSUM") as psU:

            def scores_qt(hp, expts2, qt):
                # expts2 [P, head2, kc, q] for heads (2hp, 2hp+1)
                rc = 8 + hp
                for kc in vis_kcs(qt):
                    lo = max(qt * 512, kc * P - start)
                    w = qt * 512 + 512 - lo
                    rel = lo - qt * 512
                    sc2 = psS.tile([P, 2, 512], f32, tag="sc", name="sc2")
                    for h2 in range(2):
                        h = 2 * hp + h2
                        nc.tensor.matmul(
                            sc2[:, h2, rel:],
                            t_kn[:, h, bass.ts(kc, P)],
                            t_q[:, h, bass.ds(lo, w)],
                            start=True, stop=False,
                        )
                    for h2 in range(2):
                        r0 = h2 * ROPE
                        nc.tensor.matmul(
                            sc2[:, h2, rel:],
                            t_kr[r0:r0 + ROPE, bass.ts(kc, P)],
                            t_q[r0:r0 + ROPE, rc, bass.ds(lo, w)],
                            start=False, stop=True,
                        )
                    # partially-masked diagonal band
                    b_lo = max(lo, kc * P - start)
                    b_hi = min(qt * 512 + 512, kc * P - start + P)
                    bw = b_hi - b_lo
                    if bw > 0:
                        j0 = b_lo - (kc * P - start)
                        br = b_lo - qt * 512
                        for h2 in range(2):
                            nc.vector.tensor_add(
                                sc2[:, h2, br:br + bw],
                                sc2[:, h2, br:br + bw],
                                c_tri[:, j0:j0 + bw])
                    nc.scalar.activation(
                        expts2[:, :, kc, bass.ds(lo, w)],
                        sc2[:, :, rel:], AF.Exp)

            def den_outU_head(hp, expts2, h2):
                h = 2 * hp + h2
                den2 = psD.tile([P, 2, 512], f32, name="den2")
                for qt in range(2):
                    kcs = vis_kcs(qt)
                    for i, kc in enumerate(kcs):
                        lo = max(qt * 512, kc * P - start)
                        rel = lo - qt * 512
                        nc.tensor.matmul(
                            den2[:, qt, rel:], ones_bf[:],
                            expts2[:, h2, kc, bass.ds(lo, 512 - rel)],
                            start=(i == 0), stop=(i == len(kcs) - 1),
                        )
                rcp = att.tile([P, 2, 512], f32, tag="rcp", name="rcp")
                nc.vector.reciprocal_approx_fast(
                    out=rcp[:, :, :], in_=den2[:, :, :])
                for qt in range(2):
                    kcs = vis_kcs(qt)
                    outU = psU.tile([P, 512], f32, tag="outU", name="outU")
                    for i, kc in enumerate(kcs):
                        lo = max(qt * 512, kc * P - start)
                        rel = lo - qt * 512
                        nc.tensor.matmul(
                            outU[:, rel:], t_v[:, kc, bass.ts(h, P)],
                            expts2[:, h2, kc, bass.ds(lo, 512 - rel)],
                            start=(i == 0), stop=(i == len(kcs) - 1),
                        )
                    nc.vector.tensor_mul(
                        t_ao[:, h, bass.ts(qt, 512)], outU[:],
                        rcp[:, qt, :])

            # scores(hp) interleave with den/outU of hp-1 at qt granularity
            prev = None
            for hp in range(4):
                cur = att.tile([P, 2, 8, T], bf16, tag="expt", name="expt2")
                scores_qt(hp, cur, 0)
                if prev is not None:
                    den_outU_head(hp - 1, prev, 0)
                scores_qt(hp, cur, 1)
                if prev is not None:
                    den_outU_head(hp - 1, prev, 1)
                prev = cur
            den_outU_head(3, prev, 0)
            den_outU_head(3, prev, 1)

            # ====== phase 4: output projection ======
            for m in range(DC):
                wt = att.tile([P, HH, P], bf16, tag="wo", name="wo_t",
                              bufs=4)
                eng = nc.gpsimd if m % 2 == 0 else nc.sync
                eng.dma_start(wt[:], wo_r[:, :, bass.ts(m, P)])
                for tt in range(2):
                    ps = psU.tile([P, 512], f32, tag="outU", name="ps_o")
                    for c in range(HH):
                        nc.tensor.matmul(
                            ps, wt[:, c, :], t_ao[:, c, bass.ts(tt, 512)],
                            start=(c == 0), stop=(c == HH - 1),
                        )
                    ot = att.tile([P, 512], f32, tag="ot", name="ot",
                                  bufs=3)
                    nc.vector.tensor_copy(ot[:], ps)
                    nc.sync.dma_start(outt_r[:, m, bass.ts(tt, 512)], ot[:])

    nc.compile()
    return nc


_CACHE = {}


def _get_nc(start: int):
    if start not in _CACHE:
        _CACHE[start] = build_nc(start)
    return _CACHE[start]


def _prep_inputs(X, base_freq, Wqd, bqd, gq, Wqu, bqu, Wkv, bkv, gkv,
                 Wkvu, bkvu, Wo, bo, start):
    f = np.float32
    X = np.asarray(X, f)
    base_freq = np.asarray(base_freq, f)
    Wqd = np.asarray(Wqd, f); bqd = np.asarray(bqd, f)
    gq = np.asarray(gq, f); Wqu = np.asarray(Wqu, f); bqu = np.asarray(bqu, f)
    Wkv = np.asarray(Wkv, f); bkv = np.asarray(bkv, f)
    gkv = np.asarray(gkv, f); Wkvu = np.asarray(Wkvu, f)
    bkvu = np.asarray(bkvu, f)
    Wo = np.asarray(Wo, f); bo = np.asarray(bo, f)
    start = int(np.asarray(start).item())
    assert start >= 0

    scale = QKH ** (-0.5)
    bf = ml_dtypes.bfloat16

    # v-bias exact fold: probs sum to 1, so the v bias contributes
    # Wo @ bv to every token's output.
    bv = bkvu.reshape(H, NOPE + VH)[:, NOPE:].reshape(H * VH)
    bo_eff = bo + Wo @ bv

    # qd down W split 6/6 across the TP pair; kv down duplicated
    wqd_t = Wqd.T.astype(f)                                   # (D, QL)
    wkv_t = Wkv.T.astype(f)                                   # (D, NKV)
    wqdl, bql = [], []
    for g in range(2):
        wqdl.append(np.ascontiguousarray(
            wqd_t[:, g * 768:(g + 1) * 768]).astype(bf))
        bql.append(np.ascontiguousarray(
            bqd[g * 768:(g + 1) * 768].reshape(6, P).T))
    wkvd = np.concatenate([wkv_t[:, :576], np.zeros((D, 64), f)], 1)
    wkvd = np.ascontiguousarray(wkvd).astype(bf)
    bkvd_p = np.zeros((5 * P,), f); bkvd_p[:NKV] = bkv
    bkvd = np.ascontiguousarray(bkvd_p.reshape(5, P).T)

    ang = base_freq[:S]                                       # (S, ROPE)
    cos = np.ascontiguousarray(np.cos(ang).T.astype(f))       # (ROPE, S)
    sin = np.ascontiguousarray(np.sin(ang).T.astype(f))
    cos2 = np.ascontiguousarray(
        np.concatenate([cos, cos], 0)).astype(bf)             # (128, S)
    sgn = np.ones((ROPE, 1), f); sgn[:ROPE // 2] = -1.0
    sins = sin * sgn                                          # sign-folded
    sina = np.ascontiguousarray(np.concatenate([sins, sins], 0)).astype(bf)
    # rot-half permutation (unsigned 32<->32 swap inside each 64 block;
    # the sign lives in the sign-folded sin table)
    rotm = np.zeros((P, P), f)
    for i in range(P):
        b, il = i // 64, i % 64
        rotm[b * 64 + (il + 32) % 64, i] = 1.0
    rotm = np.ascontiguousarray(rotm).astype(bf)

    # universal diagonal-band mask: for the block at k = kc*P + p,
    # q = (kc*P - start) + j, visibility is p <= j.
    pp = np.arange(P)
    tri = np.where(pp[:, None] <= pp[None, :], 0.0, NEG).astype(bf)
    tri = np.ascontiguousarray(tri)

    # per head-group tensors
    perm_q = np.concatenate(
        [np.arange(h * QKH, h * QKH + NOPE) for h in range(HH)]
        + [np.arange(h * QKH + NOPE, (h + 1) * QKH) for h in range(HH)]
    )
    perm_kv = np.concatenate(
        [np.arange(h * (NOPE + VH), h * (NOPE + VH) + NOPE) for h in range(HH)]
        + [np.arange(h * (NOPE + VH) + NOPE, (h + 1) * (NOPE + VH))
           for h in range(HH)]
    )
    gmaps = []
    for g in range(2):
        rq = slice(g * HH * QKH, (g + 1) * HH * QKH)
        rkv = slice(g * HH * (NOPE + VH), (g + 1) * HH * (NOPE + VH))
        wqu_g = (Wqu[rq, :] * gq[None, :] * scale)[perm_q]    # (1536, QL)
        bqu_g = (bqu[rq] * scale)[perm_q]
        wkvu_g = (Wkvu[rkv, :] * gkv[None, :])[perm_kv]       # (2048, KVL)
        bkvu_g = bkvu[rkv][perm_kv]
        wo_g = Wo[:, g * HH * VH:(g + 1) * HH * VH]           # (D, 1024)
        gmaps.append({
            "wqu": np.ascontiguousarray(wqu_g.T).astype(bf),
            "bqu": np.ascontiguousarray(bqu_g.reshape(QC, P).T),
            "wkvu": np.ascontiguousarray(wkvu_g.T).astype(bf),
            "bkvuk": np.ascontiguousarray(
                bkvu_g[:HH * NOPE].reshape(HH, P).T),
            "wo": np.ascontiguousarray(wo_g.T).astype(bf),    # (1024, D)
        })

    xts = [np.ascontiguousarray(X[b].T).astype(bf) for b in range(B)]

    in_maps = []
    for c in range(8):
        b, g = c // 2, c % 2
        m = {
            "xt": xts[b], "wqdl": wqdl[g], "bql": bql[g],
            "wkvd": wkvd, "bkvd": bkvd,
            "cos2": cos2, "sina": sina, "tri": tri, "rotm": rotm,
        }
        m.update(gmaps[g])
        in_maps.append(m)
    return in_maps, bo_eff, start


def kernel(**inputs) -> np.ndarray:
    in_maps, bo_eff, start = _prep_inputs(**inputs)
    nc = _get_nc(start)
    try:
        res = run_bass_kernel_spmd(nc, in_maps, core_ids=list(range(8)))
    except Exception:
        res = run_bass_kernel_spmd(nc, in_maps, core_ids=list(range(8)))
    out = np.empty((B, S, D), np.float32)
    for b in range(B):
        acc = res.results[2 * b]["outt"] + res.results[2 * b + 1]["outt"]
        out[b] = acc.T + bo_eff[None, :]
    return out


# revision 29
# speedup vs baseline: 1.1214x; 1.0041x over previous
"""Trainium2 Bass kernel for MultiHeadLatentAttention (MLA), 8-core SPMD.

Sharding: data-parallel over batch (4) x tensor-parallel over heads (2).
Core c handles batch c//2 and heads (c%2)*8 .. +8. Each core computes its
partial output projection; the host sums the two TP partials per batch and
adds the (v-bias-folded) output bias.

Device layout is feature-on-partition / token-on-free throughout, so every
projection is a plain matmul chain with no transposes. Attention uses
transposed scores (keys on partitions) so probs feed the AV matmul directly.

v2 notes (vs the 445us baseline):
- phase order qd -> allgather -> kv -> kNope -> q-rms -> v -> qu so the
  collective and both RMS latency chains hide under independent PE work
- rsqrt via exp(-0.5*ln(x)) on the scalar engine and softmax 1/den via the
  custom-DVE reciprocal_approx_fast: one activation table set, no 6.5us
  DVE reciprocals on the critical path
- per-(kc,qt) score PSUM holds BOTH heads of a pair -> single batched exp
- head-pair software pipelining: scores(hp) overlap den/outU/norm(hp-1)
- v-up bias matmuls removed (exact host fold: bo += Wo @ bkvu_v)
- sum-of-squares matmuls in bf16; 128x128 universal triangle mask
"""

import sys
from contextlib import ExitStack

import numpy as np
import ml_dtypes

for _p in ("/opt/trn_rl_repo", "/root/.axon_site/_ro/trn_rl_repo"):
    if _p not in sys.path:
        sys.path.append(_p)

import concourse.bass as bass  # noqa: E402
import concourse.mybir as mybir  # noqa: E402
from concourse import bacc  # noqa: E402
from concourse.bass_utils import run_bass_kernel_spmd  # noqa: E402
from concourse.tile import TileContext  # noqa: E402

# Problem shapes (hardcoded per contract)
B, S, D = 4, 1024, 2048
H = 16
QL, KVL = 1536, 512
NOPE, ROPE, VH = 128, 64, 128
QKH = NOPE + ROPE  # 192
EPS = 1e-6

P = 128
T = S          # tokens per core (one batch)
DC = D // P    # 16 X chunks
QC = QL // P   # 12 q-latent chunks
KC = KVL // P  # 4 kv-latent chunks
HH = H // 2    # 8 heads per core
NKV = KVL + ROPE  # 576
NEG = -1.0e4   # mask bias (exp underflows to exactly 0)

f32 = mybir.dt.float32
bf16 = mybir.dt.bfloat16
AF = mybir.ActivationFunctionType


def build_nc(start: int):
    nc = bacc.Bacc(None, target_bir_lowering=False, debug=False)

    dp = nc.declare_dram_parameter
    xt = dp("xt", [D, T], bf16, isOutput=False)           # X[b].T
    wqdl = dp("wqdl", [D, 6 * P], bf16, isOutput=False)   # local qd W.T
    wkvd = dp("wkvd", [D, 5 * P], bf16, isOutput=False)   # kv down W.T (padded)
    wqu = dp("wqu", [QL, QL], bf16, isOutput=False)        # (perm q) Wqu_eff.T
    wkvu = dp("wkvu", [KVL, 2048], bf16, isOutput=False)   # (perm kv) Wkvu_eff.T
    wo = dp("wo", [HH * VH, D], bf16, isOutput=False)     # Wo[:, slice].T
    bql_i = dp("bql", [P, 6], f32, isOutput=False)        # local qd bias
    bkvd_i = dp("bkvd", [P, 5], f32, isOutput=False)      # kv down bias
    bqu_i = dp("bqu", [P, QC], f32, isOutput=False)       # perm + scale
    bkvuk = dp("bkvuk", [P, HH], f32, isOutput=False)     # kNope part
    cos2 = dp("cos2", [P, T], bf16, isOutput=False)       # duplicated rows
    sina = dp("sina", [P, T], bf16, isOutput=False)       # sign-folded sin
    tri_i = dp("tri", [P, P], bf16, isOutput=False)       # diagonal-band mask
    rotm_i = dp("rotm", [P, P], bf16, isOutput=False)     # rot-half permutation
    outt = dp("outt", [D, T], f32, isOutput=True)

    cc_in = nc.dram_tensor("cc_in", [6 * P, T], bf16)
    cc_out = nc.dram_tensor("cc_out", [12 * P, T], bf16)
    RG = [[0, 1], [2, 3], [4, 5], [6, 7]]

    xt_r = xt.rearrange("(c p) t -> p c t", p=P)
    wqdl_r = wqdl.rearrange("(c p) m -> p c m", p=P)
    wkvd_r = wkvd.rearrange("(c p) m -> p c m", p=P)
    wqu_r = wqu.rearrange("(c p) m -> p c m", p=P)
    wkvu_r = wkvu.rearrange("(c p) m -> p c m", p=P)
    wo_r = wo.rearrange("(c p) m -> p c m", p=P)
    outt_r = outt.rearrange("(c p) t -> p c t", p=P)
    cc_in_r = cc_in.rearrange("(c p) t -> p c t", p=P)
    cc_out_r = cc_out.rearrange("(c p) t -> p c t", p=P)

    with TileContext(nc) as tc, ExitStack() as stk:
        const = stk.enter_context(tc.tile_pool(name="const", bufs=1))
        persist = stk.enter_context(tc.tile_pool(name="persist", bufs=1))

        # ---- constants in SBUF (X goes first; see phase 1) ----
        c_bql = const.tile([P, 6], f32)
        c_bkvd = const.tile([P, 5], f32)
        c_bqu = const.tile([P, QC], f32)
        c_bkvuk = const.tile([P, HH], f32)
        c_tri = const.tile([P, P], bf16)
        c_rotm = const.tile([P, P], bf16)
        c_cos = const.tile([P, T], bf16)
        c_sin = const.tile([P, T], bf16)
        ones_bf = const.tile([P, P], bf16)
        nc.vector.memset(ones_bf[:], 1.0)
        eps_c = const.tile([P, 1], f32)
        nc.vector.memset(eps_c[:], EPS)

        # ---- persistent activations ----
        t_q = persist.tile([P, QC, T], bf16)      # q heads (nope 0-7, rope 8-11)
        t_kn = persist.tile([P, HH, T], bf16)     # kNope[feat, head, tok]
        t_v = persist.tile([P, T // P, HH * P], bf16)  # v[tok, tokchunk, hv]
        t_kr = persist.tile([P, T], bf16)         # kRot, rows duplicated
        t_ao = persist.tile([P, HH, T], bf16)     # attn out [vh, head, tok]
        rq = persist.tile([P, T], f32)            # q rms scale (per token)
        rkv = persist.tile([P, T], f32)           # kv rms scale

        # ====== phases 1+2: projections ======
        with tc.tile_pool(name="ph1", bufs=1) as ph1, \
             tc.tile_pool(name="wstream", bufs=2) as wst, \
             tc.tile_pool(name="wqu_p", bufs=2) as wqp, \
             tc.tile_pool(name="wkvu_p", bufs=2) as wkp, \
             tc.tile_pool(name="tmp", bufs=2) as tmp, \
             tc.tile_pool(name="psA", bufs=6, space="PSUM") as psA, \
             tc.tile_pool(name="psR", bufs=1, space="PSUM") as psR:

            # first two qd weights lead, then X split over sync/scalar with
            # only 1MB on gpsimd: the gpsimd queue must stay light so the
            # collective (stores -> AllGather -> readback) runs early
            t_x = ph1.tile([P, DC, T], bf16, name="t_x")
            w_qd = []
            for m in range(6):
                w_qd.append(wst.tile([P, DC, P], bf16, tag="wqd", bufs=5,
                                     name="w_qd"))
            nc.scalar.dma_start(w_qd[0][:], wqdl_r[:, :, bass.ts(0, P)])
            nc.sync.dma_start(w_qd[1][:], wqdl_r[:, :, bass.ts(1, P)])
            nc.sync.dma_start(t_x[:, 0:6, :], xt_r[:, 0:6, :])
            nc.scalar.dma_start(t_x[:, 6:12, :], xt_r[:, 6:12, :])
            nc.gpsimd.dma_start(t_x[:, 12:16, :], xt_r[:, 12:16, :])
            nc.scalar.dma_start(w_qd[2][:], wqdl_r[:, :, bass.ts(2, P)])
            nc.sync.dma_start(w_qd[3][:], wqdl_r[:, :, bass.ts(3, P)])
            nc.scalar.dma_start(w_qd[4][:], wqdl_r[:, :, bass.ts(4, P)])
            nc.gpsimd.dma_start(c_bql[:], bql_i[:])
            nc.gpsimd.dma_start(c_bkvd[:], bkvd_i[:])
            nc.gpsimd.dma_start(c_bqu[:], bqu_i[:])
            nc.gpsimd.dma_start(c_bkvuk[:], bkvuk[:])
            nc.gpsimd.dma_start(c_tri[:], tri_i[:])
            nc.gpsimd.dma_start(c_rotm[:], rotm_i[:])
            t_kv = ph1.tile([P, 5, T], bf16, name="t_kv")
            t_qd = ph1.tile([P, QC, T], bf16, name="t_qd")

            def down_chain(wt, m_rows, bias_t, bcol, out_ap):
                # out[m_rows, T] = wt.T @ X + bias, as 2 half-token chains
                for tt in range(2):
                    ps = psA.tile([P, 512], f32, tag="ev", name="ps_ev")
                    psm = ps[:m_rows, :]
                    for c in range(DC):
                        nc.tensor.matmul(
                            psm, wt[:, c, :m_rows],
                            t_x[:, c, bass.ts(tt, 512)],
                            start=(c == 0), stop=(c == DC - 1),
                        )
                    nc.vector.tensor_scalar_add(
                        out=out_ap[:m_rows, bass.ts(tt, 512)], in0=psm,
                        scalar1=bias_t[:m_rows, bcol:bcol + 1])

            # ---- q down: local 6 chunks -> exchange -> full 12 in t_qd ----
            # The AllGather output is in global QL order [g0 | g1], so
            # overwriting ALL of t_qd with cc_out leaves every core with the
            # naturally-ordered full latent regardless of its group. The
            # gpsimd queue serializes stores -> collective -> readback.
            for m in range(6):
                if m >= 5:
                    eng = nc.scalar if m % 2 == 0 else nc.sync
                    eng.dma_start(w_qd[m][:], wqdl_r[:, :, bass.ts(m, P)])
                down_chain(w_qd[m], P, c_bql, m, t_qd[:, m, :])
                nc.gpsimd.dma_start(cc_in_r[:, m, :], t_qd[:, m, :])
            nc.gpsimd.collective_compute(
                "AllGather", mybir.AluOpType.bypass,
                replica_groups=RG,
                ins=[cc_in[:]], outs=[cc_out[:]],
            )
            nc.gpsimd.dma_start(t_qd[:, 0:6, :], cc_out_r[:, 0:6, :])
            nc.gpsimd.dma_start(t_qd[:, 6:12, :], cc_out_r[:, 6:12, :])

            # ---- kv down (c 0..3 latent, then rope chunk last) ----
            # the rope-chunk weight is fetched early on sync (own slot) so
            # the rope matmuls never wait behind the rms activations
            wt5 = wst.tile([P, DC, ROPE], bf16, tag="wkv5", bufs=1,
                           name="wt5")
            nc.sync.dma_start(wt5[:, :, :ROPE],
                              wkvd_r[:, :, bass.ds(512, ROPE)])
            for m in range(4):
                wt = wst.tile([P, DC, P], bf16, tag="wqd", bufs=5)
                eng = nc.scalar if m % 2 == 0 else nc.sync
                eng.dma_start(wt[:], wkvd_r[:, :, bass.ts(m, P)])
                down_chain(wt, P, c_bkvd, m, t_kv[:, m, :])
            nc.scalar.dma_start(c_cos[:], cos2[:])
            nc.scalar.dma_start(c_sin[:], sina[:])
            # kv rms (chunks 0..3) -- the sqrt + recip + scale latency chain
            # hides under the rope-chunk matmuls that follow
            ps_ms = psR.tile([P, 2, 512], f32, tag="ms", name="ps_ms")
            for tt in range(2):
                for c in range(KC):
                    sq = tmp.tile([P, 512], bf16, tag="sq")
                    nc.vector.tensor_mul(
                        sq[:], t_kv[:, c, bass.ts(tt, 512)],
                        t_kv[:, c, bass.ts(tt, 512)])
                    nc.tensor.matmul(
                        ps_ms[:, tt, :], ones_bf[:], sq[:],
                        start=(c == 0), stop=(c == KC - 1),
                    )
            # rsqrt(mean+eps): scalar Sqrt then custom-DVE fast reciprocal
            # (no Ln: it lives in a different act table set than Exp and
            # would thrash the table loads)
            with tc.high_priority():
                for tt in range(2):
                    h = bass.ts(tt, 512)
                    nc.scalar.activation(rkv[:, h], ps_ms[:, tt, :],
                                         AF.Sqrt, bias=eps_c[:],
                                         scale=1.0 / KVL)
                    nc.vector.reciprocal_approx_fast(out=rkv[:, h],
                                                     in_=rkv[:, h])
                    for c in range(KC):
                        nc.vector.tensor_mul(
                            t_kv[:, c, h], t_kv[:, c, h], rkv[:, h])
            # rope chunk of kv-down (weight prefetched above)
            down_chain(wt5, ROPE, c_bkvd, 4, t_kv[:, 4, :])
            # RoPE on k (rows duplicated to 128 for the two packed heads);
            # swaps on the sync queue so the DVE is not left waiting
            swp = tmp.tile([P, T], bf16, tag="swp", name="swp",
                           bufs=1)[:ROPE, :]
            nc.sync.dma_start(swp[0:32, :], t_kv[32:64, 4, :])
            nc.sync.dma_start(swp[32:64, :], t_kv[0:32, 4, :])
            nc.vector.tensor_mul(t_kr[0:ROPE, :], t_kv[0:ROPE, 4, :],
                                 c_cos[0:ROPE, :])
            nc.vector.tensor_mul(swp[:], swp[:], c_sin[0:ROPE, :])
            nc.vector.tensor_add(t_kr[0:ROPE, :], t_kr[0:ROPE, :], swp[:])
            nc.sync.dma_start(t_kr[ROPE:P, :], t_kr[0:ROPE, :])

            # ---- kNope up-projection (bias add on scalar engine) ----
            # weights all on sync: the scalar queue head-blocks on the rms
            # activations and would delay them
            kn_w = []
            for m in range(HH):
                wt = wkp.tile([P, KC, P], bf16, tag="wkn", bufs=8,
                              name="kn_w")
                nc.sync.dma_start(wt[:], wkvu_r[:, :, bass.ts(m, P)])
                kn_w.append(wt)
            for m in range(HH):
                wt = kn_w[m]
                for tt in range(2):
                    ps = psA.tile([P, 512], f32, tag="ev", name="ps_kn")
                    for c in range(KC):
                        nc.tensor.matmul(
                            ps, wt[:, c, :],
                            t_kv[:, c, bass.ts(tt, 512)],
                            start=(c == 0), stop=(c == KC - 1),
                        )
                    nc.scalar.activation(
                        t_kn[:, m, bass.ts(tt, 512)], ps, AF.Identity,
                        bias=c_bkvuk[:, m:m + 1])

            # ---- v up-projection (token-on-partition; copies on scalar) ----
            # runs before q-rms so the collective readback of t_qd finishes
            # in its shadow
            for gg in range(2):
                wt = wkp.tile([P, KC, 512], bf16, tag="wv")
                (nc.sync if gg == 0 else nc.scalar).dma_start(
                    wt[:], wkvu_r[:, :, bass.ds(1024 + gg * 512, 512)])
                for tcb in range(8):
                    ps = psA.tile([P, 512], f32, tag="ev", name="ps_v")
                    for c in range(KC):
                        nc.tensor.matmul(
                            ps,
                            t_kv[:, c, bass.ts(tcb, P)],
                            wt[:, c, :],
                            start=(c == 0), stop=(c == KC - 1),
                        )
                    nc.scalar.activation(
                        t_v[:, tcb, bass.ts(gg, 512)], ps, AF.Copy)

            # ---- q rms (t_qd now holds the full gathered latent) ----
            # pinned late on the scheduler's model clock: its DMA/collective
            # model is optimistic, and without the pin it hoists these ops
            # ahead of kv/kNope/v work, head-blocking whichever engine they
            # sit on until the readback really lands
            ps_mq = psR.tile([P, 2, 512], f32, tag="ms", name="ps_mq")
            with tc.tile_wait_until(0.120):
                for tt in range(2):
                    hs = bass.ts(tt, 512)
                    for c in range(QC):
                        sq = tmp.tile([P, 512], bf16, tag="sq")
                        nc.vector.tensor_mul(
                            sq[:], t_qd[:, c, hs], t_qd[:, c, hs])
                        nc.tensor.matmul(
                            ps_mq[:, tt, :], ones_bf[:], sq[:],
                            start=(c == 0), stop=(c == QC - 1),
                        )
                    nc.scalar.activation(rq[:, hs], ps_mq[:, tt, :],
                                         AF.Sqrt, bias=eps_c[:],
                                         scale=1.0 / QL)
                    nc.vector.reciprocal_approx_fast(out=rq[:, hs],
                                                     in_=rq[:, hs])
            # dummy exp: pulls the exp act-table load into the qu window
            wrm = tmp.tile([P, 1], f32, tag="wrm", bufs=1, name="wrm")
            nc.scalar.activation(wrm[:], eps_c[:], AF.Exp)

            # ---- q up-projection ----
            # post-processing of chunk m's psums is issued after chunk m+1's
            # matmul chains, so the PE never waits on the DVE stage tiles
            def qu_post(m, ps, tt):
                tsl = bass.ts(tt, 512)
                if m < 8:
                    qsb = tmp.tile([P, 512], bf16, tag="qsb", bufs=2)
                    nc.vector.tensor_mul(qsb[:], ps, rq[:, tsl])
                    nc.scalar.activation(
                        t_q[:, m, tsl], qsb, AF.Identity,
                        bias=c_bqu[:, m:m + 1],
                    )
                else:
                    sq = tmp.tile([P, 512], bf16, tag="ropestage",
                                  bufs=2)
                    nc.vector.tensor_mul(sq[:], ps, rq[:, tsl])
                    nc.vector.tensor_scalar_add(
                        out=sq[:], in0=sq, scalar1=c_bqu[:, m:m + 1],
                    )
                    # rotate-half via a PE permutation matmul
                    swq = psA.tile([P, 512], f32, tag="ev", name="ps_rot")
                    nc.tensor.matmul(swq[:], c_rotm[:], sq[:],
                                     start=True, stop=True)
                    nc.vector.tensor_mul(sq[:], sq[:], c_cos[:, tsl])
                    swb = tmp.tile([P, 512], bf16, tag="ropeswap",
                                   bufs=2)
                    nc.vector.tensor_mul(swb[:], swq[:], c_sin[:, tsl])
                    nc.vector.tensor_add(t_q[:, m, tsl], sq[:], swb[:])

            pend = None
            for m in (8, 0, 1, 9, 2, 3, 10, 4, 5, 11, 6, 7):
                wt = wqp.tile([P, QC, P], bf16, tag="wqu")
                eng = nc.scalar if m % 2 == 0 else nc.sync
                eng.dma_start(wt[:], wqu_r[:, :, bass.ts(m, P)])
                cur = []
                for tt in range(2):
                    tsl = bass.ts(tt, 512)
                    ps = psA.tile([P, 512], f32, tag="ev", name="ps_qu")
                    for c in range(QC):
                        nc.tensor.matmul(
                            ps, wt[:, c, :], t_qd[:, c, tsl],
                            start=(c == 0), stop=(c == QC - 1),
                        )
                    cur.append(ps)
                if pend is not None:
                    pm, pps = pend
                    for tt in range(2):
                        qu_post(pm, pps[tt], tt)
                pend = (m, cur)
            pm, pps = pend
            for tt in range(2):
                qu_post(pm, pps[tt], tt)

        # ====== phase 3: attention (transposed scores, max-free) ======
        def vis_kcs(qt):
            return [kc for kc in range(8)
                    if qt * 512 + 511 >= kc * P - start]

        with tc.tile_pool(name="att", bufs=2) as att, \
             tc.tile_pool(name="psS", bufs=2, space="PSUM") as psS, \
             tc.tile_pool(name="psD", bufs=1, space="PSUM") as psD, \
             tc.tile_pool(name="psU", bufs=2, space="PSUM") as psU:

            def scores_qt(hp, expts2, qt):
                # expts2 [P, head2, kc, q] for heads (2hp, 2hp+1)
                rc = 8 + hp
                for kc in vis_kcs(qt):
                    lo = max(qt * 512, kc * P - start)
                    w = qt * 512 + 512 - lo
                    rel = lo - qt * 512
                    sc2 = psS.tile([P, 2, 512], f32, tag="sc", name="sc2")
                    for h2 in range(2):
                        h = 2 * hp + h2
                        nc.tensor.matmul(
                            sc2[:, h2, rel:],
                            t_kn[:, h, bass.ts(kc, P)],
                            t_q[:, h, bass.ds(lo, w)],
                            start=True, stop=False,
                        )
                    for h2 in range(2):
                        r0 = h2 * ROPE
                        nc.tensor.matmul(
                            sc2[:, h2, rel:],
                            t_kr[r0:r0 + ROPE, bass.ts(kc, P)],
                            t_q[r0:r0 + ROPE, rc, bass.ds(lo, w)],
                            start=False, stop=True,
                        )
                    # partially-masked diagonal band
                    b_lo = max(lo, kc * P - start)
                    b_hi = min(qt * 512 + 512, kc * P - start + P)
                    bw = b_hi - b_lo
                    if bw > 0:
                        j0 = b_lo - (kc * P - start)
                        br = b_lo - qt * 512
                        for h2 in range(2):
                            nc.vector.tensor_add(
                                sc2[:, h2, br:br + bw],
                                sc2[:, h2, br:br + bw],
                                c_tri[:, j0:j0 + bw])
                    nc.scalar.activation(
                        expts2[:, :, kc, bass.ds(lo, w)],
                        sc2[:, :, rel:], AF.Exp)

            def den_outU_head(hp, expts2, h2):
                h = 2 * hp + h2
                den2 = psD.tile([P, 2, 512], f32, name="den2")
                for qt in range(2):
                    kcs = vis_kcs(qt)
                    for i, kc in enumerate(kcs):
                        lo = max(qt * 512, kc * P - start)
                        rel = lo - qt * 512
                        nc.tensor.matmul(
                            den2[:, qt, rel:], ones_bf[:],
                            expts2[:, h2, kc, bass.ds(lo, 512 - rel)],
                            start=(i == 0), stop=(i == len(kcs) - 1),
                        )
                rcp = att.tile([P, 2, 512], f32, tag="rcp", name="rcp")
                nc.vector.reciprocal_approx_fast(
                    out=rcp[:, :, :], in_=den2[:, :, :])
                for qt in range(2):
                    kcs = vis_kcs(qt)
                    outU = psU.tile([P, 512], f32, tag="outU", name="outU")
                    for i, kc in enumerate(kcs):
                        lo = max(qt * 512, kc * P - start)
                        rel = lo - qt * 512
                        nc.tensor.matmul(
                            outU[:, rel:], t_v[:, kc, bass.ts(h, P)],
                            expts2[:, h2, kc, bass.ds(lo, 512 - rel)],
                            start=(i == 0), stop=(i == len(kcs) - 1),
                        )
                    nc.vector.tensor_mul(
                        t_ao[:, h, bass.ts(qt, 512)], outU[:],
                        rcp[:, qt, :])

            # scores(hp) interleave with den/outU of hp-1 at qt granularity
            prev = None
            for hp in range(4):
                cur = att.tile([P, 2, 8, T], bf16, tag="expt", name="expt2")
                scores_qt(hp, cur, 0)
                if prev is not None:
                    den_outU_head(hp - 1, prev, 0)
                scores_qt(hp, cur, 1)
                if prev is not None:
                    den_outU_head(hp - 1, prev, 1)
                prev = cur
            den_outU_head(3, prev, 0)
            den_outU_head(3, prev, 1)

            # ====== phase 4: output projection ======
            for m in range(DC):
                wt = att.tile([P, HH, P], bf16, tag="wo", name="wo_t",
                              bufs=4)
                eng = nc.gpsimd if m % 2 == 0 else nc.sync
                eng.dma_start(wt[:], wo_r[:, :, bass.ts(m, P)])
                for tt in range(2):
                    ps = psU.tile([P, 512], f32, tag="outU", name="ps_o")
                    for c in range(HH):
                        nc.tensor.matmul(
                            ps, wt[:, c, :], t_ao[:, c, bass.ts(tt, 512)],
                            start=(c == 0), stop=(c == HH - 1),
                        )
                    ot = att.tile([P, 512], f32, tag="ot", name="ot",
                                  bufs=3)
                    nc.vector.tensor_copy(ot[:], ps)
                    nc.sync.dma_start(outt_r[:, m, bass.ts(tt, 512)], ot[:])

    nc.compile()
    return nc


_CACHE = {}


def _get_nc(start: int):
    if start not in _CACHE:
        _CACHE[start] = build_nc(start)
    return _CACHE[start]


def _prep_inputs(X, base_freq, Wqd, bqd, gq, Wqu, bqu, Wkv, bkv, gkv,
                 Wkvu, bkvu, Wo, bo, start):
    f = np.float32
    X = np.asarray(X, f)
    base_freq = np.asarray(base_freq, f)
    Wqd = np.asarray(Wqd, f); bqd = np.asarray(bqd, f)
    gq = np.asarray(gq, f); Wqu = np.asarray(Wqu, f); bqu = np.asarray(bqu, f)
    Wkv = np.asarray(Wkv, f); bkv = np.asarray(bkv, f)
    gkv = np.asarray(gkv, f); Wkvu = np.asarray(Wkvu, f)
    bkvu = np.asarray(bkvu, f)
    Wo = np.asarray(Wo, f); bo = np.asarray(bo, f)
    start = int(np.asarray(start).item())
    assert start >= 0

    scale = QKH ** (-0.5)
    bf = ml_dtypes.bfloat16

    # v-bias exact fold: probs sum to 1, so the v bias contributes
    # Wo @ bv to every token's output.
    bv = bkvu.reshape(H, NOPE + VH)[:, NOPE:].reshape(H * VH)
    bo_eff = bo + Wo @ bv

    # qd down W split 6/6 across the TP pair; kv down duplicated
    wqd_t = Wqd.T.astype(f)                                   # (D, QL)
    wkv_t = Wkv.T.astype(f)                                   # (D, NKV)
    wqdl, bql = [], []
    for g in range(2):
        wqdl.append(np.ascontiguousarray(
            wqd_t[:, g * 768:(g + 1) * 768]).astype(bf))
        bql.append(np.ascontiguousarray(
            bqd[g * 768:(g + 1) * 768].reshape(6, P).T))
    wkvd = np.concatenate([wkv_t[:, :576], np.zeros((D, 64), f)], 1)
    wkvd = np.ascontiguousarray(wkvd).astype(bf)
    bkvd_p = np.zeros((5 * P,), f); bkvd_p[:NKV] = bkv
    bkvd = np.ascontiguousarray(bkvd_p.reshape(5, P).T)

    ang = base_freq[:S]                                       # (S, ROPE)
    cos = np.ascontiguousarray(np.cos(ang).T.astype(f))       # (ROPE, S)
    sin = np.ascontiguousarray(np.sin(ang).T.astype(f))
    cos2 = np.ascontiguousarray(
        np.concatenate([cos, cos], 0)).astype(bf)             # (128, S)
    sgn = np.ones((ROPE, 1), f); sgn[:ROPE // 2] = -1.0
    sins = sin * sgn                                          # sign-folded
    sina = np.ascontiguousarray(np.concatenate([sins, sins], 0)).astype(bf)
    # rot-half permutation (unsigned 32<->32 swap inside each 64 block;
    # the sign lives in the sign-folded sin table)
    rotm = np.zeros((P, P), f)
    for i in range(P):
        b, il = i // 64, i % 64
        rotm[b * 64 + (il + 32) % 64, i] = 1.0
    rotm = np.ascontiguousarray(rotm).astype(bf)

    # universal diagonal-band mask: for the block at k = kc*P + p,
    # q = (kc*P - start) + j, visibility is p <= j.
    pp = np.arange(P)
    tri = np.where(pp[:, None] <= pp[None, :], 0.0, NEG).astype(bf)
    tri = np.ascontiguousarray(tri)

    # per head-group tensors
    perm_q = np.concatenate(
        [np.arange(h * QKH, h * QKH + NOPE) for h in range(HH)]
        + [np.arange(h * QKH + NOPE, (h + 1) * QKH) for h in range(HH)]
    )
    perm_kv = np.concatenate(
        [np.arange(h * (NOPE + VH), h * (NOPE + VH) + NOPE) for h in range(HH)]
        + [np.arange(h * (NOPE + VH) + NOPE, (h + 1) * (NOPE + VH))
           for h in range(HH)]
    )
    gmaps = []
    for g in range(2):
        rq = slice(g * HH * QKH, (g + 1) * HH * QKH)
        rkv = slice(g * HH * (NOPE + VH), (g + 1) * HH * (NOPE + VH))
        wqu_g = (Wqu[rq, :] * gq[None, :] * scale)[perm_q]    # (1536, QL)
        bqu_g = (bqu[rq] * scale)[perm_q]
        wkvu_g = (Wkvu[rkv, :] * gkv[None, :])[perm_kv]       # (2048, KVL)
        bkvu_g = bkvu[rkv][perm_kv]
        wo_g = Wo[:, g * HH * VH:(g + 1) * HH * VH]           # (D, 1024)
        gmaps.append({
            "wqu": np.ascontiguousarray(wqu_g.T).astype(bf),
            "bqu": np.ascontiguousarray(bqu_g.reshape(QC, P).T),
            "wkvu": np.ascontiguousarray(wkvu_g.T).astype(bf),
            "bkvuk": np.ascontiguousarray(
                bkvu_g[:HH * NOPE].reshape(HH, P).T),
            "wo": np.ascontiguousarray(wo_g.T).astype(bf),    # (1024, D)
        })

    xts = [np.ascontiguousarray(X[b].T).astype(bf) for b in range(B)]

    in_maps = []
    for c in range(8):
        b, g = c // 2, c % 2
        m = {
            "xt": xts[b], "wqdl": wqdl[g], "bql": bql[g],
            "wkvd": wkvd, "bkvd": bkvd,
            "cos2": cos2, "sina": sina, "tri": tri, "rotm": rotm,
        }
        m.update(gmaps[g])
        in_maps.append(m)
    return in_maps, bo_eff, start


def kernel(**inputs) -> np.ndarray:
    in_maps, bo_eff, start = _prep_inputs(**inputs)
    nc = _get_nc(start)
    try:
        res = run_bass_kernel_spmd(nc, in_maps, core_ids=list(range(8)))
    except Exception:
        res = run_bass_kernel_spmd(nc, in_maps, core_ids=list(range(8)))
    out = np.empty((B, S, D), np.float32)
    for b in range(B):
        acc = res.results[2 * b]["outt"] + res.results[2 * b + 1]["outt"]
        out[b] = acc.T + bo_eff[None, :]
    return out
